# revision 2
# baseline (speedup 1.0000x reference)
"""Trainium2 Bass kernel: 16-head causal attention (B=4, S=2048, E=1024).

Sharding: 8 cores = 4 batches x 2 head-groups (8 heads each).

Per-core pipeline (all matmul operands fp16; PSUM accumulates fp32):
  - q^T = Wq_g X^T, k^T = Wk_g X^T    (transposed projections, [dq, S] f16)
  - V   = X^T.T Wv_g^T                (natural [S, dv] f16, +ones column per
                                       head so PV also yields denominators)
  - scores^T[k, q] at 128x128 causal granularity: fully-masked sub-blocks are
    skipped; the diagonal-crossing sub-block gets one extra [128,128] additive
    mask matmul (identity stationary, f16 mask moving, NEG=-60000).
  - P^T = exp(scores^T/8) on ACT -> f16 tiles (masked lanes underflow to 0)
  - PV: out[q,65] += P^T_block^T V_aug  (stationary = P^T [128,128], moving =
    V_aug [128,65] f16 -> full 128 output partitions, 65 rows/block)
  - normalize: DVE reciprocal of the denominator column + tensor_scalar_mul
  - attn [q, dq] f16 -> PE-transpose [dq, q] -> Wo matmul -> f16 partials
  - host sums the two head-group partials (fp32) and adds bo.
"""

import contextlib

import numpy as np

import bass_rust
import concourse.bass as bass
import concourse.mybir as mybir
import concourse.tile as tile

F32 = mybir.dt.float32
F16 = mybir.dt.float16
BF16 = mybir.dt.bfloat16
AF = mybir.ActivationFunctionType

B, S, E = 4, 2048, 1024
H, D = 16, 64
NCORES = 8
NGROUPS = 2            # head groups (tensor parallel)
HPC = H // NGROUPS     # heads per core
DQ = HPC * D           # per-core projection width = 512
NEG = -60000.0         # f16-representable; exp(NEG/8) == 0.0 in fp32

SK = 128               # k sub-block (partition dim of scores^T)
SQ = 512               # q window
GW = 1024              # exp group width (psum [128, GW])


def split_excess_waits(nc, maxw=1):
    """This container's walrus supports one sem wait per instruction;
    hoist extras onto same-engine nops just before the instruction."""
    n_new = 0
    for bb in nc.main_func.blocks:
        new_list = []
        changed = False
        for inst in list(bb.instructions):
            si = inst.sync_info
            waits = list(si.on_wait) if si and si.on_wait else []
            if len(waits) > maxw:
                changed = True
                extra, keep = waits[:-maxw], waits[-maxw:]
                for ci in range(0, len(extra), maxw):
                    nop = bass_rust.InstNoOp(
                        name=f"I-waitsplit-{n_new}", ins=[], outs=[]
                    )
                    n_new += 1
                    nop.engine = inst.engine
                    nop.sync_info = mybir.SyncInfo(
                        on_wait=extra[ci : ci + maxw], on_update=[]
                    )
                    new_list.append(nop)
                inst.sync_info = mybir.SyncInfo(
                    on_wait=keep,
                    on_update=list(si.on_update) if si.on_update else [],
                )
            new_list.append(inst)
        if changed:
            bb.instructions = new_list
    return n_new


def build_kernel(causal=True, split_waits=True, debug=False):
    s, e, hpc, d = S, E, HPC, D
    dq = hpc * d              # 512
    nec = e // 128            # 8 input-feature chunks
    ndq = dq // 128           # 4 projection partition chunks
    nwin = s // SQ            # 4 q windows
    nsc = s // 128            # 16 s chunks

    nc = bass.Bass()

    xq = nc.declare_dram_parameter("xq_t", [e, s], F16, isOutput=False)
    xk = nc.declare_dram_parameter("xk_t", [e, s], F16, isOutput=False)
    xv = nc.declare_dram_parameter("xv_t", [e, s], F16, isOutput=False)
    wqd = nc.declare_dram_parameter("wq_t", [e, dq], F16, isOutput=False)
    wkd = nc.declare_dram_parameter("wk_t", [e, dq], F16, isOutput=False)
    wvd = nc.declare_dram_parameter("wv_t", [e, dq], F16, isOutput=False)
    wod = nc.declare_dram_parameter("wo_t", [dq, e], F16, isOutput=False)
    # packed constants: [bq(4) | bk(4) | bv_b(512)] f32, [ident | crossmask] f16
    cfd = nc.declare_dram_parameter("consts_f32", [128, 2 * ndq + dq], F32,
                                    isOutput=False)
    chd = nc.declare_dram_parameter("consts_f16", [128, 256], F16,
                                    isOutput=False)
    out = nc.declare_dram_parameter("out", [s, e], F16, isOutput=True)
    if debug:
        dbg_q = nc.declare_dram_parameter("dbg_q", [dq, s], F16, isOutput=True)
        dbg_k = nc.declare_dram_parameter("dbg_k", [dq, s], F16, isOutput=True)
        dbg_v = nc.declare_dram_parameter(
            "dbg_v", [s, hpc * (d + 1)], BF16, isOutput=True
        )
        dbg_at = nc.declare_dram_parameter("dbg_at", [s, dq], F16, isOutput=True)
        dbg_pt = nc.declare_dram_parameter("dbg_pt", [128, 17408], BF16,
                                           isOutput=True)
        dbg_rc = nc.declare_dram_parameter("dbg_rc", [128, 16], F32,
                                           isOutput=True)
        dbg_off = [0]

    with tile.TileContext(nc) as tc, contextlib.ExitStack() as ctx:
        pers = ctx.enter_context(tc.tile_pool(name="pers", bufs=1))
        xpool = ctx.enter_context(tc.tile_pool(name="xp", bufs=3))
        ppool = ctx.enter_context(tc.tile_pool(name="ppl", bufs=3))
        atn = ctx.enter_context(tc.tile_pool(name="atn", bufs=2))
        att = ctx.enter_context(tc.tile_pool(name="att", bufs=2))
        nrm = ctx.enter_context(tc.tile_pool(name="nrm", bufs=4))
        opool = ctx.enter_context(tc.tile_pool(name="opl", bufs=3))
        pp = ctx.enter_context(tc.tile_pool(name="pp", bufs=2, space="PSUM"))
        sp = ctx.enter_context(tc.tile_pool(name="sp", bufs=2, space="PSUM"))
        vp = ctx.enter_context(tc.tile_pool(name="vp", bufs=2, space="PSUM"))

        # ---- persistent tensors ----
        cf_sb = pers.tile([128, 2 * ndq + dq], F32, name="cf_sb")
        ch_sb = pers.tile([128, 256], F16, name="ch_sb")
        bq_sb = cf_sb[:, 0:ndq]
        bk_sb = cf_sb[:, ndq : 2 * ndq]
        bv_sb = cf_sb[:, 2 * ndq : 2 * ndq + dq]
        id_sb = ch_sb[:, 0:128]
        mk_sb = ch_sb[:, 128:256]
        q_sb = [
            [pers.tile([128, SQ], F16, name=f"q_sb{c}_{w}") for w in range(nwin)]
            for c in range(ndq)
        ]
        k_sb = [
            [pers.tile([128, SQ], F16, name=f"k_sb{c}_{w}") for w in range(nwin)]
            for c in range(ndq)
        ]
        v_sb = [
            pers.tile([128, hpc * (d + 1)], BF16, name=f"v_sb{i}")
            for i in range(nsc)
        ]
        wq_sb = pers.tile([128, nec * dq], F16, name="wq_sb")
        wk_sb = pers.tile([128, nec * dq], F16, name="wk_sb")
        wv_sb = pers.tile([128, nec * dq], F16, name="wv_sb")
        wo_sb = pers.tile([128, ndq * e], F16, name="wo_sb")

        # ---- DMA helpers (SP engine -> one HWDGE queue, program order) ----
        def load_w_half(wt, dst, half):
            # rows [half*512, half*512+512) of [e, dq] -> dst cols
            src = wt.rearrange("(n p) m -> p n m", p=128)
            nc.sync.dma_start(
                out=dst.rearrange("p (n m) -> p n m", m=dq)[
                    :, half * 4 : (half + 1) * 4, :
                ],
                in_=src[:, half * 4 : (half + 1) * 4, :],
            )

        def load_x_slab(xt, dst, sb, half=None):
            # dst: [128, nec*512] tile; cols [sb*512,(sb+1)*512) of [e, s]
            src = xt.rearrange("(n p) m -> p n m", p=128)
            d3 = dst.rearrange("p (n m) -> p n m", m=SQ)
            if half is None:
                nc.sync.dma_start(
                    out=d3[:, :, :],
                    in_=src[:, :, sb * SQ : (sb + 1) * SQ],
                )
            else:
                nc.sync.dma_start(
                    out=d3[:, half * 4 : (half + 1) * 4, :],
                    in_=src[:, half * 4 : (half + 1) * 4, sb * SQ : (sb + 1) * SQ],
                )

        # small params first (packed: 2 DMAs)
        nc.sync.dma_start(out=cf_sb[:, :], in_=cfd[:, :])
        nc.sync.dma_start(out=ch_sb[:, :], in_=chd[:, :])

        x_t = {}  # (tensor, sb) -> slab tile
        for t, xd in (("q", xq), ("k", xk), ("v", xv)):
            x_t[t, 0] = xpool.tile([128, nec * SQ], F16, tag=f"x{t}",
                                   name=f"x{t}0", bufs=3)
        # slab 0 interleaved with weight halves for earliest unblock
        load_w_half(wqd, wq_sb, 0)
        load_x_slab(xq, x_t["q", 0], 0, half=0)
        load_w_half(wqd, wq_sb, 1)
        load_x_slab(xq, x_t["q", 0], 0, half=1)
        load_w_half(wkd, wk_sb, 0)
        load_x_slab(xk, x_t["k", 0], 0, half=0)
        load_w_half(wkd, wk_sb, 1)
        load_x_slab(xk, x_t["k", 0], 0, half=1)
        load_w_half(wvd, wv_sb, 0)
        load_x_slab(xv, x_t["v", 0], 0, half=0)
        load_w_half(wvd, wv_sb, 1)
        load_x_slab(xv, x_t["v", 0], 0, half=1)
        for sb in range(1, nwin):
            for t, xd in (("q", xq), ("k", xk), ("v", xv)):
                x_t[t, sb] = xpool.tile([128, nec * SQ], F16, tag=f"x{t}",
                                        name=f"x{t}{sb}", bufs=3)
                load_x_slab(xd, x_t[t, sb], sb)
            if sb == 1:
                nc.sync.dma_start(
                    out=wo_sb.rearrange("p (n m) -> p n m", m=e),
                    in_=wod.rearrange("(n p) m -> p n m", p=128),
                )

        # ones columns of v_sb, once, on the idle gpsimd engine
        for i in range(nsc):
            v3 = v_sb[i].rearrange("p (h t) -> p h t", t=d + 1)
            nc.gpsimd.memset(v3[:, :, d], 1.0)

        # ---- compute unit generators ----
        def w3(wt):
            return wt.rearrange("p (n m) -> p n m", m=dq)

        def proj_qk_unit(w_sb_t, xt, dst, bias, sb, c):
            """One [128,512] slab-column of a transposed projection."""
            ps = pp.tile([128, SQ], F32, tag="pp", name="ps_pj")
            wv_ = w3(w_sb_t)
            for ec in range(nec):
                nc.tensor.matmul(
                    ps[:, :],
                    wv_[:, ec, c * 128 : (c + 1) * 128],
                    x_t[xt, sb][:, ec * SQ : (ec + 1) * SQ],
                    start=(ec == 0),
                    stop=(ec == nec - 1),
                )
            nc.vector.tensor_scalar_add(
                dst[c][sb][:, :], ps[:, :], bias[:, c : c + 1]
            )

        def proj_v_unit(sb, ii):
            """One [128(s), dq] natural-layout V chunk (i = sb*4+ii)."""
            i = sb * 4 + ii
            ps = pp.tile([128, dq], F32, tag="pp", name="ps_v")
            wv_ = w3(wv_sb)
            for ec in range(nec):
                nc.tensor.matmul(
                    ps[:, :],
                    x_t["v", sb][:, ec * SQ + ii * 128 : ec * SQ + ii * 128 + 128],
                    wv_[:, ec, :],
                    start=(ec == 0),
                    stop=(ec == nec - 1),
                )
            v3 = v_sb[i].rearrange("p (h t) -> p h t", t=d + 1)
            nc.vector.tensor_add(
                v3[:, :, 0:d],
                ps[:, :].rearrange("p (h t) -> p h t", t=d),
                bv_sb[:, :].rearrange("p (h t) -> p h t", t=d),
            )

        # static PE/ACT occupancy estimate driving filler insertion
        eng_ns = {"pe": 0.0, "act": 0.0}

        def pe_rows(n):
            eng_ns["pe"] += n * 0.4167

        def act_cols(n):
            eng_ns["act"] += n * 0.8333 + 185.0

        def attention_head(qb, h, att_tiles):
            """scores+exp+PV+normalize for one (window, head).

            Generator: yields after each score-group / PV emission so the
            driver can interleave PE filler while ACT churns through exps.
            """
            c, hp = h // 2, (h % 2) * 64
            nkb = 4 * qb + 4 if causal else nsc
            # segments: (kb, qstart_global, width)
            segs = []
            for kb in range(nkb):
                if causal and kb >= 4 * qb:
                    qs = kb * 128
                else:
                    qs = qb * SQ
                segs.append((kb, qs, (qb + 1) * SQ - qs))
            # greedy-pack into exp groups of width <= GW
            groups, cur, curw = [], [], 0
            for seg in segs:
                if curw + seg[2] > GW:
                    groups.append(cur)
                    cur, curw = [], 0
                cur.append(seg)
                curw += seg[2]
            if cur:
                groups.append(cur)

            vpa = vp.tile([128, 4 * (d + 1)], F32, tag="vo", name="vpa")
            last_kb = nkb - 1

            def emit_scores(grp):
                gw = sum(g[2] for g in grp)
                scp = sp.tile([128, GW], F32, tag="sc", name="scp")
                off = 0
                for kb, qs, w in grp:
                    ks = k_sb[c][kb // 4][hp : hp + d,
                                          (kb % 4) * 128 : (kb % 4) * 128 + 128]
                    qw_ = q_sb[c][qs // SQ]
                    if causal and kb >= 4 * qb:
                        # additive mask for the diagonal-crossing sub-block
                        nc.tensor.matmul(scp[:, off : off + 128], id_sb[:, :],
                                         mk_sb[:, :], start=True, stop=False)
                        nc.tensor.matmul(
                            scp[:, off : off + 128], ks,
                            qw_[hp : hp + d, qs % SQ : qs % SQ + 128],
                            start=False, stop=True,
                        )
                        pe_rows(256)
                        if w > 128:
                            nc.tensor.matmul(
                                scp[:, off + 128 : off + w], ks,
                                qw_[hp : hp + d, qs % SQ + 128 : qs % SQ + w],
                                start=True, stop=True,
                            )
                            pe_rows(w - 128)
                    else:
                        nc.tensor.matmul(
                            scp[:, off : off + w], ks,
                            qw_[hp : hp + d, qs % SQ : qs % SQ + w],
                            start=True, stop=True,
                        )
                        pe_rows(w)
                    off += w
                pt = ppool.tile([128, GW], BF16, tag="pt", name="pt")
                nc.scalar.activation(
                    pt[:, 0:gw], scp[:, 0:gw], AF.Exp,
                    scale=float(1.0 / np.sqrt(d)),
                )
                act_cols(gw)
                if debug and h == 0:
                    nc.sync.dma_start(
                        out=dbg_pt[:, dbg_off[0] : dbg_off[0] + gw],
                        in_=pt[:, 0:gw])
                    dbg_off[0] += gw
                return pt

            def emit_pv(grp, pt):
                # One psum accumulation group for the whole vpa bank: a
                # start marks the full 2KB zero-region pending-zero, so only
                # the first matmul may carry start and only the last stop;
                # each sub-region auto-initializes on its first write.
                off = 0
                for kb, qs, w in grp:
                    for qcl in range(4):
                        qg = 4 * qb + qcl           # global q chunk
                        if causal and qg < kb:
                            continue                 # fully masked block
                        boff = off + qcl * 128 + qb * SQ - qs
                        nc.tensor.matmul(
                            vpa[:, qcl * (d + 1) : (qcl + 1) * (d + 1)],
                            pt[:, boff : boff + 128],
                            v_sb[kb][:, h * (d + 1) : (h + 1) * (d + 1)],
                            start=(kb == 0 and qcl == 0),
                            stop=(kb == last_kb and qcl == 3),
                        )
                        pe_rows(d + 1)
                    off += w

            # lag-1 software pipeline: scores g+1 overlaps exp g
            prev = None
            for grp in groups:
                pt = emit_scores(grp)
                yield
                if prev is not None:
                    emit_pv(*prev)
                    yield
                prev = (grp, pt)
            emit_pv(*prev)

            v4 = vpa.rearrange("p (qc t) -> p qc t", t=d + 1)
            rcp = nrm.tile([128, 4], F32, tag="rcp", name="rcp")
            nc.vector.reciprocal(rcp[:, :], v4[:, :, d])
            if debug and h == 0:
                nc.sync.dma_start(out=dbg_rc[:, qb * 4 : qb * 4 + 4],
                                  in_=rcp[:, :])
            for qcl in range(4):
                nc.vector.tensor_scalar_mul(
                    att_tiles[qcl][:, h * d : (h + 1) * d],
                    v4[:, qcl, 0:d],
                    rcp[:, qcl : qcl + 1],
                )

        def wo_transpose_unit(att_tiles, cc, at_store):
            """Transpose attn chunk cc (heads 2cc, 2cc+1) -> at_store[cc]."""
            tp = pp.tile([128, SQ], F16, tag="pp", name="tp")
            for qcl in range(4):
                nc.tensor.transpose(
                    tp[:, qcl * 128 : (qcl + 1) * 128],
                    att_tiles[qcl][:, cc * 128 : (cc + 1) * 128],
                    id_sb[:, :],
                )
                pe_rows(128)
            at_ = att.tile([128, SQ], F16, tag=f"at{cc}", name="at_")
            nc.vector.tensor_copy(at_[:, :], tp[:, :])
            at_store[cc] = at_

        def wo_matmul_unit(at_store, qb, i, copy_eng=None):
            """Output projection + store for s-chunk i of window qb."""
            wo3 = wo_sb.rearrange("p (n m) -> p n m", m=e)
            ot = opool.tile([128, e], F16, tag="ot", name="ot")
            si = qb * 4 + i
            for ob in range(2):
                ps = pp.tile([128, 512], F32, tag="pp", name="ps_o")
                for cc in range(ndq):
                    nc.tensor.matmul(
                        ps[:, :],
                        at_store[cc][:, i * 128 : (i + 1) * 128],
                        wo3[:, cc, ob * 512 : (ob + 1) * 512],
                        start=(cc == 0),
                        stop=(cc == ndq - 1),
                    )
                    pe_rows(512)
                if copy_eng is None:
                    nc.vector.tensor_copy(
                        ot[:, ob * 512 : (ob + 1) * 512], ps[:, :])
                else:
                    copy_eng.copy(ot[:, ob * 512 : (ob + 1) * 512], ps[:, :])
                nc.sync.dma_start(
                    out=out[si * 128 : (si + 1) * 128,
                            ob * 512 : (ob + 1) * 512],
                    in_=ot[:, ob * 512 : (ob + 1) * 512],
                )

        # ---- projection queue, deadline-ordered ----
        # Per window sb: q/k chunk c due just before head 2c; v slab due
        # during head 0's score groups (its diag PV needs it). Deadline key:
        # (sb, h_due) with v at h_due=1 (forced explicitly at h0's yields).
        proj_queue = []
        for sb in range(nwin):
            proj_queue.append((sb, 0, "q", sb, 0))
            proj_queue.append((sb, 0, "k", sb, 0))
            for ii in range(4):
                proj_queue.append((sb, 1, "v", sb, ii))
            for c in range(1, ndq):
                proj_queue.append((sb, 2 * c, "q", sb, c))
                proj_queue.append((sb, 2 * c, "k", sb, c))
        wo_queue = []

        def emit_proj_unit():
            _, _, kind, sb, j = proj_queue.pop(0)
            if kind == "q":
                proj_qk_unit(wq_sb, "q", q_sb, bq_sb, sb, j)
            elif kind == "k":
                proj_qk_unit(wk_sb, "k", k_sb, bk_sb, sb, j)
            else:
                proj_v_unit(sb, j)
            pe_rows(nec * SQ)

        def balance_filler():
            # keep PE fed while ACT is the pacing engine
            while eng_ns["pe"] < eng_ns["act"]:
                if proj_queue:
                    emit_proj_unit()
                elif wo_queue:
                    wo_queue.pop(0)()
                else:
                    return

        def force_due(qb, h):
            while proj_queue and (proj_queue[0][0], proj_queue[0][1]) <= (qb, h):
                emit_proj_unit()

        def wo_full(qb, att_tiles, last=False):
            at_store = [None] * ndq
            for cc in range(ndq):
                wo_transpose_unit(att_tiles, cc, at_store)
            if debug:
                for qcl in range(4):
                    nc.sync.dma_start(
                        out=dbg_at[(qb * 4 + qcl) * 128 :
                                   (qb * 4 + qcl + 1) * 128, :],
                        in_=att_tiles[qcl][:, :],
                    )
            for i in range(4):
                # final window: ACT is idle by now, DVE is not
                wo_matmul_unit(at_store, qb, i,
                               copy_eng=nc.scalar if last else None)

        # ---- emission ----
        # bootstrap: the startup is DMA-bound; emit the units whose inputs
        # arrive first (all of q0 + k0c0) so PE never out-runs the DMA stream
        boot = {("q", 0, 0), ("q", 0, 1), ("q", 0, 2), ("q", 0, 3), ("k", 0, 0)}
        for c in range(ndq):
            proj_qk_unit(wq_sb, "q", q_sb, bq_sb, 0, c)
        proj_qk_unit(wk_sb, "k", k_sb, bk_sb, 0, 0)
        proj_queue = [u for u in proj_queue if (u[2], u[3], u[4]) not in boot]

        prev = None  # deferred (qb, att_tiles) for wo
        for qb in range(nwin):
            att_tiles = [
                atn.tile([128, dq], F16, tag=f"an{qcl}", name=f"an{qcl}_{qb}")
                for qcl in range(4)
            ]
            for h in range(hpc):
                force_due(qb, h)
                yi = 0
                for _ in attention_head(qb, h, att_tiles):
                    yi += 1
                    if h == 0 and yi <= 2:
                        # v slab for this window's diagonal, 2 units per yield
                        for _ in range(2):
                            if proj_queue and proj_queue[0][2] == "v" \
                                    and proj_queue[0][3] == qb:
                                emit_proj_unit()
                    balance_filler()
                if prev is not None and h == 1:
                    wo_full(*prev)     # previous window's Wo inside this one
                    prev = None
            prev = (qb, att_tiles)
        while proj_queue:
            emit_proj_unit()
        wo_full(*prev, last=True)

        if debug:
            for c in range(ndq):
                for w in range(nwin):
                    cs = slice(c * 128, (c + 1) * 128)
                    ws = slice(w * SQ, (w + 1) * SQ)
                    nc.sync.dma_start(out=dbg_q[cs, ws], in_=q_sb[c][w][:, :])
                    nc.sync.dma_start(out=dbg_k[cs, ws], in_=k_sb[c][w][:, :])
            for i in range(nsc):
                nc.sync.dma_start(
                    out=dbg_v[i * 128 : (i + 1) * 128, :], in_=v_sb[i][:, :]
                )

    if split_waits:
        split_excess_waits(nc)
    return nc


def make_crossmask():
    kk = np.arange(128)[:, None]
    qq = np.arange(128)[None, :]
    return np.where(kk <= qq, 0.0, NEG).astype(np.float16)


def classify_mask(mask):
    m = np.asarray(mask).reshape(S, S)
    if np.array_equal(m, np.tril(np.ones((S, S), bool))):
        return "causal"
    if m.all():
        return "dense"
    return "generic"


def prep_core_inputs(query, key, value, Wq, bq, Wk, bk, Wv, bv, Wo, bo, mask):
    """Shard + lay out host-side numpy inputs for the 8 cores."""
    kind = classify_mask(mask)
    maps = []
    for core in range(NCORES):
        b, gi = core // NGROUPS, core % NGROUPS
        gs = slice(gi * DQ, (gi + 1) * DQ)
        im = {
            "xq_t": np.ascontiguousarray(
                np.asarray(query[b]).T.astype(np.float16)),
            "xk_t": np.ascontiguousarray(
                np.asarray(key[b]).T.astype(np.float16)),
            "xv_t": np.ascontiguousarray(
                np.asarray(value[b]).T.astype(np.float16)),
            "wq_t": np.ascontiguousarray(
                np.asarray(Wq)[gs, :].T.astype(np.float16)),
            "wk_t": np.ascontiguousarray(
                np.asarray(Wk)[gs, :].T.astype(np.float16)),
            "wv_t": np.ascontiguousarray(
                np.asarray(Wv)[gs, :].T.astype(np.float16)),
            "wo_t": np.ascontiguousarray(
                np.asarray(Wo)[:, gs].T.astype(np.float16)),
            "consts_f32": np.ascontiguousarray(np.concatenate([
                np.asarray(bq)[gs].astype(np.float32).reshape(-1, 128).T,
                np.asarray(bk)[gs].astype(np.float32).reshape(-1, 128).T,
                np.broadcast_to(
                    np.asarray(bv)[gs].astype(np.float32), (128, DQ)),
            ], axis=1)),
            "consts_f16": np.ascontiguousarray(np.concatenate([
                np.eye(128, dtype=np.float16), make_crossmask()
            ], axis=1)),
        }
        maps.append(im)
    return maps, kind


def make_runner(nc, n_cores=NCORES):
    """Build a reusable jitted SPMD executor for `nc` on cores 0..n_cores-1."""
    import jax
    from jax.experimental.shard_map import shard_map
    from jax.sharding import Mesh, PartitionSpec

    from concourse import bass2jax, mybir as _mybir

    bass2jax.install_neuronx_cc_hook()

    partition_name = (
        nc.partition_id_tensor.name if nc.partition_id_tensor else None
    )
    in_names, out_names, out_avals, zero_shapes = [], [], [], []
    for alloc in nc.m.functions[0].allocations:
        if not isinstance(alloc, _mybir.MemoryLocationSet):
            continue
        name = alloc.memorylocations[0].name
        if alloc.kind == "ExternalInput":
            if name != partition_name:
                in_names.append(name)
        elif alloc.kind == "ExternalOutput":
            out_names.append(name)
            shape = tuple(alloc.tensor_shape)
            dtype = _mybir.dt.np(alloc.dtype)
            out_avals.append(jax.core.ShapedArray(shape, dtype))
            zero_shapes.append((shape, dtype))
    n_params = len(in_names)
    all_in = list(in_names) + list(out_names)
    if partition_name is not None:
        all_in.append(partition_name)

    def _body(*args):
        operands = list(args)
        if partition_name is not None:
            operands.append(bass2jax.partition_id_tensor())
        outs = bass2jax._bass_exec_p.bind(
            *operands,
            out_avals=tuple(out_avals),
            in_names=tuple(all_in),
            out_names=tuple(out_names),
            lowering_input_output_aliases=(),
            sim_require_finite=True,
            sim_require_nnan=True,
            nc=nc,
        )
        return tuple(outs)

    devices = jax.devices()[:n_cores]
    assert len(devices) == n_cores
    mesh = Mesh(np.asarray(devices), ("core",))
    in_specs = (PartitionSpec("core"),) * (n_params + len(out_names))
    out_specs = (PartitionSpec("core"),) * len(out_names)
    sharded = jax.jit(
        shard_map(
            _body,
            mesh=mesh,
            in_specs=in_specs,
            out_specs=out_specs,
            check_rep=False,
        ),
        keep_unused=True,
    )
    zeros = [
        np.zeros((n_cores * sh[0], *sh[1:]), dt) for sh, dt in zero_shapes
    ]

    def concat_inputs(in_maps):
        return [
            np.concatenate(
                [np.asarray(in_maps[c][n]) for c in range(n_cores)], axis=0
            )
            for n in in_names
        ]

    def run(in_maps):
        out_arrs = sharded(*concat_inputs(in_maps), *zeros)
        return [
            {
                name: np.asarray(out_arrs[i]).reshape(
                    n_cores, *out_avals[i].shape
                )[c]
                for i, name in enumerate(out_names)
            }
            for c in range(n_cores)
        ]

    run.sharded = sharded
    run.concat_inputs = concat_inputs
    run.zeros = zeros
    run.out_names = out_names
    run.out_avals = out_avals
    return run


_CACHE = {}


def get_runner(kind="causal"):
    if kind not in _CACHE:
        nc = build_kernel(causal=(kind == "causal"))
        _CACHE[kind] = make_runner(nc)
    return _CACHE[kind]


def _numpy_reference(query, key, value, Wq, bq, Wk, bk, Wv, bv, Wo, bo, mask):
    q = (query @ Wq.T + bq).reshape(B, S, H, D).transpose(0, 2, 1, 3)
    k = (key @ Wk.T + bk).reshape(B, S, H, D).transpose(0, 2, 1, 3)
    v = (value @ Wv.T + bv).reshape(B, S, H, D).transpose(0, 2, 1, 3)
    sc = np.einsum("bhqd,bhkd->bhqk", q, k) / np.sqrt(D)
    sc = np.where(np.asarray(mask).reshape(1, 1, S, S), sc, -np.inf)
    sc -= sc.max(axis=-1, keepdims=True)
    p = np.exp(sc)
    p /= p.sum(axis=-1, keepdims=True)
    o = np.einsum("bhqk,bhkd->bhqd", p, v)
    o = o.transpose(0, 2, 1, 3).reshape(B, S, E)
    return o @ Wo.T + bo


def kernel(**inputs) -> np.ndarray:
    kind = classify_mask(inputs["mask"])
    if kind == "generic":
        fp = {k: np.asarray(v, np.float32) for k, v in inputs.items()
              if k != "mask"}
        return _numpy_reference(mask=inputs["mask"], **fp).astype(np.float32)
    in_maps, kind = prep_core_inputs(**inputs)
    run = get_runner(kind)
    results = run(in_maps)
    bo = np.asarray(inputs["bo"], dtype=np.float32)
    out = np.empty((B, S, E), dtype=np.float32)
    for b in range(B):
        acc = results[b * NGROUPS]["out"].astype(np.float32)
        for gi in range(1, NGROUPS):
            acc = acc + results[b * NGROUPS + gi]["out"].astype(np.float32)
        out[b] = acc + bo[None, :]
    return out


# revision 3
# speedup vs baseline: 1.0269x; 1.0269x over previous
"""Trainium2 Bass kernel: 16-head causal attention (B=4, S=2048, E=1024).

Sharding: 8 cores = 4 batches x 2 head-groups (8 heads each); host sums the
two head-group partials (fp32) and adds bo.

Per-core pipeline (fp16/bf16 matmul operands; PSUM accumulates fp32):
  - q^T = Wq_g X^T, k^T = Wk_g X^T    (transposed projections, [dq, S] f16)
  - V   = X^T.T Wv_g^T                (natural [S, dv] bf16, +ones column per
                                       head so PV also yields denominators)
  - scores^T[k, q] at 128x128 causal granularity: fully-masked sub-blocks are
    skipped; each diagonal-crossing sub-block gets one [128,128] additive mask
    matmul (identity stationary, f16 mask moving, NEG=-60000).
  - P^T = exp(scores^T/8) on ACT -> bf16 (range-safe: exp can reach ~1.3e8,
    which overflows f16; masked lanes underflow to exactly 0)
  - PV: out[q, 65] += P^T_block^T V_aug: stationary = P^T [128,128], moving =
    V_aug [128,65] bf16 -> full 128 output partitions at 65 rows/block. One
    PSUM accumulation group per vpa bank (single start/stop; sub-regions
    auto-initialize via the pending-zero mechanism).
  - normalize: DVE reciprocal of the denominator column + tensor_scalar_mul
  - attn [q, dq] f16 -> PE-transpose [dq, q] -> Wo matmul -> f16 partials
Scheduling: the emitter interleaves projection/output-projection work into the
ACT-bound attention windows (deadline queue + PE-vs-ACT balance heuristic),
batches DMAs into ~45 large transfers, and software-pipelines scores/exp/PV
with a lag of one exp group.
"""

import contextlib

import numpy as np

import bass_rust
import concourse.bass as bass
import concourse.mybir as mybir
import concourse.tile as tile

F32 = mybir.dt.float32
F16 = mybir.dt.float16
BF16 = mybir.dt.bfloat16
AF = mybir.ActivationFunctionType

B, S, E = 4, 2048, 1024
H, D = 16, 64
NCORES = 8
NGROUPS = 2            # head groups (tensor parallel)
HPC = H // NGROUPS     # heads per core
DQ = HPC * D           # per-core projection width = 512
NEG = -60000.0         # f16-representable; exp(NEG/8) == 0.0 in fp32

SK = 128               # k sub-block (partition dim of scores^T)
SQ = 512               # q window
GW = 1024              # exp group width (psum [128, GW])


def split_excess_waits(nc, maxw=1):
    """This container's walrus supports one sem wait per instruction;
    hoist extras onto same-engine nops just before the instruction."""
    n_new = 0
    for bb in nc.main_func.blocks:
        new_list = []
        changed = False
        for inst in list(bb.instructions):
            si = inst.sync_info
            waits = list(si.on_wait) if si and si.on_wait else []
            if len(waits) > maxw:
                changed = True
                extra, keep = waits[:-maxw], waits[-maxw:]
                for ci in range(0, len(extra), maxw):
                    nop = bass_rust.InstNoOp(
                        name=f"I-waitsplit-{n_new}", ins=[], outs=[]
                    )
                    n_new += 1
                    nop.engine = inst.engine
                    nop.sync_info = mybir.SyncInfo(
                        on_wait=extra[ci : ci + maxw], on_update=[]
                    )
                    new_list.append(nop)
                inst.sync_info = mybir.SyncInfo(
                    on_wait=keep,
                    on_update=list(si.on_update) if si.on_update else [],
                )
            new_list.append(inst)
        if changed:
            bb.instructions = new_list
    return n_new


def build_kernel(causal=True, split_waits=True, debug=False):
    s, e, hpc, d = S, E, HPC, D
    dq = hpc * d              # 512
    nec = e // 128            # 8 input-feature chunks
    ndq = dq // 128           # 4 projection partition chunks
    nwin = s // SQ            # 4 q windows
    nsc = s // 128            # 16 s chunks

    nc = bass.Bass()

    xq = nc.declare_dram_parameter("xq_t", [e, s], F16, isOutput=False)
    xk = nc.declare_dram_parameter("xk_t", [e, s], F16, isOutput=False)
    xv = nc.declare_dram_parameter("xv_t", [e, s], F16, isOutput=False)
    wqd = nc.declare_dram_parameter("wq_t", [e, dq], F16, isOutput=False)
    wkd = nc.declare_dram_parameter("wk_t", [e, dq], F16, isOutput=False)
    wvd = nc.declare_dram_parameter("wv_t", [e, dq], F16, isOutput=False)
    wod = nc.declare_dram_parameter("wo_t", [dq, e], F16, isOutput=False)
    # packed constants: [bq(4) | bk(4) | bv_b(512)] f32, [ident | crossmask] f16
    cfd = nc.declare_dram_parameter("consts_f32", [128, 2 * ndq + dq], F32,
                                    isOutput=False)
    chd = nc.declare_dram_parameter("consts_f16", [128, 256], F16,
                                    isOutput=False)
    out = nc.declare_dram_parameter("out", [s, e], F16, isOutput=True)
    if debug:
        dbg_q = nc.declare_dram_parameter("dbg_q", [dq, s], F16, isOutput=True)
        dbg_k = nc.declare_dram_parameter("dbg_k", [dq, s], F16, isOutput=True)
        dbg_v = nc.declare_dram_parameter(
            "dbg_v", [s, hpc * (d + 1)], BF16, isOutput=True
        )
        dbg_at = nc.declare_dram_parameter("dbg_at", [s, dq], F16, isOutput=True)
        dbg_pt = nc.declare_dram_parameter("dbg_pt", [128, 17408], BF16,
                                           isOutput=True)
        dbg_rc = nc.declare_dram_parameter("dbg_rc", [128, 16], F32,
                                           isOutput=True)
        dbg_off = [0]

    with tile.TileContext(nc) as tc, contextlib.ExitStack() as ctx:
        pers = ctx.enter_context(tc.tile_pool(name="pers", bufs=1))
        xpool = ctx.enter_context(tc.tile_pool(name="xp", bufs=3))
        ppool = ctx.enter_context(tc.tile_pool(name="ppl", bufs=4))
        atn = ctx.enter_context(tc.tile_pool(name="atn", bufs=2))
        att = ctx.enter_context(tc.tile_pool(name="att", bufs=2))
        nrm = ctx.enter_context(tc.tile_pool(name="nrm", bufs=4))
        opool = ctx.enter_context(tc.tile_pool(name="opl", bufs=3))
        pp = ctx.enter_context(tc.tile_pool(name="pp", bufs=2, space="PSUM"))
        sp = ctx.enter_context(tc.tile_pool(name="sp", bufs=2, space="PSUM"))
        vp = ctx.enter_context(tc.tile_pool(name="vp", bufs=2, space="PSUM"))

        # ---- persistent tensors ----
        cf_sb = pers.tile([128, 2 * ndq + dq], F32, name="cf_sb")
        ch_sb = pers.tile([128, 256], F16, name="ch_sb")
        bq_sb = cf_sb[:, 0:ndq]
        bk_sb = cf_sb[:, ndq : 2 * ndq]
        bv_sb = cf_sb[:, 2 * ndq : 2 * ndq + dq]
        id_sb = ch_sb[:, 0:128]
        mk_sb = ch_sb[:, 128:256]
        q_sb = [
            [pers.tile([128, SQ], F16, name=f"q_sb{c}_{w}") for w in range(nwin)]
            for c in range(ndq)
        ]
        k_sb = [
            [pers.tile([128, SQ], F16, name=f"k_sb{c}_{w}") for w in range(nwin)]
            for c in range(ndq)
        ]
        v_sb = [
            pers.tile([128, hpc * (d + 1)], BF16, name=f"v_sb{i}")
            for i in range(nsc)
        ]
        wq_sb = pers.tile([128, nec * dq], F16, name="wq_sb")
        wk_sb = pers.tile([128, nec * dq], F16, name="wk_sb")
        wv_sb = pers.tile([128, nec * dq], F16, name="wv_sb")
        wo_sb = pers.tile([128, ndq * e], F16, name="wo_sb")

        # ---- DMA helpers (SP engine -> one HWDGE queue, program order) ----
        def load_w_part(wt, dst, part, nparts=2):
            # e-chunk group `part` of [e, dq] -> dst cols
            g = nec // nparts
            src = wt.rearrange("(n p) m -> p n m", p=128)
            nc.sync.dma_start(
                out=dst.rearrange("p (n m) -> p n m", m=dq)[
                    :, part * g : (part + 1) * g, :
                ],
                in_=src[:, part * g : (part + 1) * g, :],
            )

        def load_x_slab(xt, dst, sb, part=None, nparts=2):
            # dst: [128, nec*512] tile; cols [sb*512,(sb+1)*512) of [e, s]
            src = xt.rearrange("(n p) m -> p n m", p=128)
            d3 = dst.rearrange("p (n m) -> p n m", m=SQ)
            if part is None:
                nc.sync.dma_start(
                    out=d3[:, :, :],
                    in_=src[:, :, sb * SQ : (sb + 1) * SQ],
                )
            else:
                g = nec // nparts
                nc.sync.dma_start(
                    out=d3[:, part * g : (part + 1) * g, :],
                    in_=src[:, part * g : (part + 1) * g,
                            sb * SQ : (sb + 1) * SQ],
                )



        x_t = {}  # (tensor, sb) -> slab tile
        for t, xd in (("q", xq), ("k", xk), ("v", xv)):
            x_t[t, 0] = xpool.tile([128, nec * SQ], F16, tag=f"x{t}",
                                   name=f"x{t}0", bufs=3)
        # slab 0 interleaved with weight pieces for earliest unblock;
        # wq/xq0 in quarters so the first projection matmuls start ASAP
        for part in range(4):
            load_w_part(wqd, wq_sb, part, nparts=4)
            load_x_slab(xq, x_t["q", 0], 0, part=part, nparts=4)
        # packed constants (biases for the first bias-add, mask for h0 scores)
        nc.sync.dma_start(out=cf_sb[:, :], in_=cfd[:, :])
        nc.sync.dma_start(out=ch_sb[:, :], in_=chd[:, :])
        load_w_part(wkd, wk_sb, 0)
        load_x_slab(xk, x_t["k", 0], 0, part=0)
        load_w_part(wkd, wk_sb, 1)
        load_x_slab(xk, x_t["k", 0], 0, part=1)
        load_w_part(wvd, wv_sb, 0)
        load_x_slab(xv, x_t["v", 0], 0, part=0)
        load_w_part(wvd, wv_sb, 1)
        load_x_slab(xv, x_t["v", 0], 0, part=1)
        x_t["q", 1] = xpool.tile([128, nec * SQ], F16, tag="xq",
                                 name="xq1", bufs=3)
        load_x_slab(xq, x_t["q", 1], 1)
        for sb in range(1, nwin):
            for t, xd in (("q", xq), ("k", xk), ("v", xv)):
                if (t, sb) in x_t:
                    continue
                x_t[t, sb] = xpool.tile([128, nec * SQ], F16, tag=f"x{t}",
                                        name=f"x{t}{sb}", bufs=3)
                load_x_slab(xd, x_t[t, sb], sb)
            if sb == 1:
                nc.sync.dma_start(
                    out=wo_sb.rearrange("p (n m) -> p n m", m=e),
                    in_=wod.rearrange("(n p) m -> p n m", p=128),
                )

        # ones columns of v_sb, once, on the idle gpsimd engine
        for i in range(nsc):
            v3 = v_sb[i].rearrange("p (h t) -> p h t", t=d + 1)
            nc.gpsimd.memset(v3[:, :, d], 1.0)

        # ---- compute unit generators ----
        def w3(wt):
            return wt.rearrange("p (n m) -> p n m", m=dq)

        def proj_qk_unit(w_sb_t, xt, dst, bias, sb, c):
            """One [128,512] slab-column of a transposed projection."""
            ps = pp.tile([128, SQ], F32, tag="pp", name="ps_pj")
            wv_ = w3(w_sb_t)
            for ec in range(nec):
                nc.tensor.matmul(
                    ps[:, :],
                    wv_[:, ec, c * 128 : (c + 1) * 128],
                    x_t[xt, sb][:, ec * SQ : (ec + 1) * SQ],
                    start=(ec == 0),
                    stop=(ec == nec - 1),
                )
            nc.vector.tensor_scalar_add(
                dst[c][sb][:, :], ps[:, :], bias[:, c : c + 1]
            )

        def proj_v_unit(sb, ii):
            """One [128(s), dq] natural-layout V chunk (i = sb*4+ii)."""
            i = sb * 4 + ii
            ps = pp.tile([128, dq], F32, tag="pp", name="ps_v")
            wv_ = w3(wv_sb)
            for ec in range(nec):
                nc.tensor.matmul(
                    ps[:, :],
                    x_t["v", sb][:, ec * SQ + ii * 128 : ec * SQ + ii * 128 + 128],
                    wv_[:, ec, :],
                    start=(ec == 0),
                    stop=(ec == nec - 1),
                )
            v3 = v_sb[i].rearrange("p (h t) -> p h t", t=d + 1)
            nc.vector.tensor_add(
                v3[:, :, 0:d],
                ps[:, :].rearrange("p (h t) -> p h t", t=d),
                bv_sb[:, :].rearrange("p (h t) -> p h t", t=d),
            )

        # static PE/ACT occupancy estimate driving filler insertion
        eng_ns = {"pe": 0.0, "act": 0.0}

        def pe_rows(n):
            eng_ns["pe"] += n * 0.4167

        def act_cols(n):
            eng_ns["act"] += n * 0.8333 + 185.0

        def attention_head(qb, h, att_tiles):
            """scores+exp+PV+normalize for one (window, head).

            Generator: yields after each score-group / PV emission so the
            driver can interleave PE filler while ACT churns through exps.
            """
            c, hp = h // 2, (h % 2) * 64
            nkb = 4 * qb + 4 if causal else nsc
            # segments: (kb, qstart_global, width)
            segs = []
            for kb in range(nkb):
                if causal and kb >= 4 * qb:
                    qs = kb * 128
                else:
                    qs = qb * SQ
                segs.append((kb, qs, (qb + 1) * SQ - qs))
            # greedy-pack into exp groups of width <= GW
            groups, cur, curw = [], [], 0
            for seg in segs:
                if curw + seg[2] > GW:
                    groups.append(cur)
                    cur, curw = [], 0
                cur.append(seg)
                curw += seg[2]
            if cur:
                groups.append(cur)

            vpa = vp.tile([128, 4 * (d + 1)], F32, tag="vo", name="vpa")
            last_kb = nkb - 1

            def emit_scores(grp):
                gw = sum(g[2] for g in grp)
                scp = sp.tile([128, GW], F32, tag="sc", name="scp")
                off = 0
                for kb, qs, w in grp:
                    ks = k_sb[c][kb // 4][hp : hp + d,
                                          (kb % 4) * 128 : (kb % 4) * 128 + 128]
                    qw_ = q_sb[c][qs // SQ]
                    if causal and kb >= 4 * qb:
                        # additive mask for the diagonal-crossing sub-block
                        nc.tensor.matmul(scp[:, off : off + 128], id_sb[:, :],
                                         mk_sb[:, :], start=True, stop=False)
                        nc.tensor.matmul(
                            scp[:, off : off + 128], ks,
                            qw_[hp : hp + d, qs % SQ : qs % SQ + 128],
                            start=False, stop=True,
                        )
                        pe_rows(256)
                        if w > 128:
                            nc.tensor.matmul(
                                scp[:, off + 128 : off + w], ks,
                                qw_[hp : hp + d, qs % SQ + 128 : qs % SQ + w],
                                start=True, stop=True,
                            )
                            pe_rows(w - 128)
                    else:
                        nc.tensor.matmul(
                            scp[:, off : off + w], ks,
                            qw_[hp : hp + d, qs % SQ : qs % SQ + w],
                            start=True, stop=True,
                        )
                        pe_rows(w)
                    off += w
                pt = ppool.tile([128, GW], BF16, tag="pt", name="pt")
                nc.scalar.activation(
                    pt[:, 0:gw], scp[:, 0:gw], AF.Exp,
                    scale=float(1.0 / np.sqrt(d)),
                )
                act_cols(gw)
                if debug and h == 0:
                    nc.sync.dma_start(
                        out=dbg_pt[:, dbg_off[0] : dbg_off[0] + gw],
                        in_=pt[:, 0:gw])
                    dbg_off[0] += gw
                return pt

            def emit_pv(grp, pt):
                # One psum accumulation group for the whole vpa bank: a
                # start marks the full 2KB zero-region pending-zero, so only
                # the first matmul may carry start and only the last stop;
                # each sub-region auto-initializes on its first write.
                off = 0
                for kb, qs, w in grp:
                    for qcl in range(4):
                        qg = 4 * qb + qcl           # global q chunk
                        if causal and qg < kb:
                            continue                 # fully masked block
                        boff = off + qcl * 128 + qb * SQ - qs
                        nc.tensor.matmul(
                            vpa[:, qcl * (d + 1) : (qcl + 1) * (d + 1)],
                            pt[:, boff : boff + 128],
                            v_sb[kb][:, h * (d + 1) : (h + 1) * (d + 1)],
                            start=(kb == 0 and qcl == 0),
                            stop=(kb == last_kb and qcl == 3),
                        )
                        pe_rows(d + 1)
                    off += w

            # lag-1 software pipeline: scores g+1 overlaps exp g
            prev = None
            for grp in groups:
                pt = emit_scores(grp)
                yield
                if prev is not None:
                    emit_pv(*prev)
                    yield
                prev = (grp, pt)
            emit_pv(*prev)

            v4 = vpa.rearrange("p (qc t) -> p qc t", t=d + 1)
            rcp = nrm.tile([128, 4], F32, tag="rcp", name="rcp")
            nc.vector.reciprocal(rcp[:, :], v4[:, :, d])
            if debug and h == 0:
                nc.sync.dma_start(out=dbg_rc[:, qb * 4 : qb * 4 + 4],
                                  in_=rcp[:, :])
            for qcl in range(4):
                nc.vector.tensor_scalar_mul(
                    att_tiles[qcl][:, h * d : (h + 1) * d],
                    v4[:, qcl, 0:d],
                    rcp[:, qcl : qcl + 1],
                )

        def wo_transpose_unit(att_tiles, cc, at_store, copy_eng=None):
            """Transpose attn chunk cc (heads 2cc, 2cc+1) -> at_store[cc]."""
            tp = pp.tile([128, SQ], F16, tag="pp", name="tp")
            for qcl in range(4):
                nc.tensor.transpose(
                    tp[:, qcl * 128 : (qcl + 1) * 128],
                    att_tiles[qcl][:, cc * 128 : (cc + 1) * 128],
                    id_sb[:, :],
                )
                pe_rows(128)
            at_ = att.tile([128, SQ], F16, tag=f"at{cc}", name="at_")
            if copy_eng is None:
                nc.vector.tensor_copy(at_[:, :], tp[:, :])
            else:
                copy_eng.copy(at_[:, :], tp[:, :])
            at_store[cc] = at_

        def wo_matmul_unit(at_store, qb, i, copy_eng=None):
            """Output projection + store for s-chunk i of window qb."""
            wo3 = wo_sb.rearrange("p (n m) -> p n m", m=e)
            ot = opool.tile([128, e], F16, tag="ot", name="ot")
            si = qb * 4 + i
            for ob in range(2):
                ps = pp.tile([128, 512], F32, tag="pp", name="ps_o")
                for cc in range(ndq):
                    nc.tensor.matmul(
                        ps[:, :],
                        at_store[cc][:, i * 128 : (i + 1) * 128],
                        wo3[:, cc, ob * 512 : (ob + 1) * 512],
                        start=(cc == 0),
                        stop=(cc == ndq - 1),
                    )
                    pe_rows(512)
                if copy_eng is None:
                    nc.vector.tensor_copy(
                        ot[:, ob * 512 : (ob + 1) * 512], ps[:, :])
                else:
                    copy_eng.copy(ot[:, ob * 512 : (ob + 1) * 512], ps[:, :])
                nc.sync.dma_start(
                    out=out[si * 128 : (si + 1) * 128,
                            ob * 512 : (ob + 1) * 512],
                    in_=ot[:, ob * 512 : (ob + 1) * 512],
                )

        # ---- projection queue, deadline-ordered ----
        # Per window sb: q/k chunk c due just before head 2c; v slab due
        # during head 0's score groups (its diag PV needs it). Deadline key:
        # (sb, h_due) with v at h_due=1 (forced explicitly at h0's yields).
        proj_queue = []
        for sb in range(nwin):
            proj_queue.append((sb, 0, "q", sb, 0))
            proj_queue.append((sb, 0, "k", sb, 0))
            for ii in range(4):
                proj_queue.append((sb, 1, "v", sb, ii))
            for c in range(1, ndq):
                proj_queue.append((sb, 2 * c, "q", sb, c))
                proj_queue.append((sb, 2 * c, "k", sb, c))
        wo_queue = []

        def emit_proj_unit():
            _, _, kind, sb, j = proj_queue.pop(0)
            if kind == "q":
                proj_qk_unit(wq_sb, "q", q_sb, bq_sb, sb, j)
            elif kind == "k":
                proj_qk_unit(wk_sb, "k", k_sb, bk_sb, sb, j)
            else:
                proj_v_unit(sb, j)
            pe_rows(nec * SQ)

        def balance_filler(qb):
            # Keep PE fed while ACT is the pacing engine — but don't consume
            # units whose deadline lets them fill a FUTURE window's ACT-bound
            # stretch (they are the only legal filler there).
            while eng_ns["pe"] < eng_ns["act"] and proj_queue and (
                (proj_queue[0][0], proj_queue[0][1]) < (qb + 1, 1)
            ):
                emit_proj_unit()

        def force_due(qb, h):
            while proj_queue and (proj_queue[0][0], proj_queue[0][1]) <= (qb, h):
                emit_proj_unit()

        def wo_full(qb, att_tiles, last=False):
            at_store = [None] * ndq
            for cc in range(ndq):
                wo_transpose_unit(att_tiles, cc, at_store)
            if debug:
                for qcl in range(4):
                    nc.sync.dma_start(
                        out=dbg_at[(qb * 4 + qcl) * 128 :
                                   (qb * 4 + qcl + 1) * 128, :],
                        in_=att_tiles[qcl][:, :],
                    )
            for i in range(4):
                # final window: ACT is idle by now, DVE is not
                wo_matmul_unit(at_store, qb, i,
                               copy_eng=nc.scalar if last else None)

        # ---- emission ----
        # bootstrap: the startup is DMA-bound; emit the units whose inputs
        # arrive first (all of q0 + k0c0) so PE never out-runs the DMA stream
        boot = {("q", 0, 0), ("q", 0, 1), ("q", 0, 2), ("q", 0, 3), ("k", 0, 0)}
        for c in range(ndq):
            proj_qk_unit(wq_sb, "q", q_sb, bq_sb, 0, c)
        proj_qk_unit(wk_sb, "k", k_sb, bk_sb, 0, 0)
        proj_queue = [u for u in proj_queue if (u[2], u[3], u[4]) not in boot]

        prev = None  # deferred (qb, att_tiles, at_store) for wo
        last_store = [None] * ndq
        for qb in range(nwin):
            att_tiles = [
                atn.tile([128, dq], F16, tag=f"an{qcl}", name=f"an{qcl}_{qb}")
                for qcl in range(4)
            ]
            for h in range(hpc):
                force_due(qb, h)
                yi = 0
                for _ in attention_head(qb, h, att_tiles):
                    yi += 1
                    if h == 0 and yi <= 2:
                        # v slab for this window's diagonal, 2 units per yield
                        for _ in range(2):
                            if proj_queue and proj_queue[0][2] == "v" \
                                    and proj_queue[0][3] == qb:
                                emit_proj_unit()
                    balance_filler(qb)
                if prev is not None and h >= 1:
                    # spread previous window's Wo through this one: the late
                    # windows are ACT-bound and have no projection filler left
                    pqb, ptiles, pstore = prev
                    if h == 1:
                        for cc in range(ndq):
                            wo_transpose_unit(ptiles, cc, pstore)
                        if debug:
                            for qcl in range(4):
                                nc.sync.dma_start(
                                    out=dbg_at[(pqb * 4 + qcl) * 128 :
                                               (pqb * 4 + qcl + 1) * 128, :],
                                    in_=ptiles[qcl][:, :],
                                )
                    elif h in (3, 5, 6, 7):
                        wo_matmul_unit(pstore, pqb, (3, 5, 6, 7).index(h))
                        if h == 7:
                            prev = None
            prev = (qb, att_tiles, [None] * ndq)
        while proj_queue:
            emit_proj_unit()
        wo_full(*prev[:2], last=True)

        if debug:
            for c in range(ndq):
                for w in range(nwin):
                    cs = slice(c * 128, (c + 1) * 128)
                    ws = slice(w * SQ, (w + 1) * SQ)
                    nc.sync.dma_start(out=dbg_q[cs, ws], in_=q_sb[c][w][:, :])
                    nc.sync.dma_start(out=dbg_k[cs, ws], in_=k_sb[c][w][:, :])
            for i in range(nsc):
                nc.sync.dma_start(
                    out=dbg_v[i * 128 : (i + 1) * 128, :], in_=v_sb[i][:, :]
                )

    if split_waits:
        split_excess_waits(nc)
    return nc


def make_crossmask():
    kk = np.arange(128)[:, None]
    qq = np.arange(128)[None, :]
    return np.where(kk <= qq, 0.0, NEG).astype(np.float16)


def classify_mask(mask):
    m = np.asarray(mask).reshape(S, S)
    if np.array_equal(m, np.tril(np.ones((S, S), bool))):
        return "causal"
    if m.all():
        return "dense"
    return "generic"


def prep_core_inputs(query, key, value, Wq, bq, Wk, bk, Wv, bv, Wo, bo, mask):
    """Shard + lay out host-side numpy inputs for the 8 cores."""
    kind = classify_mask(mask)
    maps = []
    for core in range(NCORES):
        b, gi = core // NGROUPS, core % NGROUPS
        gs = slice(gi * DQ, (gi + 1) * DQ)
        im = {
            "xq_t": np.ascontiguousarray(
                np.asarray(query[b]).T.astype(np.float16)),
            "xk_t": np.ascontiguousarray(
                np.asarray(key[b]).T.astype(np.float16)),
            "xv_t": np.ascontiguousarray(
                np.asarray(value[b]).T.astype(np.float16)),
            "wq_t": np.ascontiguousarray(
                np.asarray(Wq)[gs, :].T.astype(np.float16)),
            "wk_t": np.ascontiguousarray(
                np.asarray(Wk)[gs, :].T.astype(np.float16)),
            "wv_t": np.ascontiguousarray(
                np.asarray(Wv)[gs, :].T.astype(np.float16)),
            "wo_t": np.ascontiguousarray(
                np.asarray(Wo)[:, gs].T.astype(np.float16)),
            "consts_f32": np.ascontiguousarray(np.concatenate([
                np.asarray(bq)[gs].astype(np.float32).reshape(-1, 128).T,
                np.asarray(bk)[gs].astype(np.float32).reshape(-1, 128).T,
                np.broadcast_to(
                    np.asarray(bv)[gs].astype(np.float32), (128, DQ)),
            ], axis=1)),
            "consts_f16": np.ascontiguousarray(np.concatenate([
                np.eye(128, dtype=np.float16), make_crossmask()
            ], axis=1)),
        }
        maps.append(im)
    return maps, kind


def make_runner(nc, n_cores=NCORES):
    """Build a reusable jitted SPMD executor for `nc` on cores 0..n_cores-1."""
    import jax
    from jax.experimental.shard_map import shard_map
    from jax.sharding import Mesh, PartitionSpec

    from concourse import bass2jax, mybir as _mybir

    bass2jax.install_neuronx_cc_hook()

    partition_name = (
        nc.partition_id_tensor.name if nc.partition_id_tensor else None
    )
    in_names, out_names, out_avals, zero_shapes = [], [], [], []
    for alloc in nc.m.functions[0].allocations:
        if not isinstance(alloc, _mybir.MemoryLocationSet):
            continue
        name = alloc.memorylocations[0].name
        if alloc.kind == "ExternalInput":
            if name != partition_name:
                in_names.append(name)
        elif alloc.kind == "ExternalOutput":
            out_names.append(name)
            shape = tuple(alloc.tensor_shape)
            dtype = _mybir.dt.np(alloc.dtype)
            out_avals.append(jax.core.ShapedArray(shape, dtype))
            zero_shapes.append((shape, dtype))
    n_params = len(in_names)
    all_in = list(in_names) + list(out_names)
    if partition_name is not None:
        all_in.append(partition_name)

    def _body(*args):
        operands = list(args)
        if partition_name is not None:
            operands.append(bass2jax.partition_id_tensor())
        outs = bass2jax._bass_exec_p.bind(
            *operands,
            out_avals=tuple(out_avals),
            in_names=tuple(all_in),
            out_names=tuple(out_names),
            lowering_input_output_aliases=(),
            sim_require_finite=True,
            sim_require_nnan=True,
            nc=nc,
        )
        return tuple(outs)

    devices = jax.devices()[:n_cores]
    assert len(devices) == n_cores
    mesh = Mesh(np.asarray(devices), ("core",))
    in_specs = (PartitionSpec("core"),) * (n_params + len(out_names))
    out_specs = (PartitionSpec("core"),) * len(out_names)
    sharded = jax.jit(
        shard_map(
            _body,
            mesh=mesh,
            in_specs=in_specs,
            out_specs=out_specs,
            check_rep=False,
        ),
        keep_unused=True,
    )
    zeros = [
        np.zeros((n_cores * sh[0], *sh[1:]), dt) for sh, dt in zero_shapes
    ]

    def concat_inputs(in_maps):
        return [
            np.concatenate(
                [np.asarray(in_maps[c][n]) for c in range(n_cores)], axis=0
            )
            for n in in_names
        ]

    def run(in_maps):
        out_arrs = sharded(*concat_inputs(in_maps), *zeros)
        return [
            {
                name: np.asarray(out_arrs[i]).reshape(
                    n_cores, *out_avals[i].shape
                )[c]
                for i, name in enumerate(out_names)
            }
            for c in range(n_cores)
        ]

    run.sharded = sharded
    run.concat_inputs = concat_inputs
    run.zeros = zeros
    run.out_names = out_names
    run.out_avals = out_avals
    return run


_CACHE = {}


def get_runner(kind="causal"):
    if kind not in _CACHE:
        nc = build_kernel(causal=(kind == "causal"))
        _CACHE[kind] = make_runner(nc)
    return _CACHE[kind]


def _numpy_reference(query, key, value, Wq, bq, Wk, bk, Wv, bv, Wo, bo, mask):
    q = (query @ Wq.T + bq).reshape(B, S, H, D).transpose(0, 2, 1, 3)
    k = (key @ Wk.T + bk).reshape(B, S, H, D).transpose(0, 2, 1, 3)
    v = (value @ Wv.T + bv).reshape(B, S, H, D).transpose(0, 2, 1, 3)
    sc = np.einsum("bhqd,bhkd->bhqk", q, k) / np.sqrt(D)
    sc = np.where(np.asarray(mask).reshape(1, 1, S, S), sc, -np.inf)
    sc -= sc.max(axis=-1, keepdims=True)
    p = np.exp(sc)
    p /= p.sum(axis=-1, keepdims=True)
    o = np.einsum("bhqk,bhkd->bhqd", p, v)
    o = o.transpose(0, 2, 1, 3).reshape(B, S, E)
    return o @ Wo.T + bo


def kernel(**inputs) -> np.ndarray:
    kind = classify_mask(inputs["mask"])
    if kind == "generic":
        fp = {k: np.asarray(v, np.float32) for k, v in inputs.items()
              if k != "mask"}
        return _numpy_reference(mask=inputs["mask"], **fp).astype(np.float32)
    in_maps, kind = prep_core_inputs(**inputs)
    run = get_runner(kind)
    results = run(in_maps)
    bo = np.asarray(inputs["bo"], dtype=np.float32)
    out = np.empty((B, S, E), dtype=np.float32)
    for b in range(B):
        acc = results[b * NGROUPS]["out"].astype(np.float32)
        for gi in range(1, NGROUPS):
            acc = acc + results[b * NGROUPS + gi]["out"].astype(np.float32)
        out[b] = acc + bo[None, :]
    return out


# revision 4
# speedup vs baseline: 1.0565x; 1.0288x over previous
"""Trainium2 Bass kernel: 16-head causal attention (B=4, S=2048, E=1024).

Sharding: 8 cores = 4 batches x 2 head-groups (8 heads each); host sums the
two head-group partials (fp32) and adds bo.

Per-core pipeline (fp16/bf16 matmul operands; PSUM accumulates fp32):
  - q^T = Wq_g X^T, k^T = Wk_g X^T    (transposed projections, [dq, S] f16)
  - V   = X^T.T Wv_g^T                (natural [S, dv] bf16, +ones column per
                                       head so PV also yields denominators)
  - scores^T[k, q] at 128x128 causal granularity: fully-masked sub-blocks are
    skipped; each diagonal-crossing sub-block gets one [128,128] additive mask
    matmul (identity stationary, f16 mask moving, NEG=-60000).
  - P^T = exp(scores^T/8) on ACT -> bf16 (range-safe: exp can reach ~1.3e8,
    which overflows f16; masked lanes underflow to exactly 0)
  - PV: out[q, 65] += P^T_block^T V_aug: stationary = P^T [128,128], moving =
    V_aug [128,65] bf16 -> full 128 output partitions at 65 rows/block. One
    PSUM accumulation group per vpa bank (single start/stop; sub-regions
    auto-initialize via the pending-zero mechanism).
  - normalize: DVE reciprocal of the denominator column + tensor_scalar_mul
  - attn [q, dq] f16 -> PE-transpose [dq, q] -> Wo matmul -> f16 partials
Scheduling: the emitter interleaves projection/output-projection work into the
ACT-bound attention windows (deadline queue + PE-vs-ACT balance heuristic),
batches DMAs into ~45 large transfers, and software-pipelines scores/exp/PV
with a lag of one exp group.
"""

import contextlib

import numpy as np

import bass_rust
import concourse.bass as bass
import concourse.mybir as mybir
import concourse.tile as tile

F32 = mybir.dt.float32
F16 = mybir.dt.float16
BF16 = mybir.dt.bfloat16
AF = mybir.ActivationFunctionType

B, S, E = 4, 2048, 1024
H, D = 16, 64
NCORES = 8
NGROUPS = 2            # head groups (tensor parallel)
HPC = H // NGROUPS     # heads per core
DQ = HPC * D           # per-core projection width = 512
NEG = -60000.0         # f16-representable; exp(NEG/8) == 0.0 in fp32

SK = 128               # k sub-block (partition dim of scores^T)
SQ = 512               # q window
GW = 1024              # exp group width (psum [128, GW])


def split_excess_waits(nc, maxw=1):
    """This container's walrus supports one sem wait per instruction;
    hoist extras onto same-engine nops just before the instruction."""
    n_new = 0
    for bb in nc.main_func.blocks:
        new_list = []
        changed = False
        for inst in list(bb.instructions):
            si = inst.sync_info
            waits = list(si.on_wait) if si and si.on_wait else []
            if len(waits) > maxw:
                changed = True
                extra, keep = waits[:-maxw], waits[-maxw:]
                for ci in range(0, len(extra), maxw):
                    nop = bass_rust.InstNoOp(
                        name=f"I-waitsplit-{n_new}", ins=[], outs=[]
                    )
                    n_new += 1
                    nop.engine = inst.engine
                    nop.sync_info = mybir.SyncInfo(
                        on_wait=extra[ci : ci + maxw], on_update=[]
                    )
                    new_list.append(nop)
                inst.sync_info = mybir.SyncInfo(
                    on_wait=keep,
                    on_update=list(si.on_update) if si.on_update else [],
                )
            new_list.append(inst)
        if changed:
            bb.instructions = new_list
    return n_new


def build_kernel(causal=True, split_waits=True, debug=False):
    s, e, hpc, d = S, E, HPC, D
    dq = hpc * d              # 512
    nec = e // 128            # 8 input-feature chunks
    ndq = dq // 128           # 4 projection partition chunks
    nwin = s // SQ            # 4 q windows
    nsc = s // 128            # 16 s chunks

    nc = bass.Bass()

    xq = nc.declare_dram_parameter("xq_t", [e, s], F16, isOutput=False)
    xk = nc.declare_dram_parameter("xk_t", [e, s], F16, isOutput=False)
    xv = nc.declare_dram_parameter("xv_t", [e, s], F16, isOutput=False)
    wqd = nc.declare_dram_parameter("wq_t", [e, dq], F16, isOutput=False)
    wkd = nc.declare_dram_parameter("wk_t", [e, dq], F16, isOutput=False)
    wvd = nc.declare_dram_parameter("wv_t", [e, dq], F16, isOutput=False)
    wod = nc.declare_dram_parameter("wo_t", [dq, e], F16, isOutput=False)
    # packed constants: [bq(4) | bk(4) | bv_b(512)] f32, [ident | crossmask] f16
    cfd = nc.declare_dram_parameter("consts_f32", [128, 2 * ndq + dq], F32,
                                    isOutput=False)
    chd = nc.declare_dram_parameter("consts_f16", [128, 256], F16,
                                    isOutput=False)
    out = nc.declare_dram_parameter("out", [s, e], F16, isOutput=True)
    if debug:
        dbg_q = nc.declare_dram_parameter("dbg_q", [dq, s], F16, isOutput=True)
        dbg_k = nc.declare_dram_parameter("dbg_k", [dq, s], F16, isOutput=True)
        dbg_v = nc.declare_dram_parameter(
            "dbg_v", [s, hpc * (d + 1)], BF16, isOutput=True
        )
        dbg_at = nc.declare_dram_parameter("dbg_at", [s, dq], F16, isOutput=True)
        dbg_pt = nc.declare_dram_parameter("dbg_pt", [128, 17408], BF16,
                                           isOutput=True)
        dbg_rc = nc.declare_dram_parameter("dbg_rc", [128, 16], F32,
                                           isOutput=True)
        dbg_off = [0]

    with tile.TileContext(nc) as tc, contextlib.ExitStack() as ctx:
        pers = ctx.enter_context(tc.tile_pool(name="pers", bufs=1))
        xpool = ctx.enter_context(tc.tile_pool(name="xp", bufs=3))
        ppool = ctx.enter_context(tc.tile_pool(name="ppl", bufs=4))
        atn = ctx.enter_context(tc.tile_pool(name="atn", bufs=4))
        att = ctx.enter_context(tc.tile_pool(name="att", bufs=4))
        nrm = ctx.enter_context(tc.tile_pool(name="nrm", bufs=4))
        opool = ctx.enter_context(tc.tile_pool(name="opl", bufs=3))
        pp = ctx.enter_context(tc.tile_pool(name="pp", bufs=2, space="PSUM"))
        sp = ctx.enter_context(tc.tile_pool(name="sp", bufs=2, space="PSUM"))
        vp = ctx.enter_context(tc.tile_pool(name="vp", bufs=2, space="PSUM"))

        # ---- persistent tensors ----
        cf_sb = pers.tile([128, 2 * ndq + dq], F32, name="cf_sb")
        ch_sb = pers.tile([128, 256], F16, name="ch_sb")
        bq_sb = cf_sb[:, 0:ndq]
        bk_sb = cf_sb[:, ndq : 2 * ndq]
        bv_sb = cf_sb[:, 2 * ndq : 2 * ndq + dq]
        id_sb = ch_sb[:, 0:128]
        mk_sb = ch_sb[:, 128:256]
        q_sb = [
            [pers.tile([128, SQ], F16, name=f"q_sb{c}_{w}") for w in range(nwin)]
            for c in range(ndq)
        ]
        k_sb = [
            [pers.tile([128, SQ], F16, name=f"k_sb{c}_{w}") for w in range(nwin)]
            for c in range(ndq)
        ]
        v_sb = [
            pers.tile([128, hpc * (d + 1)], BF16, name=f"v_sb{i}")
            for i in range(nsc)
        ]
        wq_sb = pers.tile([128, nec * dq], F16, name="wq_sb")
        wk_sb = pers.tile([128, nec * dq], F16, name="wk_sb")
        wv_sb = pers.tile([128, nec * dq], F16, name="wv_sb")
        wo_sb = pers.tile([128, ndq * e], F16, name="wo_sb")

        # ---- DMA helpers (SP engine -> one HWDGE queue, program order) ----
        def load_w_part(wt, dst, part, nparts=2):
            # e-chunk group `part` of [e, dq] -> dst cols
            g = nec // nparts
            src = wt.rearrange("(n p) m -> p n m", p=128)
            nc.sync.dma_start(
                out=dst.rearrange("p (n m) -> p n m", m=dq)[
                    :, part * g : (part + 1) * g, :
                ],
                in_=src[:, part * g : (part + 1) * g, :],
            )

        def load_x_slab(xt, dst, sb, part=None, nparts=2):
            # dst: [128, nec*512] tile; cols [sb*512,(sb+1)*512) of [e, s]
            src = xt.rearrange("(n p) m -> p n m", p=128)
            d3 = dst.rearrange("p (n m) -> p n m", m=SQ)
            if part is None:
                nc.sync.dma_start(
                    out=d3[:, :, :],
                    in_=src[:, :, sb * SQ : (sb + 1) * SQ],
                )
            else:
                g = nec // nparts
                nc.sync.dma_start(
                    out=d3[:, part * g : (part + 1) * g, :],
                    in_=src[:, part * g : (part + 1) * g,
                            sb * SQ : (sb + 1) * SQ],
                )



        x_t = {}  # (tensor, sb) -> slab tile
        for t, xd in (("q", xq), ("k", xk), ("v", xv)):
            x_t[t, 0] = xpool.tile([128, nec * SQ], F16, tag=f"x{t}",
                                   name=f"x{t}0", bufs=3)
        # slab 0 interleaved with weight pieces for earliest unblock;
        # wq/xq0 in quarters so the first projection matmuls start ASAP
        for part in range(4):
            load_w_part(wqd, wq_sb, part, nparts=4)
            load_x_slab(xq, x_t["q", 0], 0, part=part, nparts=4)
        # packed constants (biases for the first bias-add, mask for h0 scores)
        nc.sync.dma_start(out=cf_sb[:, :], in_=cfd[:, :])
        nc.sync.dma_start(out=ch_sb[:, :], in_=chd[:, :])
        load_w_part(wkd, wk_sb, 0)
        load_x_slab(xk, x_t["k", 0], 0, part=0)
        load_w_part(wkd, wk_sb, 1)
        load_x_slab(xk, x_t["k", 0], 0, part=1)
        load_w_part(wvd, wv_sb, 0)
        load_x_slab(xv, x_t["v", 0], 0, part=0)
        load_w_part(wvd, wv_sb, 1)
        load_x_slab(xv, x_t["v", 0], 0, part=1)
        x_t["q", 1] = xpool.tile([128, nec * SQ], F16, tag="xq",
                                 name="xq1", bufs=3)
        load_x_slab(xq, x_t["q", 1], 1)
        for sb in range(1, nwin):
            for t, xd in (("q", xq), ("k", xk), ("v", xv)):
                if (t, sb) in x_t:
                    continue
                x_t[t, sb] = xpool.tile([128, nec * SQ], F16, tag=f"x{t}",
                                        name=f"x{t}{sb}", bufs=3)
                load_x_slab(xd, x_t[t, sb], sb)
            if sb == 1:
                nc.sync.dma_start(
                    out=wo_sb.rearrange("p (n m) -> p n m", m=e),
                    in_=wod.rearrange("(n p) m -> p n m", p=128),
                )

        # ones columns of v_sb, once, on the idle gpsimd engine
        for i in range(nsc):
            v3 = v_sb[i].rearrange("p (h t) -> p h t", t=d + 1)
            nc.gpsimd.memset(v3[:, :, d], 1.0)

        # ---- compute unit generators ----
        def w3(wt):
            return wt.rearrange("p (n m) -> p n m", m=dq)

        def proj_qk_unit(w_sb_t, xt, dst, bias, sb, c):
            """One [128,512] slab-column of a transposed projection."""
            ps = pp.tile([128, SQ], F32, tag="pp", name="ps_pj")
            wv_ = w3(w_sb_t)
            for ec in range(nec):
                nc.tensor.matmul(
                    ps[:, :],
                    wv_[:, ec, c * 128 : (c + 1) * 128],
                    x_t[xt, sb][:, ec * SQ : (ec + 1) * SQ],
                    start=(ec == 0),
                    stop=(ec == nec - 1),
                )
            nc.vector.tensor_scalar_add(
                dst[c][sb][:, :], ps[:, :], bias[:, c : c + 1]
            )

        def proj_v_unit(sb, ii):
            """One [128(s), dq] natural-layout V chunk (i = sb*4+ii)."""
            i = sb * 4 + ii
            ps = pp.tile([128, dq], F32, tag="pp", name="ps_v")
            wv_ = w3(wv_sb)
            for ec in range(nec):
                nc.tensor.matmul(
                    ps[:, :],
                    x_t["v", sb][:, ec * SQ + ii * 128 : ec * SQ + ii * 128 + 128],
                    wv_[:, ec, :],
                    start=(ec == 0),
                    stop=(ec == nec - 1),
                )
            v3 = v_sb[i].rearrange("p (h t) -> p h t", t=d + 1)
            nc.vector.tensor_add(
                v3[:, :, 0:d],
                ps[:, :].rearrange("p (h t) -> p h t", t=d),
                bv_sb[:, :].rearrange("p (h t) -> p h t", t=d),
            )

        # static PE/ACT occupancy estimate driving filler insertion
        eng_ns = {"pe": 0.0, "act": 0.0}

        def pe_rows(n):
            eng_ns["pe"] += n * 0.4167

        def act_cols(n):
            eng_ns["act"] += n * 0.8333 + 185.0

        def attention_head(qb, h, att_tiles, pre_last_cb=None,
                           act_norm=False):
            """scores+exp+PV+normalize for one (window, head).

            Generator: yields after each score-group / PV emission so the
            driver can interleave PE filler while ACT churns through exps.
            pre_last_cb: emitted right after the last score group (tail
            shortening for the final head). act_norm: do half the normalize
            multiplies on ACT (only sensible when ACT is idle afterwards).
            """
            c, hp = h // 2, (h % 2) * 64
            nkb = 4 * qb + 4 if causal else nsc
            # segments: (kb, qstart_global, width)
            segs = []
            for kb in range(nkb):
                if causal and kb >= 4 * qb:
                    qs = kb * 128
                else:
                    qs = qb * SQ
                segs.append((kb, qs, (qb + 1) * SQ - qs))
            # greedy-pack into exp groups of width <= GW
            groups, cur, curw = [], [], 0
            for seg in segs:
                if curw + seg[2] > GW:
                    groups.append(cur)
                    cur, curw = [], 0
                cur.append(seg)
                curw += seg[2]
            if cur:
                groups.append(cur)

            vpa = vp.tile([128, 4 * (d + 1)], F32, tag="vo", name="vpa")
            last_kb = nkb - 1

            def emit_scores(grp):
                gw = sum(g[2] for g in grp)
                scp = sp.tile([128, GW], F32, tag="sc", name="scp")
                off = 0
                for kb, qs, w in grp:
                    ks = k_sb[c][kb // 4][hp : hp + d,
                                          (kb % 4) * 128 : (kb % 4) * 128 + 128]
                    qw_ = q_sb[c][qs // SQ]
                    if causal and kb >= 4 * qb:
                        # additive mask for the diagonal-crossing sub-block
                        nc.tensor.matmul(scp[:, off : off + 128], id_sb[:, :],
                                         mk_sb[:, :], start=True, stop=False)
                        nc.tensor.matmul(
                            scp[:, off : off + 128], ks,
                            qw_[hp : hp + d, qs % SQ : qs % SQ + 128],
                            start=False, stop=True,
                        )
                        pe_rows(256)
                        if w > 128:
                            nc.tensor.matmul(
                                scp[:, off + 128 : off + w], ks,
                                qw_[hp : hp + d, qs % SQ + 128 : qs % SQ + w],
                                start=True, stop=True,
                            )
                            pe_rows(w - 128)
                    else:
                        nc.tensor.matmul(
                            scp[:, off : off + w], ks,
                            qw_[hp : hp + d, qs % SQ : qs % SQ + w],
                            start=True, stop=True,
                        )
                        pe_rows(w)
                    off += w
                pt = ppool.tile([128, GW], BF16, tag="pt", name="pt")
                nc.scalar.activation(
                    pt[:, 0:gw], scp[:, 0:gw], AF.Exp,
                    scale=float(1.0 / np.sqrt(d)),
                )
                act_cols(gw)
                if debug and h == 0:
                    nc.sync.dma_start(
                        out=dbg_pt[:, dbg_off[0] : dbg_off[0] + gw],
                        in_=pt[:, 0:gw])
                    dbg_off[0] += gw
                return pt

            def emit_pv(grp, pt):
                # One psum accumulation group for the whole vpa bank: a
                # start marks the full 2KB zero-region pending-zero, so only
                # the first matmul may carry start and only the last stop;
                # each sub-region auto-initializes on its first write.
                off = 0
                for kb, qs, w in grp:
                    for qcl in range(4):
                        qg = 4 * qb + qcl           # global q chunk
                        if causal and qg < kb:
                            continue                 # fully masked block
                        boff = off + qcl * 128 + qb * SQ - qs
                        nc.tensor.matmul(
                            vpa[:, qcl * (d + 1) : (qcl + 1) * (d + 1)],
                            pt[:, boff : boff + 128],
                            v_sb[kb][:, h * (d + 1) : (h + 1) * (d + 1)],
                            start=(kb == 0 and qcl == 0),
                            stop=(kb == last_kb and qcl == 3),
                        )
                        pe_rows(d + 1)
                    off += w

            # lag-1 software pipeline: scores g+1 overlaps exp g
            prev = None
            for gi, grp in enumerate(groups):
                pt = emit_scores(grp)
                if pre_last_cb is not None and gi == len(groups) - 1:
                    pre_last_cb()
                yield
                if prev is not None:
                    emit_pv(*prev)
                    yield
                prev = (grp, pt)
            emit_pv(*prev)

            v4 = vpa.rearrange("p (qc t) -> p qc t", t=d + 1)
            rcp = nrm.tile([128, 4], F32, tag="rcp", name="rcp")
            nc.vector.reciprocal(rcp[:, :], v4[:, :, d])
            if debug and h == 0:
                nc.sync.dma_start(out=dbg_rc[:, qb * 4 : qb * 4 + 4],
                                  in_=rcp[:, :])
            for qcl in range(4):
                if act_norm and qcl >= 2:
                    nc.scalar.activation(
                        att_tiles[qcl][:, h * d : (h + 1) * d],
                        v4[:, qcl, 0:d],
                        AF.Copy,
                        scale=rcp[:, qcl : qcl + 1],
                    )
                else:
                    nc.vector.tensor_scalar_mul(
                        att_tiles[qcl][:, h * d : (h + 1) * d],
                        v4[:, qcl, 0:d],
                        rcp[:, qcl : qcl + 1],
                    )

        def wo_transpose_unit(att_tiles, cc, at_store, copy_eng=None):
            """Transpose attn chunk cc (heads 2cc, 2cc+1) -> at_store[cc]."""
            tp = pp.tile([128, SQ], F16, tag="pp", name="tp")
            for qcl in range(4):
                nc.tensor.transpose(
                    tp[:, qcl * 128 : (qcl + 1) * 128],
                    att_tiles[qcl][:, cc * 128 : (cc + 1) * 128],
                    id_sb[:, :],
                )
                pe_rows(128)
            at_ = att.tile([128, SQ], F16, tag=f"at{cc}", name="at_")
            if copy_eng is None:
                nc.vector.tensor_copy(at_[:, :], tp[:, :])
            else:
                copy_eng.copy(at_[:, :], tp[:, :])
            at_store[cc] = at_

        def wo_matmul_unit(at_store, qb, i, copy_eng=None):
            """Output projection + store for s-chunk i of window qb."""
            wo3 = wo_sb.rearrange("p (n m) -> p n m", m=e)
            ot = opool.tile([128, e], F16, tag="ot", name="ot")
            si = qb * 4 + i
            for ob in range(2):
                ps = pp.tile([128, 512], F32, tag="pp", name="ps_o")
                for cc in range(ndq):
                    nc.tensor.matmul(
                        ps[:, :],
                        at_store[cc][:, i * 128 : (i + 1) * 128],
                        wo3[:, cc, ob * 512 : (ob + 1) * 512],
                        start=(cc == 0),
                        stop=(cc == ndq - 1),
                    )
                    pe_rows(512)
                if copy_eng is None:
                    nc.vector.tensor_copy(
                        ot[:, ob * 512 : (ob + 1) * 512], ps[:, :])
                else:
                    copy_eng.copy(ot[:, ob * 512 : (ob + 1) * 512], ps[:, :])
                nc.sync.dma_start(
                    out=out[si * 128 : (si + 1) * 128,
                            ob * 512 : (ob + 1) * 512],
                    in_=ot[:, ob * 512 : (ob + 1) * 512],
                )

        # ---- projection queue, deadline-ordered ----
        # Per window sb: q/k chunk c due just before head 2c; v slab due
        # during head 0's score groups (its diag PV needs it). Deadline key:
        # (sb, h_due) with v at h_due=1 (forced explicitly at h0's yields).
        proj_queue = []
        for sb in range(nwin):
            proj_queue.append((sb, 0, "q", sb, 0))
            proj_queue.append((sb, 0, "k", sb, 0))
            for ii in range(4):
                proj_queue.append((sb, 1, "v", sb, ii))
            for c in range(1, ndq):
                proj_queue.append((sb, 2 * c, "q", sb, c))
                proj_queue.append((sb, 2 * c, "k", sb, c))
        wo_queue = []

        def emit_proj_unit():
            _, _, kind, sb, j = proj_queue.pop(0)
            if kind == "q":
                proj_qk_unit(wq_sb, "q", q_sb, bq_sb, sb, j)
            elif kind == "k":
                proj_qk_unit(wk_sb, "k", k_sb, bk_sb, sb, j)
            else:
                proj_v_unit(sb, j)
            pe_rows(nec * SQ)

        def balance_filler(qb):
            # Keep PE fed while ACT is the pacing engine — but don't consume
            # units whose deadline lets them fill a FUTURE window's ACT-bound
            # stretch (they are the only legal filler there).
            while eng_ns["pe"] < eng_ns["act"]:
                if proj_queue and (
                    (proj_queue[0][0], proj_queue[0][1]) < (qb + 1, 1)
                ):
                    emit_proj_unit()
                elif wo_queue:
                    wo_queue.pop(0)()
                else:
                    return

        def force_due(qb, h):
            while proj_queue and (proj_queue[0][0], proj_queue[0][1]) <= (qb, h):
                emit_proj_unit()

        def wo_full(qb, att_tiles, last=False):
            at_store = [None] * ndq
            for cc in range(ndq):
                wo_transpose_unit(att_tiles, cc, at_store)
            if debug:
                for qcl in range(4):
                    nc.sync.dma_start(
                        out=dbg_at[(qb * 4 + qcl) * 128 :
                                   (qb * 4 + qcl + 1) * 128, :],
                        in_=att_tiles[qcl][:, :],
                    )
            for i in range(4):
                # final window: ACT is idle by now, DVE is not
                wo_matmul_unit(at_store, qb, i,
                               copy_eng=nc.scalar if last else None)

        # ---- emission ----
        # bootstrap: the startup is DMA-bound; emit the units whose inputs
        # arrive first (all of q0 + k0c0) so PE never out-runs the DMA stream
        boot = {("q", 0, 0), ("q", 0, 1), ("q", 0, 2), ("q", 0, 3), ("k", 0, 0)}
        for c in range(ndq):
            proj_qk_unit(wq_sb, "q", q_sb, bq_sb, 0, c)
        proj_qk_unit(wk_sb, "k", k_sb, bk_sb, 0, 0)
        proj_queue = [u for u in proj_queue if (u[2], u[3], u[4]) not in boot]

        prev = None  # deferred (qb, att_tiles, at_store) for wo
        last_store = [None] * ndq
        for qb in range(nwin):
            att_tiles = [
                atn.tile([128, dq], F16, tag=f"an{qcl}", name=f"an{qcl}_{qb}")
                for qcl in range(4)
            ]
            for h in range(hpc):
                force_due(qb, h)
                yi = 0
                for _ in attention_head(qb, h, att_tiles):
                    yi += 1
                    if h == 0 and yi <= 2:
                        # v slab for this window's diagonal, 2 units per yield
                        for _ in range(2):
                            if proj_queue and proj_queue[0][2] == "v" \
                                    and proj_queue[0][3] == qb:
                                emit_proj_unit()
                    balance_filler(qb)
            # defer this window's Wo into the balance queue: it is the only
            # PE work with no deadline, so it belongs in the late ACT-bound
            # holes (atn/att bufs=4 make any emission order inversion-free)
            pqb, ptiles, pstore = qb, att_tiles, [None] * ndq

            def mk_tr(ptiles=ptiles, pstore=pstore, pqb=pqb):
                for cc in range(ndq):
                    wo_transpose_unit(ptiles, cc, pstore)
                if debug:
                    for qcl in range(4):
                        nc.sync.dma_start(
                            out=dbg_at[(pqb * 4 + qcl) * 128 :
                                       (pqb * 4 + qcl + 1) * 128, :],
                            in_=ptiles[qcl][:, :],
                        )

            if qb < nwin - 1:
                wo_queue.append(mk_tr)
                for i in range(4):
                    wo_queue.append(
                        lambda st=pstore, w=pqb, j=i: wo_matmul_unit(st, w, j))
            else:
                prev = (qb, att_tiles)
        while proj_queue:
            emit_proj_unit()
        while wo_queue:
            wo_queue.pop(0)()
        wo_full(*prev, last=True)

        if debug:
            for c in range(ndq):
                for w in range(nwin):
                    cs = slice(c * 128, (c + 1) * 128)
                    ws = slice(w * SQ, (w + 1) * SQ)
                    nc.sync.dma_start(out=dbg_q[cs, ws], in_=q_sb[c][w][:, :])
                    nc.sync.dma_start(out=dbg_k[cs, ws], in_=k_sb[c][w][:, :])
            for i in range(nsc):
                nc.sync.dma_start(
                    out=dbg_v[i * 128 : (i + 1) * 128, :], in_=v_sb[i][:, :]
                )

    if split_waits:
        split_excess_waits(nc)
    return nc


def make_crossmask():
    kk = np.arange(128)[:, None]
    qq = np.arange(128)[None, :]
    return np.where(kk <= qq, 0.0, NEG).astype(np.float16)


def classify_mask(mask):
    m = np.asarray(mask).reshape(S, S)
    if np.array_equal(m, np.tril(np.ones((S, S), bool))):
        return "causal"
    if m.all():
        return "dense"
    return "generic"


def prep_core_inputs(query, key, value, Wq, bq, Wk, bk, Wv, bv, Wo, bo, mask):
    """Shard + lay out host-side numpy inputs for the 8 cores."""
    kind = classify_mask(mask)
    maps = []
    for core in range(NCORES):
        b, gi = core // NGROUPS, core % NGROUPS
        gs = slice(gi * DQ, (gi + 1) * DQ)
        im = {
            "xq_t": np.ascontiguousarray(
                np.asarray(query[b]).T.astype(np.float16)),
            "xk_t": np.ascontiguousarray(
                np.asarray(key[b]).T.astype(np.float16)),
            "xv_t": np.ascontiguousarray(
                np.asarray(value[b]).T.astype(np.float16)),
            "wq_t": np.ascontiguousarray(
                np.asarray(Wq)[gs, :].T.astype(np.float16)),
            "wk_t": np.ascontiguousarray(
                np.asarray(Wk)[gs, :].T.astype(np.float16)),
            "wv_t": np.ascontiguousarray(
                np.asarray(Wv)[gs, :].T.astype(np.float16)),
            "wo_t": np.ascontiguousarray(
                np.asarray(Wo)[:, gs].T.astype(np.float16)),
            "consts_f32": np.ascontiguousarray(np.concatenate([
                np.asarray(bq)[gs].astype(np.float32).reshape(-1, 128).T,
                np.asarray(bk)[gs].astype(np.float32).reshape(-1, 128).T,
                np.broadcast_to(
                    np.asarray(bv)[gs].astype(np.float32), (128, DQ)),
            ], axis=1)),
            "consts_f16": np.ascontiguousarray(np.concatenate([
                np.eye(128, dtype=np.float16), make_crossmask()
            ], axis=1)),
        }
        maps.append(im)
    return maps, kind


def make_runner(nc, n_cores=NCORES):
    """Build a reusable jitted SPMD executor for `nc` on cores 0..n_cores-1."""
    import jax
    from jax.experimental.shard_map import shard_map
    from jax.sharding import Mesh, PartitionSpec

    from concourse import bass2jax, mybir as _mybir

    bass2jax.install_neuronx_cc_hook()

    partition_name = (
        nc.partition_id_tensor.name if nc.partition_id_tensor else None
    )
    in_names, out_names, out_avals, zero_shapes = [], [], [], []
    for alloc in nc.m.functions[0].allocations:
        if not isinstance(alloc, _mybir.MemoryLocationSet):
            continue
        name = alloc.memorylocations[0].name
        if alloc.kind == "ExternalInput":
            if name != partition_name:
                in_names.append(name)
        elif alloc.kind == "ExternalOutput":
            out_names.append(name)
            shape = tuple(alloc.tensor_shape)
            dtype = _mybir.dt.np(alloc.dtype)
            out_avals.append(jax.core.ShapedArray(shape, dtype))
            zero_shapes.append((shape, dtype))
    n_params = len(in_names)
    all_in = list(in_names) + list(out_names)
    if partition_name is not None:
        all_in.append(partition_name)

    def _body(*args):
        operands = list(args)
        if partition_name is not None:
            operands.append(bass2jax.partition_id_tensor())
        outs = bass2jax._bass_exec_p.bind(
            *operands,
            out_avals=tuple(out_avals),
            in_names=tuple(all_in),
            out_names=tuple(out_names),
            lowering_input_output_aliases=(),
            sim_require_finite=True,
            sim_require_nnan=True,
            nc=nc,
        )
        return tuple(outs)

    devices = jax.devices()[:n_cores]
    assert len(devices) == n_cores
    mesh = Mesh(np.asarray(devices), ("core",))
    in_specs = (PartitionSpec("core"),) * (n_params + len(out_names))
    out_specs = (PartitionSpec("core"),) * len(out_names)
    sharded = jax.jit(
        shard_map(
            _body,
            mesh=mesh,
            in_specs=in_specs,
            out_specs=out_specs,
            check_rep=False,
        ),
        keep_unused=True,
    )
    zeros = [
        np.zeros((n_cores * sh[0], *sh[1:]), dt) for sh, dt in zero_shapes
    ]

    def concat_inputs(in_maps):
        return [
            np.concatenate(
                [np.asarray(in_maps[c][n]) for c in range(n_cores)], axis=0
            )
            for n in in_names
        ]

    def run(in_maps):
        out_arrs = sharded(*concat_inputs(in_maps), *zeros)
        return [
            {
                name: np.asarray(out_arrs[i]).reshape(
                    n_cores, *out_avals[i].shape
                )[c]
                for i, name in enumerate(out_names)
            }
            for c in range(n_cores)
        ]

    run.sharded = sharded
    run.concat_inputs = concat_inputs
    run.zeros = zeros
    run.out_names = out_names
    run.out_avals = out_avals
    return run


_CACHE = {}


def get_runner(kind="causal"):
    if kind not in _CACHE:
        nc = build_kernel(causal=(kind == "causal"))
        _CACHE[kind] = make_runner(nc)
    return _CACHE[kind]


def _numpy_reference(query, key, value, Wq, bq, Wk, bk, Wv, bv, Wo, bo, mask):
    q = (query @ Wq.T + bq).reshape(B, S, H, D).transpose(0, 2, 1, 3)
    k = (key @ Wk.T + bk).reshape(B, S, H, D).transpose(0, 2, 1, 3)
    v = (value @ Wv.T + bv).reshape(B, S, H, D).transpose(0, 2, 1, 3)
    sc = np.einsum("bhqd,bhkd->bhqk", q, k) / np.sqrt(D)
    sc = np.where(np.asarray(mask).reshape(1, 1, S, S), sc, -np.inf)
    sc -= sc.max(axis=-1, keepdims=True)
    p = np.exp(sc)
    p /= p.sum(axis=-1, keepdims=True)
    o = np.einsum("bhqk,bhkd->bhqd", p, v)
    o = o.transpose(0, 2, 1, 3).reshape(B, S, E)
    return o @ Wo.T + bo


def kernel(**inputs) -> np.ndarray:
    kind = classify_mask(inputs["mask"])
    if kind == "generic":
        fp = {k: np.asarray(v, np.float32) for k, v in inputs.items()
              if k != "mask"}
        return _numpy_reference(mask=inputs["mask"], **fp).astype(np.float32)
    in_maps, kind = prep_core_inputs(**inputs)
    run = get_runner(kind)
    results = run(in_maps)
    bo = np.asarray(inputs["bo"], dtype=np.float32)
    out = np.empty((B, S, E), dtype=np.float32)
    for b in range(B):
        acc = results[b * NGROUPS]["out"].astype(np.float32)
        for gi in range(1, NGROUPS):
            acc = acc + results[b * NGROUPS + gi]["out"].astype(np.float32)
        out[b] = acc + bo[None, :]
    return out


# revision 5
# speedup vs baseline: 1.0605x; 1.0037x over previous
"""Trainium2 Bass kernel: 16-head causal attention (B=4, S=2048, E=1024).

Sharding: 8 cores = 4 batches x 2 head-groups (8 heads each); host sums the
two head-group partials (fp32) and adds bo.

Per-core pipeline (fp16/bf16 matmul operands; PSUM accumulates fp32):
  - q^T = Wq_g X^T, k^T = Wk_g X^T    (transposed projections, [dq, S] f16)
  - V   = X^T.T Wv_g^T                (natural [S, dv] bf16, +ones column per
                                       head so PV also yields denominators)
  - scores^T[k, q] at 128x128 causal granularity: fully-masked sub-blocks are
    skipped; each diagonal-crossing sub-block gets one [128,128] additive mask
    matmul (identity stationary, f16 mask moving, NEG=-60000).
  - P^T = exp(scores^T/8) on ACT -> bf16 (range-safe: exp can reach ~1.3e8,
    which overflows f16; masked lanes underflow to exactly 0)
  - PV: out[q, 65] += P^T_block^T V_aug: stationary = P^T [128,128], moving =
    V_aug [128,65] bf16 -> full 128 output partitions at 65 rows/block. One
    PSUM accumulation group per vpa bank (single start/stop; sub-regions
    auto-initialize via the pending-zero mechanism).
  - normalize: DVE reciprocal of the denominator column + tensor_scalar_mul
  - attn [q, dq] f16 -> PE-transpose [dq, q] -> Wo matmul -> f16 partials
Scheduling: the emitter interleaves projection/output-projection work into the
ACT-bound attention windows (deadline queue + PE-vs-ACT balance heuristic),
batches DMAs into ~45 large transfers, and software-pipelines scores/exp/PV
with a lag of one exp group.
"""

import contextlib

import numpy as np

import bass_rust
import concourse.bass as bass
import concourse.mybir as mybir
import concourse.tile as tile

F32 = mybir.dt.float32
F16 = mybir.dt.float16
BF16 = mybir.dt.bfloat16
AF = mybir.ActivationFunctionType

B, S, E = 4, 2048, 1024
H, D = 16, 64
NCORES = 8
NGROUPS = 2            # head groups (tensor parallel)
HPC = H // NGROUPS     # heads per core
DQ = HPC * D           # per-core projection width = 512
NEG = -60000.0         # f16-representable; exp(NEG/8) == 0.0 in fp32

SK = 128               # k sub-block (partition dim of scores^T)
SQ = 512               # q window
GW = 1024              # exp group width (psum [128, GW])


def split_excess_waits(nc, maxw=1):
    """This container's walrus supports one sem wait per instruction;
    hoist extras onto same-engine nops just before the instruction."""
    n_new = 0
    for bb in nc.main_func.blocks:
        new_list = []
        changed = False
        for inst in list(bb.instructions):
            si = inst.sync_info
            waits = list(si.on_wait) if si and si.on_wait else []
            if len(waits) > maxw:
                changed = True
                extra, keep = waits[:-maxw], waits[-maxw:]
                for ci in range(0, len(extra), maxw):
                    nop = bass_rust.InstNoOp(
                        name=f"I-waitsplit-{n_new}", ins=[], outs=[]
                    )
                    n_new += 1
                    nop.engine = inst.engine
                    nop.sync_info = mybir.SyncInfo(
                        on_wait=extra[ci : ci + maxw], on_update=[]
                    )
                    new_list.append(nop)
                inst.sync_info = mybir.SyncInfo(
                    on_wait=keep,
                    on_update=list(si.on_update) if si.on_update else [],
                )
            new_list.append(inst)
        if changed:
            bb.instructions = new_list
    return n_new


def build_kernel(causal=True, split_waits=True, debug=False):
    s, e, hpc, d = S, E, HPC, D
    dq = hpc * d              # 512
    nec = e // 128            # 8 input-feature chunks
    ndq = dq // 128           # 4 projection partition chunks
    nwin = s // SQ            # 4 q windows
    nsc = s // 128            # 16 s chunks

    nc = bass.Bass()

    xq = nc.declare_dram_parameter("xq_t", [e, s], F16, isOutput=False)
    xk = nc.declare_dram_parameter("xk_t", [e, s], F16, isOutput=False)
    xv = nc.declare_dram_parameter("xv_t", [e, s], F16, isOutput=False)
    wqd = nc.declare_dram_parameter("wq_t", [e, dq], F16, isOutput=False)
    wkd = nc.declare_dram_parameter("wk_t", [e, dq], F16, isOutput=False)
    wvd = nc.declare_dram_parameter("wv_t", [e, dq], F16, isOutput=False)
    wod = nc.declare_dram_parameter("wo_t", [dq, e], F16, isOutput=False)
    # packed constants: [bq(4) | bk(4) | bv_b(512)] f32, [ident | crossmask] f16
    cfd = nc.declare_dram_parameter("consts_f32", [128, 2 * ndq + dq], F32,
                                    isOutput=False)
    chd = nc.declare_dram_parameter("consts_f16", [128, 256], F16,
                                    isOutput=False)
    out = nc.declare_dram_parameter("out", [s, e], F16, isOutput=True)
    if debug:
        dbg_q = nc.declare_dram_parameter("dbg_q", [dq, s], F16, isOutput=True)
        dbg_k = nc.declare_dram_parameter("dbg_k", [dq, s], F16, isOutput=True)
        dbg_v = nc.declare_dram_parameter(
            "dbg_v", [s, hpc * (d + 1)], BF16, isOutput=True
        )
        dbg_at = nc.declare_dram_parameter("dbg_at", [s, dq], F16, isOutput=True)
        dbg_pt = nc.declare_dram_parameter("dbg_pt", [128, 17408], BF16,
                                           isOutput=True)
        dbg_rc = nc.declare_dram_parameter("dbg_rc", [128, 16], F32,
                                           isOutput=True)
        dbg_off = [0]

    with tile.TileContext(nc) as tc, contextlib.ExitStack() as ctx:
        pers = ctx.enter_context(tc.tile_pool(name="pers", bufs=1))
        xpool = ctx.enter_context(tc.tile_pool(name="xp", bufs=3))
        ppool = ctx.enter_context(tc.tile_pool(name="ppl", bufs=4))
        atn = ctx.enter_context(tc.tile_pool(name="atn", bufs=4))
        att = ctx.enter_context(tc.tile_pool(name="att", bufs=4))
        nrm = ctx.enter_context(tc.tile_pool(name="nrm", bufs=4))
        opool = ctx.enter_context(tc.tile_pool(name="opl", bufs=3))
        pp = ctx.enter_context(tc.tile_pool(name="pp", bufs=2, space="PSUM"))
        sp = ctx.enter_context(tc.tile_pool(name="sp", bufs=2, space="PSUM"))
        vp = ctx.enter_context(tc.tile_pool(name="vp", bufs=2, space="PSUM"))

        # ---- persistent tensors ----
        cf_sb = pers.tile([128, 2 * ndq + dq], F32, name="cf_sb")
        ch_sb = pers.tile([128, 256], F16, name="ch_sb")
        bq_sb = cf_sb[:, 0:ndq]
        bk_sb = cf_sb[:, ndq : 2 * ndq]
        bv_sb = cf_sb[:, 2 * ndq : 2 * ndq + dq]
        id_sb = ch_sb[:, 0:128]
        mk_sb = ch_sb[:, 128:256]
        q_sb = [
            [pers.tile([128, SQ], F16, name=f"q_sb{c}_{w}") for w in range(nwin)]
            for c in range(ndq)
        ]
        k_sb = [
            [pers.tile([128, SQ], F16, name=f"k_sb{c}_{w}") for w in range(nwin)]
            for c in range(ndq)
        ]
        v_sb = [
            pers.tile([128, hpc * (d + 1)], BF16, name=f"v_sb{i}")
            for i in range(nsc)
        ]
        wq_sb = pers.tile([128, nec * dq], F16, name="wq_sb")
        wk_sb = pers.tile([128, nec * dq], F16, name="wk_sb")
        wv_sb = pers.tile([128, nec * dq], F16, name="wv_sb")
        wo_sb = pers.tile([128, ndq * e], F16, name="wo_sb")

        # ---- DMA helpers (SP engine -> one HWDGE queue, program order) ----
        def load_w_part(wt, dst, part, nparts=2):
            # e-chunk group `part` of [e, dq] -> dst cols
            g = nec // nparts
            src = wt.rearrange("(n p) m -> p n m", p=128)
            nc.sync.dma_start(
                out=dst.rearrange("p (n m) -> p n m", m=dq)[
                    :, part * g : (part + 1) * g, :
                ],
                in_=src[:, part * g : (part + 1) * g, :],
            )

        def load_x_slab(xt, dst, sb, part=None, nparts=2):
            # dst: [128, nec*512] tile; cols [sb*512,(sb+1)*512) of [e, s]
            src = xt.rearrange("(n p) m -> p n m", p=128)
            d3 = dst.rearrange("p (n m) -> p n m", m=SQ)
            if part is None:
                nc.sync.dma_start(
                    out=d3[:, :, :],
                    in_=src[:, :, sb * SQ : (sb + 1) * SQ],
                )
            else:
                g = nec // nparts
                nc.sync.dma_start(
                    out=d3[:, part * g : (part + 1) * g, :],
                    in_=src[:, part * g : (part + 1) * g,
                            sb * SQ : (sb + 1) * SQ],
                )



        x_t = {}  # (tensor, sb) -> slab tile
        for t, xd in (("q", xq), ("k", xk), ("v", xv)):
            x_t[t, 0] = xpool.tile([128, nec * SQ], F16, tag=f"x{t}",
                                   name=f"x{t}0", bufs=3)
        # slab 0 interleaved with weight pieces for earliest unblock;
        # wq/xq0 in quarters so the first projection matmuls start ASAP
        for part in range(4):
            load_w_part(wqd, wq_sb, part, nparts=4)
            load_x_slab(xq, x_t["q", 0], 0, part=part, nparts=4)
        # packed constants (biases for the first bias-add, mask for h0 scores)
        nc.sync.dma_start(out=cf_sb[:, :], in_=cfd[:, :])
        nc.sync.dma_start(out=ch_sb[:, :], in_=chd[:, :])
        load_w_part(wkd, wk_sb, 0)
        load_x_slab(xk, x_t["k", 0], 0, part=0)
        load_w_part(wkd, wk_sb, 1)
        load_x_slab(xk, x_t["k", 0], 0, part=1)
        load_w_part(wvd, wv_sb, 0)
        load_x_slab(xv, x_t["v", 0], 0, part=0)
        load_w_part(wvd, wv_sb, 1)
        load_x_slab(xv, x_t["v", 0], 0, part=1)
        x_t["q", 1] = xpool.tile([128, nec * SQ], F16, tag="xq",
                                 name="xq1", bufs=3)
        load_x_slab(xq, x_t["q", 1], 1)
        for sb in range(1, nwin):
            for t, xd in (("q", xq), ("k", xk), ("v", xv)):
                if (t, sb) in x_t:
                    continue
                x_t[t, sb] = xpool.tile([128, nec * SQ], F16, tag=f"x{t}",
                                        name=f"x{t}{sb}", bufs=3)
                load_x_slab(xd, x_t[t, sb], sb)
            if sb == 1:
                nc.sync.dma_start(
                    out=wo_sb.rearrange("p (n m) -> p n m", m=e),
                    in_=wod.rearrange("(n p) m -> p n m", p=128),
                )

        # ones columns of v_sb, once, on the idle gpsimd engine
        for i in range(nsc):
            v3 = v_sb[i].rearrange("p (h t) -> p h t", t=d + 1)
            nc.gpsimd.memset(v3[:, :, d], 1.0)

        # ---- compute unit generators ----
        def w3(wt):
            return wt.rearrange("p (n m) -> p n m", m=dq)

        def proj_qk_unit(w_sb_t, xt, dst, bias, sb, c):
            """One [128,512] slab-column of a transposed projection."""
            ps = pp.tile([128, SQ], F32, tag="pp", name="ps_pj")
            wv_ = w3(w_sb_t)
            for ec in range(nec):
                nc.tensor.matmul(
                    ps[:, :],
                    wv_[:, ec, c * 128 : (c + 1) * 128],
                    x_t[xt, sb][:, ec * SQ : (ec + 1) * SQ],
                    start=(ec == 0),
                    stop=(ec == nec - 1),
                )
            nc.vector.tensor_scalar_add(
                dst[c][sb][:, :], ps[:, :], bias[:, c : c + 1]
            )

        def proj_v_unit(sb, ii):
            """One [128(s), dq] natural-layout V chunk (i = sb*4+ii)."""
            i = sb * 4 + ii
            ps = pp.tile([128, dq], F32, tag="pp", name="ps_v")
            wv_ = w3(wv_sb)
            for ec in range(nec):
                nc.tensor.matmul(
                    ps[:, :],
                    x_t["v", sb][:, ec * SQ + ii * 128 : ec * SQ + ii * 128 + 128],
                    wv_[:, ec, :],
                    start=(ec == 0),
                    stop=(ec == nec - 1),
                )
            v3 = v_sb[i].rearrange("p (h t) -> p h t", t=d + 1)
            nc.vector.tensor_add(
                v3[:, :, 0:d],
                ps[:, :].rearrange("p (h t) -> p h t", t=d),
                bv_sb[:, :].rearrange("p (h t) -> p h t", t=d),
            )

        # static PE/ACT occupancy estimate driving filler insertion
        eng_ns = {"pe": 0.0, "act": 0.0}

        def pe_rows(n):
            eng_ns["pe"] += n * 0.4167

        def act_cols(n):
            eng_ns["act"] += 1.25 * (n * 0.8333 + 185.0)  # 1.25: tuned filler bias

        def attention_head(qb, h, att_tiles, pre_last_cb=None,
                           act_norm=False):
            """scores+exp+PV+normalize for one (window, head).

            Generator: yields after each score-group / PV emission so the
            driver can interleave PE filler while ACT churns through exps.
            pre_last_cb: emitted right after the last score group (tail
            shortening for the final head). act_norm: do half the normalize
            multiplies on ACT (only sensible when ACT is idle afterwards).
            """
            c, hp = h // 2, (h % 2) * 64
            nkb = 4 * qb + 4 if causal else nsc
            # segments: (kb, qstart_global, width)
            segs = []
            for kb in range(nkb):
                if causal and kb >= 4 * qb:
                    qs = kb * 128
                else:
                    qs = qb * SQ
                segs.append((kb, qs, (qb + 1) * SQ - qs))
            # greedy-pack into exp groups of width <= GW
            groups, cur, curw = [], [], 0
            for seg in segs:
                if curw + seg[2] > GW:
                    groups.append(cur)
                    cur, curw = [], 0
                cur.append(seg)
                curw += seg[2]
            if cur:
                groups.append(cur)

            vpa = vp.tile([128, 4 * (d + 1)], F32, tag="vo", name="vpa")
            last_kb = nkb - 1

            def emit_scores(grp):
                gw = sum(g[2] for g in grp)
                scp = sp.tile([128, GW], F32, tag="sc", name="scp")
                off = 0
                for kb, qs, w in grp:
                    ks = k_sb[c][kb // 4][hp : hp + d,
                                          (kb % 4) * 128 : (kb % 4) * 128 + 128]
                    qw_ = q_sb[c][qs // SQ]
                    if causal and kb >= 4 * qb:
                        # additive mask for the diagonal-crossing sub-block
                        nc.tensor.matmul(scp[:, off : off + 128], id_sb[:, :],
                                         mk_sb[:, :], start=True, stop=False)
                        nc.tensor.matmul(
                            scp[:, off : off + 128], ks,
                            qw_[hp : hp + d, qs % SQ : qs % SQ + 128],
                            start=False, stop=True,
                        )
                        pe_rows(256)
                        if w > 128:
                            nc.tensor.matmul(
                                scp[:, off + 128 : off + w], ks,
                                qw_[hp : hp + d, qs % SQ + 128 : qs % SQ + w],
                                start=True, stop=True,
                            )
                            pe_rows(w - 128)
                    else:
                        nc.tensor.matmul(
                            scp[:, off : off + w], ks,
                            qw_[hp : hp + d, qs % SQ : qs % SQ + w],
                            start=True, stop=True,
                        )
                        pe_rows(w)
                    off += w
                pt = ppool.tile([128, GW], BF16, tag="pt", name="pt")
                nc.scalar.activation(
                    pt[:, 0:gw], scp[:, 0:gw], AF.Exp,
                    scale=float(1.0 / np.sqrt(d)),
                )
                act_cols(gw)
                if debug and h == 0:
                    nc.sync.dma_start(
                        out=dbg_pt[:, dbg_off[0] : dbg_off[0] + gw],
                        in_=pt[:, 0:gw])
                    dbg_off[0] += gw
                return pt

            def emit_pv(grp, pt):
                # One psum accumulation group for the whole vpa bank: a
                # start marks the full 2KB zero-region pending-zero, so only
                # the first matmul may carry start and only the last stop;
                # each sub-region auto-initializes on its first write.
                off = 0
                for kb, qs, w in grp:
                    for qcl in range(4):
                        qg = 4 * qb + qcl           # global q chunk
                        if causal and qg < kb:
                            continue                 # fully masked block
                        boff = off + qcl * 128 + qb * SQ - qs
                        nc.tensor.matmul(
                            vpa[:, qcl * (d + 1) : (qcl + 1) * (d + 1)],
                            pt[:, boff : boff + 128],
                            v_sb[kb][:, h * (d + 1) : (h + 1) * (d + 1)],
                            start=(kb == 0 and qcl == 0),
                            stop=(kb == last_kb and qcl == 3),
                        )
                        pe_rows(d + 1)
                    off += w

            # lag-1 software pipeline: scores g+1 overlaps exp g
            prev = None
            for gi, grp in enumerate(groups):
                pt = emit_scores(grp)
                if pre_last_cb is not None and gi == len(groups) - 1:
                    pre_last_cb()
                yield
                if prev is not None:
                    emit_pv(*prev)
                    yield
                prev = (grp, pt)
            emit_pv(*prev)

            v4 = vpa.rearrange("p (qc t) -> p qc t", t=d + 1)
            rcp = nrm.tile([128, 4], F32, tag="rcp", name="rcp")
            nc.vector.reciprocal(rcp[:, :], v4[:, :, d])
            if debug and h == 0:
                nc.sync.dma_start(out=dbg_rc[:, qb * 4 : qb * 4 + 4],
                                  in_=rcp[:, :])
            for qcl in range(4):
                if act_norm and qcl >= 2:
                    nc.scalar.activation(
                        att_tiles[qcl][:, h * d : (h + 1) * d],
                        v4[:, qcl, 0:d],
                        AF.Copy,
                        scale=rcp[:, qcl : qcl + 1],
                    )
                else:
                    nc.vector.tensor_scalar_mul(
                        att_tiles[qcl][:, h * d : (h + 1) * d],
                        v4[:, qcl, 0:d],
                        rcp[:, qcl : qcl + 1],
                    )

        def wo_transpose_unit(att_tiles, cc, at_store, copy_eng=None):
            """Transpose attn chunk cc (heads 2cc, 2cc+1) -> at_store[cc]."""
            tp = pp.tile([128, SQ], F16, tag="pp", name="tp")
            for qcl in range(4):
                nc.tensor.transpose(
                    tp[:, qcl * 128 : (qcl + 1) * 128],
                    att_tiles[qcl][:, cc * 128 : (cc + 1) * 128],
                    id_sb[:, :],
                )
                pe_rows(128)
            at_ = att.tile([128, SQ], F16, tag=f"at{cc}", name="at_")
            if copy_eng is None:
                nc.vector.tensor_copy(at_[:, :], tp[:, :])
            else:
                copy_eng.copy(at_[:, :], tp[:, :])
            at_store[cc] = at_

        def wo_matmul_unit(at_store, qb, i, copy_eng=None):
            """Output projection + store for s-chunk i of window qb."""
            wo3 = wo_sb.rearrange("p (n m) -> p n m", m=e)
            ot = opool.tile([128, e], F16, tag="ot", name="ot")
            si = qb * 4 + i
            for ob in range(2):
                ps = pp.tile([128, 512], F32, tag="pp", name="ps_o")
                for cc in range(ndq):
                    nc.tensor.matmul(
                        ps[:, :],
                        at_store[cc][:, i * 128 : (i + 1) * 128],
                        wo3[:, cc, ob * 512 : (ob + 1) * 512],
                        start=(cc == 0),
                        stop=(cc == ndq - 1),
                    )
                    pe_rows(512)
                if copy_eng is None:
                    nc.vector.tensor_copy(
                        ot[:, ob * 512 : (ob + 1) * 512], ps[:, :])
                else:
                    copy_eng.copy(ot[:, ob * 512 : (ob + 1) * 512], ps[:, :])
                nc.sync.dma_start(
                    out=out[si * 128 : (si + 1) * 128,
                            ob * 512 : (ob + 1) * 512],
                    in_=ot[:, ob * 512 : (ob + 1) * 512],
                )

        # ---- projection queue, deadline-ordered ----
        # Per window sb: q/k chunk c due just before head 2c; v slab due
        # during head 0's score groups (its diag PV needs it). Deadline key:
        # (sb, h_due) with v at h_due=1 (forced explicitly at h0's yields).
        proj_queue = []
        for sb in range(nwin):
            proj_queue.append((sb, 0, "q", sb, 0))
            proj_queue.append((sb, 0, "k", sb, 0))
            for ii in range(4):
                proj_queue.append((sb, 1, "v", sb, ii))
            for c in range(1, ndq):
                proj_queue.append((sb, 2 * c, "q", sb, c))
                proj_queue.append((sb, 2 * c, "k", sb, c))
        wo_queue = []

        def emit_proj_unit():
            _, _, kind, sb, j = proj_queue.pop(0)
            if kind == "q":
                proj_qk_unit(wq_sb, "q", q_sb, bq_sb, sb, j)
            elif kind == "k":
                proj_qk_unit(wk_sb, "k", k_sb, bk_sb, sb, j)
            else:
                proj_v_unit(sb, j)
            pe_rows(nec * SQ)

        def balance_filler(qb):
            # Keep PE fed while ACT is the pacing engine — but don't consume
            # units whose deadline lets them fill a FUTURE window's ACT-bound
            # stretch (they are the only legal filler there).
            while eng_ns["pe"] < eng_ns["act"]:
                if proj_queue and (
                    (proj_queue[0][0], proj_queue[0][1]) < (qb + 1, 1)
                ):
                    emit_proj_unit()
                elif wo_queue:
                    wo_queue.pop(0)()
                else:
                    return

        def force_due(qb, h):
            while proj_queue and (proj_queue[0][0], proj_queue[0][1]) <= (qb, h):
                emit_proj_unit()

        def wo_full(qb, att_tiles, last=False):
            at_store = [None] * ndq
            for cc in range(ndq):
                wo_transpose_unit(att_tiles, cc, at_store)
            if debug:
                for qcl in range(4):
                    nc.sync.dma_start(
                        out=dbg_at[(qb * 4 + qcl) * 128 :
                                   (qb * 4 + qcl + 1) * 128, :],
                        in_=att_tiles[qcl][:, :],
                    )
            for i in range(4):
                # final window: ACT is idle by now, DVE is not
                wo_matmul_unit(at_store, qb, i,
                               copy_eng=nc.scalar if last else None)

        # ---- emission ----
        # bootstrap: the startup is DMA-bound; emit the units whose inputs
        # arrive first (all of q0 + k0c0) so PE never out-runs the DMA stream
        boot = {("q", 0, 0), ("q", 0, 1), ("q", 0, 2), ("q", 0, 3), ("k", 0, 0)}
        for c in range(ndq):
            proj_qk_unit(wq_sb, "q", q_sb, bq_sb, 0, c)
        proj_qk_unit(wk_sb, "k", k_sb, bk_sb, 0, 0)
        proj_queue = [u for u in proj_queue if (u[2], u[3], u[4]) not in boot]

        prev = None  # deferred (qb, att_tiles, at_store) for wo
        last_store = [None] * ndq
        for qb in range(nwin):
            att_tiles = [
                atn.tile([128, dq], F16, tag=f"an{qcl}", name=f"an{qcl}_{qb}")
                for qcl in range(4)
            ]
            for h in range(hpc):
                force_due(qb, h)
                yi = 0
                for _ in attention_head(qb, h, att_tiles):
                    yi += 1
                    if h == 0 and yi <= 2:
                        # v slab for this window's diagonal, 2 units per yield
                        for _ in range(2):
                            if proj_queue and proj_queue[0][2] == "v" \
                                    and proj_queue[0][3] == qb:
                                emit_proj_unit()
                    balance_filler(qb)
            # defer this window's Wo into the balance queue: it is the only
            # PE work with no deadline, so it belongs in the late ACT-bound
            # holes (atn/att bufs=4 make any emission order inversion-free)
            pqb, ptiles, pstore = qb, att_tiles, [None] * ndq

            def mk_tr(ptiles=ptiles, pstore=pstore, pqb=pqb):
                for cc in range(ndq):
                    wo_transpose_unit(ptiles, cc, pstore)
                if debug:
                    for qcl in range(4):
                        nc.sync.dma_start(
                            out=dbg_at[(pqb * 4 + qcl) * 128 :
                                       (pqb * 4 + qcl + 1) * 128, :],
                            in_=ptiles[qcl][:, :],
                        )

            if qb < nwin - 1:
                wo_queue.append(mk_tr)
                for i in range(4):
                    wo_queue.append(
                        lambda st=pstore, w=pqb, j=i: wo_matmul_unit(st, w, j))
            else:
                prev = (qb, att_tiles)
        while proj_queue:
            emit_proj_unit()
        while wo_queue:
            wo_queue.pop(0)()
        wo_full(*prev, last=True)

        if debug:
            for c in range(ndq):
                for w in range(nwin):
                    cs = slice(c * 128, (c + 1) * 128)
                    ws = slice(w * SQ, (w + 1) * SQ)
                    nc.sync.dma_start(out=dbg_q[cs, ws], in_=q_sb[c][w][:, :])
                    nc.sync.dma_start(out=dbg_k[cs, ws], in_=k_sb[c][w][:, :])
            for i in range(nsc):
                nc.sync.dma_start(
                    out=dbg_v[i * 128 : (i + 1) * 128, :], in_=v_sb[i][:, :]
                )

    if split_waits:
        split_excess_waits(nc)
    return nc


def make_crossmask():
    kk = np.arange(128)[:, None]
    qq = np.arange(128)[None, :]
    return np.where(kk <= qq, 0.0, NEG).astype(np.float16)


def classify_mask(mask):
    m = np.asarray(mask).reshape(S, S)
    if np.array_equal(m, np.tril(np.ones((S, S), bool))):
        return "causal"
    if m.all():
        return "dense"
    return "generic"


def prep_core_inputs(query, key, value, Wq, bq, Wk, bk, Wv, bv, Wo, bo, mask):
    """Shard + lay out host-side numpy inputs for the 8 cores."""
    kind = classify_mask(mask)
    maps = []
    for core in range(NCORES):
        b, gi = core // NGROUPS, core % NGROUPS
        gs = slice(gi * DQ, (gi + 1) * DQ)
        im = {
            "xq_t": np.ascontiguousarray(
                np.asarray(query[b]).T.astype(np.float16)),
            "xk_t": np.ascontiguousarray(
                np.asarray(key[b]).T.astype(np.float16)),
            "xv_t": np.ascontiguousarray(
                np.asarray(value[b]).T.astype(np.float16)),
            "wq_t": np.ascontiguousarray(
                np.asarray(Wq)[gs, :].T.astype(np.float16)),
            "wk_t": np.ascontiguousarray(
                np.asarray(Wk)[gs, :].T.astype(np.float16)),
            "wv_t": np.ascontiguousarray(
                np.asarray(Wv)[gs, :].T.astype(np.float16)),
            "wo_t": np.ascontiguousarray(
                np.asarray(Wo)[:, gs].T.astype(np.float16)),
            "consts_f32": np.ascontiguousarray(np.concatenate([
                np.asarray(bq)[gs].astype(np.float32).reshape(-1, 128).T,
                np.asarray(bk)[gs].astype(np.float32).reshape(-1, 128).T,
                np.broadcast_to(
                    np.asarray(bv)[gs].astype(np.float32), (128, DQ)),
            ], axis=1)),
            "consts_f16": np.ascontiguousarray(np.concatenate([
                np.eye(128, dtype=np.float16), make_crossmask()
            ], axis=1)),
        }
        maps.append(im)
    return maps, kind


def make_runner(nc, n_cores=NCORES):
    """Build a reusable jitted SPMD executor for `nc` on cores 0..n_cores-1."""
    import jax
    from jax.experimental.shard_map import shard_map
    from jax.sharding import Mesh, PartitionSpec

    from concourse import bass2jax, mybir as _mybir

    bass2jax.install_neuronx_cc_hook()

    partition_name = (
        nc.partition_id_tensor.name if nc.partition_id_tensor else None
    )
    in_names, out_names, out_avals, zero_shapes = [], [], [], []
    for alloc in nc.m.functions[0].allocations:
        if not isinstance(alloc, _mybir.MemoryLocationSet):
            continue
        name = alloc.memorylocations[0].name
        if alloc.kind == "ExternalInput":
            if name != partition_name:
                in_names.append(name)
        elif alloc.kind == "ExternalOutput":
            out_names.append(name)
            shape = tuple(alloc.tensor_shape)
            dtype = _mybir.dt.np(alloc.dtype)
            out_avals.append(jax.core.ShapedArray(shape, dtype))
            zero_shapes.append((shape, dtype))
    n_params = len(in_names)
    all_in = list(in_names) + list(out_names)
    if partition_name is not None:
        all_in.append(partition_name)

    def _body(*args):
        operands = list(args)
        if partition_name is not None:
            operands.append(bass2jax.partition_id_tensor())
        outs = bass2jax._bass_exec_p.bind(
            *operands,
            out_avals=tuple(out_avals),
            in_names=tuple(all_in),
            out_names=tuple(out_names),
            lowering_input_output_aliases=(),
            sim_require_finite=True,
            sim_require_nnan=True,
            nc=nc,
        )
        return tuple(outs)

    devices = jax.devices()[:n_cores]
    assert len(devices) == n_cores
    mesh = Mesh(np.asarray(devices), ("core",))
    in_specs = (PartitionSpec("core"),) * (n_params + len(out_names))
    out_specs = (PartitionSpec("core"),) * len(out_names)
    sharded = jax.jit(
        shard_map(
            _body,
            mesh=mesh,
            in_specs=in_specs,
            out_specs=out_specs,
            check_rep=False,
        ),
        keep_unused=True,
    )
    zeros = [
        np.zeros((n_cores * sh[0], *sh[1:]), dt) for sh, dt in zero_shapes
    ]

    def concat_inputs(in_maps):
        return [
            np.concatenate(
                [np.asarray(in_maps[c][n]) for c in range(n_cores)], axis=0
            )
            for n in in_names
        ]

    def run(in_maps):
        out_arrs = sharded(*concat_inputs(in_maps), *zeros)
        return [
            {
                name: np.asarray(out_arrs[i]).reshape(
                    n_cores, *out_avals[i].shape
                )[c]
                for i, name in enumerate(out_names)
            }
            for c in range(n_cores)
        ]

    run.sharded = sharded
    run.concat_inputs = concat_inputs
    run.zeros = zeros
    run.out_names = out_names
    run.out_avals = out_avals
    return run


_CACHE = {}


def get_runner(kind="causal"):
    if kind not in _CACHE:
        nc = build_kernel(causal=(kind == "causal"))
        _CACHE[kind] = make_runner(nc)
    return _CACHE[kind]


def _numpy_reference(query, key, value, Wq, bq, Wk, bk, Wv, bv, Wo, bo, mask):
    q = (query @ Wq.T + bq).reshape(B, S, H, D).transpose(0, 2, 1, 3)
    k = (key @ Wk.T + bk).reshape(B, S, H, D).transpose(0, 2, 1, 3)
    v = (value @ Wv.T + bv).reshape(B, S, H, D).transpose(0, 2, 1, 3)
    sc = np.einsum("bhqd,bhkd->bhqk", q, k) / np.sqrt(D)
    sc = np.where(np.asarray(mask).reshape(1, 1, S, S), sc, -np.inf)
    sc -= sc.max(axis=-1, keepdims=True)
    p = np.exp(sc)
    p /= p.sum(axis=-1, keepdims=True)
    o = np.einsum("bhqk,bhkd->bhqd", p, v)
    o = o.transpose(0, 2, 1, 3).reshape(B, S, E)
    return o @ Wo.T + bo


def kernel(**inputs) -> np.ndarray:
    kind = classify_mask(inputs["mask"])
    if kind == "generic":
        fp = {k: np.asarray(v, np.float32) for k, v in inputs.items()
              if k != "mask"}
        return _numpy_reference(mask=inputs["mask"], **fp).astype(np.float32)
    in_maps, kind = prep_core_inputs(**inputs)
    run = get_runner(kind)
    results = run(in_maps)
    bo = np.asarray(inputs["bo"], dtype=np.float32)
    out = np.empty((B, S, E), dtype=np.float32)
    for b in range(B):
        acc = results[b * NGROUPS]["out"].astype(np.float32)
        for gi in range(1, NGROUPS):
            acc = acc + results[b * NGROUPS + gi]["out"].astype(np.float32)
        out[b] = acc + bo[None, :]
    return out


# revision 6
# speedup vs baseline: 1.0623x; 1.0017x over previous
"""Trainium2 Bass kernel: 16-head causal attention (B=4, S=2048, E=1024).

Sharding: 8 cores = 4 batches x 2 head-groups (8 heads each); host sums the
two head-group partials (fp32) and adds bo.

Per-core pipeline (fp16/bf16 matmul operands; PSUM accumulates fp32):
  - q^T = Wq_g X^T, k^T = Wk_g X^T    (transposed projections, [dq, S] f16)
  - V   = X^T.T Wv_g^T                (natural [S, dv] bf16, +ones column per
                                       head so PV also yields denominators)
  - scores^T[k, q] at 128x128 causal granularity: fully-masked sub-blocks are
    skipped; each diagonal-crossing sub-block gets one [128,128] additive mask
    matmul (identity stationary, f16 mask moving, NEG=-60000).
  - P^T = exp(scores^T/8) on ACT -> bf16 (range-safe: exp can reach ~1.3e8,
    which overflows f16; masked lanes underflow to exactly 0)
  - PV: out[q, 65] += P^T_block^T V_aug: stationary = P^T [128,128], moving =
    V_aug [128,65] bf16 -> full 128 output partitions at 65 rows/block. One
    PSUM accumulation group per vpa bank (single start/stop; sub-regions
    auto-initialize via the pending-zero mechanism).
  - normalize: DVE reciprocal of the denominator column + tensor_scalar_mul
  - attn [q, dq] f16 -> PE-transpose [dq, q] -> Wo matmul -> f16 partials
Scheduling: the emitter interleaves projection/output-projection work into the
ACT-bound attention windows (deadline queue + PE-vs-ACT balance heuristic),
batches DMAs into ~45 large transfers, and software-pipelines scores/exp/PV
with a lag of one exp group.
"""

import contextlib

import numpy as np

import bass_rust
import concourse.bass as bass
import concourse.mybir as mybir
import concourse.tile as tile

F32 = mybir.dt.float32
F16 = mybir.dt.float16
BF16 = mybir.dt.bfloat16
AF = mybir.ActivationFunctionType

B, S, E = 4, 2048, 1024
H, D = 16, 64
NCORES = 8
NGROUPS = 2            # head groups (tensor parallel)
HPC = H // NGROUPS     # heads per core
DQ = HPC * D           # per-core projection width = 512
NEG = -60000.0         # f16-representable; exp(NEG/8) == 0.0 in fp32

SK = 128               # k sub-block (partition dim of scores^T)
SQ = 512               # q window
GW = 1024              # exp group width (psum [128, GW])


def split_excess_waits(nc, maxw=1):
    """This container's walrus supports one sem wait per instruction;
    hoist extras onto same-engine nops just before the instruction."""
    n_new = 0
    for bb in nc.main_func.blocks:
        new_list = []
        changed = False
        for inst in list(bb.instructions):
            si = inst.sync_info
            waits = list(si.on_wait) if si and si.on_wait else []
            if len(waits) > maxw:
                changed = True
                extra, keep = waits[:-maxw], waits[-maxw:]
                for ci in range(0, len(extra), maxw):
                    nop = bass_rust.InstNoOp(
                        name=f"I-waitsplit-{n_new}", ins=[], outs=[]
                    )
                    n_new += 1
                    nop.engine = inst.engine
                    nop.sync_info = mybir.SyncInfo(
                        on_wait=extra[ci : ci + maxw], on_update=[]
                    )
                    new_list.append(nop)
                inst.sync_info = mybir.SyncInfo(
                    on_wait=keep,
                    on_update=list(si.on_update) if si.on_update else [],
                )
            new_list.append(inst)
        if changed:
            bb.instructions = new_list
    return n_new


def build_kernel(causal=True, split_waits=True, debug=False):
    s, e, hpc, d = S, E, HPC, D
    dq = hpc * d              # 512
    nec = e // 128            # 8 input-feature chunks
    ndq = dq // 128           # 4 projection partition chunks
    nwin = s // SQ            # 4 q windows
    nsc = s // 128            # 16 s chunks

    nc = bass.Bass()

    xq = nc.declare_dram_parameter("xq_t", [e, s], F16, isOutput=False)
    xk = nc.declare_dram_parameter("xk_t", [e, s], F16, isOutput=False)
    xv = nc.declare_dram_parameter("xv_t", [e, s], F16, isOutput=False)
    wqd = nc.declare_dram_parameter("wq_t", [e, dq], F16, isOutput=False)
    wkd = nc.declare_dram_parameter("wk_t", [e, dq], F16, isOutput=False)
    wvd = nc.declare_dram_parameter("wv_t", [e, dq], F16, isOutput=False)
    wod = nc.declare_dram_parameter("wo_t", [dq, e], F16, isOutput=False)
    # packed constants: [bq(4) | bk(4) | bv_b(512)] f32, [ident | crossmask] f16
    cfd = nc.declare_dram_parameter("consts_f32", [128, 2 * ndq + dq], F32,
                                    isOutput=False)
    chd = nc.declare_dram_parameter("consts_f16", [128, 256], F16,
                                    isOutput=False)
    out = nc.declare_dram_parameter("out", [s, e], F16, isOutput=True)
    if debug:
        dbg_q = nc.declare_dram_parameter("dbg_q", [dq, s], F16, isOutput=True)
        dbg_k = nc.declare_dram_parameter("dbg_k", [dq, s], F16, isOutput=True)
        dbg_v = nc.declare_dram_parameter(
            "dbg_v", [s, hpc * (d + 1)], BF16, isOutput=True
        )
        dbg_at = nc.declare_dram_parameter("dbg_at", [s, dq], F16, isOutput=True)
        dbg_pt = nc.declare_dram_parameter("dbg_pt", [128, 17408], BF16,
                                           isOutput=True)
        dbg_rc = nc.declare_dram_parameter("dbg_rc", [128, 16], F32,
                                           isOutput=True)
        dbg_off = [0]

    with tile.TileContext(nc) as tc, contextlib.ExitStack() as ctx:
        pers = ctx.enter_context(tc.tile_pool(name="pers", bufs=1))
        xpool = ctx.enter_context(tc.tile_pool(name="xp", bufs=3))
        ppool = ctx.enter_context(tc.tile_pool(name="ppl", bufs=4))
        atn = ctx.enter_context(tc.tile_pool(name="atn", bufs=4))
        att = ctx.enter_context(tc.tile_pool(name="att", bufs=4))
        nrm = ctx.enter_context(tc.tile_pool(name="nrm", bufs=4))
        opool = ctx.enter_context(tc.tile_pool(name="opl", bufs=3))
        pp = ctx.enter_context(tc.tile_pool(name="pp", bufs=2, space="PSUM"))
        sp = ctx.enter_context(tc.tile_pool(name="sp", bufs=2, space="PSUM"))
        vp = ctx.enter_context(tc.tile_pool(name="vp", bufs=2, space="PSUM"))

        # ---- persistent tensors ----
        cf_sb = pers.tile([128, 2 * ndq + dq], F32, name="cf_sb")
        ch_sb = pers.tile([128, 256], F16, name="ch_sb")
        bq_sb = cf_sb[:, 0:ndq]
        bk_sb = cf_sb[:, ndq : 2 * ndq]
        bv_sb = cf_sb[:, 2 * ndq : 2 * ndq + dq]
        id_sb = ch_sb[:, 0:128]
        mk_sb = ch_sb[:, 128:256]
        q_sb = [
            [pers.tile([128, SQ], F16, name=f"q_sb{c}_{w}") for w in range(nwin)]
            for c in range(ndq)
        ]
        k_sb = [
            [pers.tile([128, SQ], F16, name=f"k_sb{c}_{w}") for w in range(nwin)]
            for c in range(ndq)
        ]
        v_sb = [
            pers.tile([128, hpc * (d + 1)], BF16, name=f"v_sb{i}")
            for i in range(nsc)
        ]
        wq_sb = pers.tile([128, nec * dq], F16, name="wq_sb")
        wk_sb = pers.tile([128, nec * dq], F16, name="wk_sb")
        wv_sb = pers.tile([128, nec * dq], F16, name="wv_sb")
        wo_sb = pers.tile([128, ndq * e], F16, name="wo_sb")

        # ---- DMA helpers (SP engine -> one HWDGE queue, program order) ----
        def load_w_part(wt, dst, part, nparts=2):
            # e-chunk group `part` of [e, dq] -> dst cols
            g = nec // nparts
            src = wt.rearrange("(n p) m -> p n m", p=128)
            nc.sync.dma_start(
                out=dst.rearrange("p (n m) -> p n m", m=dq)[
                    :, part * g : (part + 1) * g, :
                ],
                in_=src[:, part * g : (part + 1) * g, :],
            )

        def load_x_slab(xt, dst, sb, part=None, nparts=2):
            # dst: [128, nec*512] tile; cols [sb*512,(sb+1)*512) of [e, s]
            src = xt.rearrange("(n p) m -> p n m", p=128)
            d3 = dst.rearrange("p (n m) -> p n m", m=SQ)
            if part is None:
                nc.sync.dma_start(
                    out=d3[:, :, :],
                    in_=src[:, :, sb * SQ : (sb + 1) * SQ],
                )
            else:
                g = nec // nparts
                nc.sync.dma_start(
                    out=d3[:, part * g : (part + 1) * g, :],
                    in_=src[:, part * g : (part + 1) * g,
                            sb * SQ : (sb + 1) * SQ],
                )



        x_t = {}  # (tensor, sb) -> slab tile
        for t, xd in (("q", xq), ("k", xk), ("v", xv)):
            x_t[t, 0] = xpool.tile([128, nec * SQ], F16, tag=f"x{t}",
                                   name=f"x{t}0", bufs=3)
        # slab 0 interleaved with weight pieces for earliest unblock;
        # wq/xq0 in quarters so the first projection matmuls start ASAP
        for part in range(4):
            load_w_part(wqd, wq_sb, part, nparts=4)
            load_x_slab(xq, x_t["q", 0], 0, part=part, nparts=4)
        # packed constants (biases for the first bias-add, mask for h0 scores)
        nc.sync.dma_start(out=cf_sb[:, :], in_=cfd[:, :])
        nc.sync.dma_start(out=ch_sb[:, :], in_=chd[:, :])
        load_w_part(wkd, wk_sb, 0)
        load_x_slab(xk, x_t["k", 0], 0, part=0)
        load_w_part(wkd, wk_sb, 1)
        load_x_slab(xk, x_t["k", 0], 0, part=1)
        load_w_part(wvd, wv_sb, 0)
        load_x_slab(xv, x_t["v", 0], 0, part=0)
        load_w_part(wvd, wv_sb, 1)
        load_x_slab(xv, x_t["v", 0], 0, part=1)
        x_t["q", 1] = xpool.tile([128, nec * SQ], F16, tag="xq",
                                 name="xq1", bufs=3)
        load_x_slab(xq, x_t["q", 1], 1)
        for sb in range(1, nwin):
            for t, xd in (("q", xq), ("k", xk), ("v", xv)):
                if (t, sb) in x_t:
                    continue
                x_t[t, sb] = xpool.tile([128, nec * SQ], F16, tag=f"x{t}",
                                        name=f"x{t}{sb}", bufs=3)
                load_x_slab(xd, x_t[t, sb], sb)
            if sb == 1:
                nc.sync.dma_start(
                    out=wo_sb.rearrange("p (n m) -> p n m", m=e),
                    in_=wod.rearrange("(n p) m -> p n m", p=128),
                )

        # ones columns of v_sb, once, on the idle gpsimd engine
        for i in range(nsc):
            v3 = v_sb[i].rearrange("p (h t) -> p h t", t=d + 1)
            nc.gpsimd.memset(v3[:, :, d], 1.0)

        # ---- compute unit generators ----
        def w3(wt):
            return wt.rearrange("p (n m) -> p n m", m=dq)

        open_ps = {}

        def proj_qk_phase(w_sb_t, xt, dst, bias, sb, c, phase):
            """Half-contraction phase of a q/k projection unit. Phase 0
            allocates the psum tile and contracts ec 0..3; phase 1 finishes
            ec 4..7 and applies the bias. Between a unit's phases at most one
            other pp allocation may occur (pp bufs=2)."""
            key = ("qk", xt, sb, c)
            if phase == 0:
                ps = pp.tile([128, SQ], F32, tag="pp", name="ps_pj")
                open_ps[key] = ps
                ecs = range(0, nec // 2)
            else:
                ps = open_ps.pop(key)
                ecs = range(nec // 2, nec)
            for ec in ecs:
                nc.tensor.matmul(
                    ps[:, :],
                    w3(w_sb_t)[:, ec, c * 128 : (c + 1) * 128],
                    x_t[xt, sb][:, ec * SQ : (ec + 1) * SQ],
                    start=(ec == 0),
                    stop=(ec == nec - 1),
                )
            pe_rows(nec * SQ // 2)
            if phase == 1:
                nc.vector.tensor_scalar_add(
                    dst[c][sb][:, :], ps[:, :], bias[:, c : c + 1]
                )

        def proj_v_phase(sb, ii, phase):
            key = ("v", sb, ii)
            if phase == 0:
                ps = pp.tile([128, dq], F32, tag="pp", name="ps_v")
                open_ps[key] = ps
                ecs = range(0, nec // 2)
            else:
                ps = open_ps.pop(key)
                ecs = range(nec // 2, nec)
            wv_ = w3(wv_sb)
            for ec in ecs:
                nc.tensor.matmul(
                    ps[:, :],
                    x_t["v", sb][:, ec * SQ + ii * 128 : ec * SQ + ii * 128 + 128],
                    wv_[:, ec, :],
                    start=(ec == 0),
                    stop=(ec == nec - 1),
                )
            pe_rows(nec * SQ // 2)
            if phase == 1:
                i = sb * 4 + ii
                v3 = v_sb[i].rearrange("p (h t) -> p h t", t=d + 1)
                nc.vector.tensor_add(
                    v3[:, :, 0:d],
                    ps[:, :].rearrange("p (h t) -> p h t", t=d),
                    bv_sb[:, :].rearrange("p (h t) -> p h t", t=d),
                )

        def proj_qk_unit(w_sb_t, xt, dst, bias, sb, c):
            """One [128,512] slab-column of a transposed projection."""
            ps = pp.tile([128, SQ], F32, tag="pp", name="ps_pj")
            wv_ = w3(w_sb_t)
            for ec in range(nec):
                nc.tensor.matmul(
                    ps[:, :],
                    wv_[:, ec, c * 128 : (c + 1) * 128],
                    x_t[xt, sb][:, ec * SQ : (ec + 1) * SQ],
                    start=(ec == 0),
                    stop=(ec == nec - 1),
                )
            nc.vector.tensor_scalar_add(
                dst[c][sb][:, :], ps[:, :], bias[:, c : c + 1]
            )

        def proj_v_unit(sb, ii):
            """One [128(s), dq] natural-layout V chunk (i = sb*4+ii)."""
            i = sb * 4 + ii
            ps = pp.tile([128, dq], F32, tag="pp", name="ps_v")
            wv_ = w3(wv_sb)
            for ec in range(nec):
                nc.tensor.matmul(
                    ps[:, :],
                    x_t["v", sb][:, ec * SQ + ii * 128 : ec * SQ + ii * 128 + 128],
                    wv_[:, ec, :],
                    start=(ec == 0),
                    stop=(ec == nec - 1),
                )
            v3 = v_sb[i].rearrange("p (h t) -> p h t", t=d + 1)
            nc.vector.tensor_add(
                v3[:, :, 0:d],
                ps[:, :].rearrange("p (h t) -> p h t", t=d),
                bv_sb[:, :].rearrange("p (h t) -> p h t", t=d),
            )

        # static PE/ACT occupancy estimate driving filler insertion
        eng_ns = {"pe": 0.0, "act": 0.0}

        def pe_rows(n):
            eng_ns["pe"] += n * 0.4167

        def act_cols(n):
            eng_ns["act"] += 1.25 * (n * 0.8333 + 185.0)  # 1.25: tuned filler bias

        def attention_head(qb, h, att_tiles, pre_last_cb=None,
                           act_norm=False):
            """scores+exp+PV+normalize for one (window, head).

            Generator: yields after each score-group / PV emission so the
            driver can interleave PE filler while ACT churns through exps.
            pre_last_cb: emitted right after the last score group (tail
            shortening for the final head). act_norm: do half the normalize
            multiplies on ACT (only sensible when ACT is idle afterwards).
            """
            c, hp = h // 2, (h % 2) * 64
            nkb = 4 * qb + 4 if causal else nsc
            # segments: (kb, qstart_global, width)
            segs = []
            for kb in range(nkb):
                if causal and kb >= 4 * qb:
                    qs = kb * 128
                else:
                    qs = qb * SQ
                segs.append((kb, qs, (qb + 1) * SQ - qs))
            # greedy-pack into exp groups of width <= GW
            groups, cur, curw = [], [], 0
            for seg in segs:
                if curw + seg[2] > GW:
                    groups.append(cur)
                    cur, curw = [], 0
                cur.append(seg)
                curw += seg[2]
            if cur:
                groups.append(cur)

            vpa = vp.tile([128, 4 * (d + 1)], F32, tag="vo", name="vpa")
            last_kb = nkb - 1

            def emit_scores(grp):
                gw = sum(g[2] for g in grp)
                scp = sp.tile([128, GW], F32, tag="sc", name="scp")
                off = 0
                for kb, qs, w in grp:
                    ks = k_sb[c][kb // 4][hp : hp + d,
                                          (kb % 4) * 128 : (kb % 4) * 128 + 128]
                    qw_ = q_sb[c][qs // SQ]
                    if causal and kb >= 4 * qb:
                        # additive mask for the diagonal-crossing sub-block
                        nc.tensor.matmul(scp[:, off : off + 128], id_sb[:, :],
                                         mk_sb[:, :], start=True, stop=False)
                        nc.tensor.matmul(
                            scp[:, off : off + 128], ks,
                            qw_[hp : hp + d, qs % SQ : qs % SQ + 128],
                            start=False, stop=True,
                        )
                        pe_rows(256)
                        if w > 128:
                            nc.tensor.matmul(
                                scp[:, off + 128 : off + w], ks,
                                qw_[hp : hp + d, qs % SQ + 128 : qs % SQ + w],
                                start=True, stop=True,
                            )
                            pe_rows(w - 128)
                    else:
                        nc.tensor.matmul(
                            scp[:, off : off + w], ks,
                            qw_[hp : hp + d, qs % SQ : qs % SQ + w],
                            start=True, stop=True,
                        )
                        pe_rows(w)
                    off += w
                pt = ppool.tile([128, GW], BF16, tag="pt", name="pt")
                nc.scalar.activation(
                    pt[:, 0:gw], scp[:, 0:gw], AF.Exp,
                    scale=float(1.0 / np.sqrt(d)),
                )
                act_cols(gw)
                if debug and h == 0:
                    nc.sync.dma_start(
                        out=dbg_pt[:, dbg_off[0] : dbg_off[0] + gw],
                        in_=pt[:, 0:gw])
                    dbg_off[0] += gw
                return pt

            def emit_pv(grp, pt):
                # One psum accumulation group for the whole vpa bank: a
                # start marks the full 2KB zero-region pending-zero, so only
                # the first matmul may carry start and only the last stop;
                # each sub-region auto-initializes on its first write.
                off = 0
                for kb, qs, w in grp:
                    for qcl in range(4):
                        qg = 4 * qb + qcl           # global q chunk
                        if causal and qg < kb:
                            continue                 # fully masked block
                        boff = off + qcl * 128 + qb * SQ - qs
                        nc.tensor.matmul(
                            vpa[:, qcl * (d + 1) : (qcl + 1) * (d + 1)],
                            pt[:, boff : boff + 128],
                            v_sb[kb][:, h * (d + 1) : (h + 1) * (d + 1)],
                            start=(kb == 0 and qcl == 0),
                            stop=(kb == last_kb and qcl == 3),
                        )
                        pe_rows(d + 1)
                    off += w

            # lag-1 software pipeline: scores g+1 overlaps exp g
            prev = None
            for gi, grp in enumerate(groups):
                pt = emit_scores(grp)
                if pre_last_cb is not None and gi == len(groups) - 1:
                    pre_last_cb()
                yield
                if prev is not None:
                    emit_pv(*prev)
                    yield
                prev = (grp, pt)
            emit_pv(*prev)

            v4 = vpa.rearrange("p (qc t) -> p qc t", t=d + 1)
            rcp = nrm.tile([128, 4], F32, tag="rcp", name="rcp")
            nc.vector.reciprocal(rcp[:, :], v4[:, :, d])
            if debug and h == 0:
                nc.sync.dma_start(out=dbg_rc[:, qb * 4 : qb * 4 + 4],
                                  in_=rcp[:, :])
            for qcl in range(4):
                if act_norm and qcl >= 2:
                    nc.scalar.activation(
                        att_tiles[qcl][:, h * d : (h + 1) * d],
                        v4[:, qcl, 0:d],
                        AF.Copy,
                        scale=rcp[:, qcl : qcl + 1],
                    )
                else:
                    nc.vector.tensor_scalar_mul(
                        att_tiles[qcl][:, h * d : (h + 1) * d],
                        v4[:, qcl, 0:d],
                        rcp[:, qcl : qcl + 1],
                    )

        def wo_transpose_unit(att_tiles, cc, at_store, copy_eng=None):
            """Transpose attn chunk cc (heads 2cc, 2cc+1) -> at_store[cc]."""
            tp = pp.tile([128, SQ], F16, tag="pp", name="tp")
            for qcl in range(4):
                nc.tensor.transpose(
                    tp[:, qcl * 128 : (qcl + 1) * 128],
                    att_tiles[qcl][:, cc * 128 : (cc + 1) * 128],
                    id_sb[:, :],
                )
                pe_rows(128)
            at_ = att.tile([128, SQ], F16, tag=f"at{cc}", name="at_")
            if copy_eng is None:
                nc.vector.tensor_copy(at_[:, :], tp[:, :])
            else:
                copy_eng.copy(at_[:, :], tp[:, :])
            at_store[cc] = at_

        def wo_matmul_unit(at_store, qb, i, copy_eng=None):
            """Output projection + store for s-chunk i of window qb."""
            wo3 = wo_sb.rearrange("p (n m) -> p n m", m=e)
            ot = opool.tile([128, e], F16, tag="ot", name="ot")
            si = qb * 4 + i
            for ob in range(2):
                ps = pp.tile([128, 512], F32, tag="pp", name="ps_o")
                for cc in range(ndq):
                    nc.tensor.matmul(
                        ps[:, :],
                        at_store[cc][:, i * 128 : (i + 1) * 128],
                        wo3[:, cc, ob * 512 : (ob + 1) * 512],
                        start=(cc == 0),
                        stop=(cc == ndq - 1),
                    )
                    pe_rows(512)
                if copy_eng is None:
                    nc.vector.tensor_copy(
                        ot[:, ob * 512 : (ob + 1) * 512], ps[:, :])
                else:
                    copy_eng.copy(ot[:, ob * 512 : (ob + 1) * 512], ps[:, :])
                nc.sync.dma_start(
                    out=out[si * 128 : (si + 1) * 128,
                            ob * 512 : (ob + 1) * 512],
                    in_=ot[:, ob * 512 : (ob + 1) * 512],
                )

        # ---- projection queue, deadline-ordered ----
        # Per window sb: q/k chunk c due just before head 2c; v slab due
        # during head 0's score groups (its diag PV needs it). Deadline key:
        # (sb, h_due) with v at h_due=1 (forced explicitly at h0's yields).
        proj_queue = []
        for sb in range(nwin):
            proj_queue.append((sb, 0, "q", sb, 0))
            proj_queue.append((sb, 0, "k", sb, 0))
            for ii in range(4):
                proj_queue.append((sb, 1, "v", sb, ii))
            for c in range(1, ndq):
                proj_queue.append((sb, 2 * c, "q", sb, c))
                proj_queue.append((sb, 2 * c, "k", sb, c))
        wo_queue = []

        def emit_proj_unit():
            _, _, kind, sb, j = proj_queue.pop(0)
            if kind == "q":
                proj_qk_unit(wq_sb, "q", q_sb, bq_sb, sb, j)
            elif kind == "k":
                proj_qk_unit(wk_sb, "k", k_sb, bk_sb, sb, j)
            else:
                proj_v_unit(sb, j)
            pe_rows(nec * SQ)

        def balance_filler(qb):
            # Keep PE fed while ACT is the pacing engine — but don't consume
            # units whose deadline lets them fill a FUTURE window's ACT-bound
            # stretch (they are the only legal filler there).
            if open_ps:
                return  # a phase-split unit owns a pp slot; don't rotate pp
            while eng_ns["pe"] < eng_ns["act"]:
                if proj_queue and (
                    (proj_queue[0][0], proj_queue[0][1]) < (qb + 1, 1)
                ):
                    emit_proj_unit()
                elif wo_queue:
                    wo_queue.pop(0)()
                else:
                    return

        def force_due(qb, h):
            while proj_queue and (proj_queue[0][0], proj_queue[0][1]) <= (qb, h):
                emit_proj_unit()

        def wo_full(qb, att_tiles, last=False):
            at_store = [None] * ndq
            for cc in range(ndq):
                wo_transpose_unit(att_tiles, cc, at_store)
            if debug:
                for qcl in range(4):
                    nc.sync.dma_start(
                        out=dbg_at[(qb * 4 + qcl) * 128 :
                                   (qb * 4 + qcl + 1) * 128, :],
                        in_=att_tiles[qcl][:, :],
                    )
            for i in range(4):
                # final window: ACT is idle by now, DVE is not
                wo_matmul_unit(at_store, qb, i,
                               copy_eng=nc.scalar if last else None)

        # ---- emission ----
        # bootstrap: the startup is DMA-bound; emit phase-split units in
        # A,A,B,B order so every unit's first contraction half runs while
        # the second DMA halves are still in flight
        boot = {("q", 0, 0), ("q", 0, 1), ("q", 0, 2), ("q", 0, 3),
                ("k", 0, 0), ("k", 0, 1), ("v", 0, 0), ("v", 0, 1),
                ("v", 0, 2), ("v", 0, 3)}
        for c0, c1 in ((0, 1), (2, 3)):
            proj_qk_phase(wq_sb, "q", q_sb, bq_sb, 0, c0, 0)
            proj_qk_phase(wq_sb, "q", q_sb, bq_sb, 0, c1, 0)
            proj_qk_phase(wq_sb, "q", q_sb, bq_sb, 0, c0, 1)
            proj_qk_phase(wq_sb, "q", q_sb, bq_sb, 0, c1, 1)
        proj_qk_phase(wk_sb, "k", k_sb, bk_sb, 0, 0, 0)
        proj_qk_phase(wk_sb, "k", k_sb, bk_sb, 0, 1, 0)
        proj_qk_phase(wk_sb, "k", k_sb, bk_sb, 0, 0, 1)
        proj_qk_phase(wk_sb, "k", k_sb, bk_sb, 0, 1, 1)
        proj_queue = [u for u in proj_queue if (u[2], u[3], u[4]) not in boot]

        prev = None  # deferred (qb, att_tiles, at_store) for wo
        last_store = [None] * ndq
        for qb in range(nwin):
            att_tiles = [
                atn.tile([128, dq], F16, tag=f"an{qcl}", name=f"an{qcl}_{qb}")
                for qcl in range(4)
            ]
            for h in range(hpc):
                force_due(qb, h)
                yi = 0
                for _ in attention_head(qb, h, att_tiles):
                    yi += 1
                    if h == 0 and qb == 0:
                        # window 0's v slab is still streaming in: run the
                        # first contraction halves while the rest arrives
                        if yi == 1:
                            proj_v_phase(0, 0, 0)
                            proj_v_phase(0, 1, 0)
                        elif yi == 2:
                            proj_v_phase(0, 0, 1)
                            proj_v_phase(0, 1, 1)
                        elif yi == 3:
                            proj_v_phase(0, 2, 0)
                            proj_v_phase(0, 3, 0)
                            proj_v_phase(0, 2, 1)
                            proj_v_phase(0, 3, 1)
                    elif h == 0 and yi <= 2:
                        # v slab for this window's diagonal, 2 units per yield
                        for _ in range(2):
                            if proj_queue and proj_queue[0][2] == "v" \
                                    and proj_queue[0][3] == qb:
                                emit_proj_unit()
                    balance_filler(qb)
            # defer this window's Wo into the balance queue: it is the only
            # PE work with no deadline, so it belongs in the late ACT-bound
            # holes (atn/att bufs=4 make any emission order inversion-free)
            pqb, ptiles, pstore = qb, att_tiles, [None] * ndq

            def mk_tr(ptiles=ptiles, pstore=pstore, pqb=pqb):
                for cc in range(ndq):
                    wo_transpose_unit(ptiles, cc, pstore)
                if debug:
                    for qcl in range(4):
                        nc.sync.dma_start(
                            out=dbg_at[(pqb * 4 + qcl) * 128 :
                                       (pqb * 4 + qcl + 1) * 128, :],
                            in_=ptiles[qcl][:, :],
                        )

            if qb < nwin - 1:
                wo_queue.append(mk_tr)
                for i in range(4):
                    wo_queue.append(
                        lambda st=pstore, w=pqb, j=i: wo_matmul_unit(st, w, j))
            else:
                prev = (qb, att_tiles)
        while proj_queue:
            emit_proj_unit()
        while wo_queue:
            wo_queue.pop(0)()
        wo_full(*prev, last=True)

        if debug:
            for c in range(ndq):
                for w in range(nwin):
                    cs = slice(c * 128, (c + 1) * 128)
                    ws = slice(w * SQ, (w + 1) * SQ)
                    nc.sync.dma_start(out=dbg_q[cs, ws], in_=q_sb[c][w][:, :])
                    nc.sync.dma_start(out=dbg_k[cs, ws], in_=k_sb[c][w][:, :])
            for i in range(nsc):
                nc.sync.dma_start(
                    out=dbg_v[i * 128 : (i + 1) * 128, :], in_=v_sb[i][:, :]
                )

    if split_waits:
        split_excess_waits(nc)
    return nc


def make_crossmask():
    kk = np.arange(128)[:, None]
    qq = np.arange(128)[None, :]
    return np.where(kk <= qq, 0.0, NEG).astype(np.float16)


def classify_mask(mask):
    m = np.asarray(mask).reshape(S, S)
    if np.array_equal(m, np.tril(np.ones((S, S), bool))):
        return "causal"
    if m.all():
        return "dense"
    return "generic"


def prep_core_inputs(query, key, value, Wq, bq, Wk, bk, Wv, bv, Wo, bo, mask):
    """Shard + lay out host-side numpy inputs for the 8 cores."""
    kind = classify_mask(mask)
    maps = []
    for core in range(NCORES):
        b, gi = core // NGROUPS, core % NGROUPS
        gs = slice(gi * DQ, (gi + 1) * DQ)
        im = {
            "xq_t": np.ascontiguousarray(
                np.asarray(query[b]).T.astype(np.float16)),
            "xk_t": np.ascontiguousarray(
                np.asarray(key[b]).T.astype(np.float16)),
            "xv_t": np.ascontiguousarray(
                np.asarray(value[b]).T.astype(np.float16)),
            "wq_t": np.ascontiguousarray(
                np.asarray(Wq)[gs, :].T.astype(np.float16)),
            "wk_t": np.ascontiguousarray(
                np.asarray(Wk)[gs, :].T.astype(np.float16)),
            "wv_t": np.ascontiguousarray(
                np.asarray(Wv)[gs, :].T.astype(np.float16)),
            "wo_t": np.ascontiguousarray(
                np.asarray(Wo)[:, gs].T.astype(np.float16)),
            "consts_f32": np.ascontiguousarray(np.concatenate([
                np.asarray(bq)[gs].astype(np.float32).reshape(-1, 128).T,
                np.asarray(bk)[gs].astype(np.float32).reshape(-1, 128).T,
                np.broadcast_to(
                    np.asarray(bv)[gs].astype(np.float32), (128, DQ)),
            ], axis=1)),
            "consts_f16": np.ascontiguousarray(np.concatenate([
                np.eye(128, dtype=np.float16), make_crossmask()
            ], axis=1)),
        }
        maps.append(im)
    return maps, kind


def make_runner(nc, n_cores=NCORES):
    """Build a reusable jitted SPMD executor for `nc` on cores 0..n_cores-1."""
    import jax
    from jax.experimental.shard_map import shard_map
    from jax.sharding import Mesh, PartitionSpec

    from concourse import bass2jax, mybir as _mybir

    bass2jax.install_neuronx_cc_hook()

    partition_name = (
        nc.partition_id_tensor.name if nc.partition_id_tensor else None
    )
    in_names, out_names, out_avals, zero_shapes = [], [], [], []
    for alloc in nc.m.functions[0].allocations:
        if not isinstance(alloc, _mybir.MemoryLocationSet):
            continue
        name = alloc.memorylocations[0].name
        if alloc.kind == "ExternalInput":
            if name != partition_name:
                in_names.append(name)
        elif alloc.kind == "ExternalOutput":
            out_names.append(name)
            shape = tuple(alloc.tensor_shape)
            dtype = _mybir.dt.np(alloc.dtype)
            out_avals.append(jax.core.ShapedArray(shape, dtype))
            zero_shapes.append((shape, dtype))
    n_params = len(in_names)
    all_in = list(in_names) + list(out_names)
    if partition_name is not None:
        all_in.append(partition_name)

    def _body(*args):
        operands = list(args)
        if partition_name is not None:
            operands.append(bass2jax.partition_id_tensor())
        outs = bass2jax._bass_exec_p.bind(
            *operands,
            out_avals=tuple(out_avals),
            in_names=tuple(all_in),
            out_names=tuple(out_names),
            lowering_input_output_aliases=(),
            sim_require_finite=True,
            sim_require_nnan=True,
            nc=nc,
        )
        return tuple(outs)

    devices = jax.devices()[:n_cores]
    assert len(devices) == n_cores
    mesh = Mesh(np.asarray(devices), ("core",))
    in_specs = (PartitionSpec("core"),) * (n_params + len(out_names))
    out_specs = (PartitionSpec("core"),) * len(out_names)
    sharded = jax.jit(
        shard_map(
            _body,
            mesh=mesh,
            in_specs=in_specs,
            out_specs=out_specs,
            check_rep=False,
        ),
        keep_unused=True,
    )
    zeros = [
        np.zeros((n_cores * sh[0], *sh[1:]), dt) for sh, dt in zero_shapes
    ]

    def concat_inputs(in_maps):
        return [
            np.concatenate(
                [np.asarray(in_maps[c][n]) for c in range(n_cores)], axis=0
            )
            for n in in_names
        ]

    def run(in_maps):
        out_arrs = sharded(*concat_inputs(in_maps), *zeros)
        return [
            {
                name: np.asarray(out_arrs[i]).reshape(
                    n_cores, *out_avals[i].shape
                )[c]
                for i, name in enumerate(out_names)
            }
            for c in range(n_cores)
        ]

    run.sharded = sharded
    run.concat_inputs = concat_inputs
    run.zeros = zeros
    run.out_names = out_names
    run.out_avals = out_avals
    return run


_CACHE = {}


def get_runner(kind="causal"):
    if kind not in _CACHE:
        nc = build_kernel(causal=(kind == "causal"))
        _CACHE[kind] = make_runner(nc)
    return _CACHE[kind]


def _numpy_reference(query, key, value, Wq, bq, Wk, bk, Wv, bv, Wo, bo, mask):
    q = (query @ Wq.T + bq).reshape(B, S, H, D).transpose(0, 2, 1, 3)
    k = (key @ Wk.T + bk).reshape(B, S, H, D).transpose(0, 2, 1, 3)
    v = (value @ Wv.T + bv).reshape(B, S, H, D).transpose(0, 2, 1, 3)
    sc = np.einsum("bhqd,bhkd->bhqk", q, k) / np.sqrt(D)
    sc = np.where(np.asarray(mask).reshape(1, 1, S, S), sc, -np.inf)
    sc -= sc.max(axis=-1, keepdims=True)
    p = np.exp(sc)
    p /= p.sum(axis=-1, keepdims=True)
    o = np.einsum("bhqk,bhkd->bhqd", p, v)
    o = o.transpose(0, 2, 1, 3).reshape(B, S, E)
    return o @ Wo.T + bo


def kernel(**inputs) -> np.ndarray:
    kind = classify_mask(inputs["mask"])
    if kind == "generic":
        fp = {k: np.asarray(v, np.float32) for k, v in inputs.items()
              if k != "mask"}
        return _numpy_reference(mask=inputs["mask"], **fp).astype(np.float32)
    in_maps, kind = prep_core_inputs(**inputs)
    run = get_runner(kind)
    results = run(in_maps)
    bo = np.asarray(inputs["bo"], dtype=np.float32)
    out = np.empty((B, S, E), dtype=np.float32)
    for b in range(B):
        acc = results[b * NGROUPS]["out"].astype(np.float32)
        for gi in range(1, NGROUPS):
            acc = acc + results[b * NGROUPS + gi]["out"].astype(np.float32)
        out[b] = acc + bo[None, :]
    return out


# revision 7
# speedup vs baseline: 1.0690x; 1.0063x over previous
"""Trainium2 Bass kernel: 16-head causal attention (B=4, S=2048, E=1024).

Sharding: 8 cores = 4 batches x 2 head-groups (8 heads each); host sums the
two head-group partials (fp32) and adds bo.

Per-core pipeline (fp16/bf16 matmul operands; PSUM accumulates fp32):
  - q^T = Wq_g X^T, k^T = Wk_g X^T    (transposed projections, [dq, S] f16)
  - V   = X^T.T Wv_g^T                (natural [S, dv] bf16, +ones column per
                                       head so PV also yields denominators)
  - scores^T[k, q] at 128x128 causal granularity: fully-masked sub-blocks are
    skipped; each diagonal-crossing sub-block gets one [128,128] additive mask
    matmul (identity stationary, f16 mask moving, NEG=-60000).
  - P^T = exp(scores^T/8) on ACT -> bf16 (range-safe: exp can reach ~1.3e8,
    which overflows f16; masked lanes underflow to exactly 0)
  - PV: out[q, 65] += P^T_block^T V_aug: stationary = P^T [128,128], moving =
    V_aug [128,65] bf16 -> full 128 output partitions at 65 rows/block. One
    PSUM accumulation group per vpa bank (single start/stop; sub-regions
    auto-initialize via the pending-zero mechanism).
  - normalize: DVE reciprocal of the denominator column + tensor_scalar_mul
  - attn [q, dq] f16 -> PE-transpose [dq, q] -> Wo matmul -> f16 partials
Scheduling: the emitter interleaves projection/output-projection work into the
ACT-bound attention windows (deadline queue + PE-vs-ACT balance heuristic),
batches DMAs into ~45 large transfers, and software-pipelines scores/exp/PV
with a lag of one exp group.
"""

import contextlib

import numpy as np

import bass_rust
import concourse.bass as bass
import concourse.mybir as mybir
import concourse.tile as tile

F32 = mybir.dt.float32
F16 = mybir.dt.float16
BF16 = mybir.dt.bfloat16
AF = mybir.ActivationFunctionType

B, S, E = 4, 2048, 1024
H, D = 16, 64
NCORES = 8
NGROUPS = 2            # head groups (tensor parallel)
HPC = H // NGROUPS     # heads per core
DQ = HPC * D           # per-core projection width = 512
NEG = -60000.0         # f16-representable; exp(NEG/8) == 0.0 in fp32

SK = 128               # k sub-block (partition dim of scores^T)
SQ = 512               # q window
GW = 1024              # exp group width (psum [128, GW])


def split_excess_waits(nc, maxw=1):
    """This container's walrus supports one sem wait per instruction;
    hoist extras onto same-engine nops just before the instruction."""
    n_new = 0
    for bb in nc.main_func.blocks:
        new_list = []
        changed = False
        for inst in list(bb.instructions):
            si = inst.sync_info
            waits = list(si.on_wait) if si and si.on_wait else []
            if len(waits) > maxw:
                changed = True
                extra, keep = waits[:-maxw], waits[-maxw:]
                for ci in range(0, len(extra), maxw):
                    nop = bass_rust.InstNoOp(
                        name=f"I-waitsplit-{n_new}", ins=[], outs=[]
                    )
                    n_new += 1
                    nop.engine = inst.engine
                    nop.sync_info = mybir.SyncInfo(
                        on_wait=extra[ci : ci + maxw], on_update=[]
                    )
                    new_list.append(nop)
                inst.sync_info = mybir.SyncInfo(
                    on_wait=keep,
                    on_update=list(si.on_update) if si.on_update else [],
                )
            new_list.append(inst)
        if changed:
            bb.instructions = new_list
    return n_new


def build_kernel(causal=True, split_waits=True, debug=False):
    s, e, hpc, d = S, E, HPC, D
    dq = hpc * d              # 512
    nec = e // 128            # 8 input-feature chunks
    ndq = dq // 128           # 4 projection partition chunks
    nwin = s // SQ            # 4 q windows
    nsc = s // 128            # 16 s chunks

    nc = bass.Bass()

    xq = nc.declare_dram_parameter("xq_t", [e, s], F16, isOutput=False)
    xk = nc.declare_dram_parameter("xk_t", [e, s], F16, isOutput=False)
    xv = nc.declare_dram_parameter("xv_t", [e, s], F16, isOutput=False)
    wqd = nc.declare_dram_parameter("wq_t", [e, dq], F16, isOutput=False)
    wkd = nc.declare_dram_parameter("wk_t", [e, dq], F16, isOutput=False)
    wvd = nc.declare_dram_parameter("wv_t", [e, dq], F16, isOutput=False)
    wod = nc.declare_dram_parameter("wo_t", [dq, e], F16, isOutput=False)
    # packed constants: [bq(4) | bk(4) | bv_b(512)] f32, [ident | crossmask] f16
    cfd = nc.declare_dram_parameter("consts_f32", [128, 2 * ndq + dq], F32,
                                    isOutput=False)
    chd = nc.declare_dram_parameter("consts_f16", [128, 256], F16,
                                    isOutput=False)
    out = nc.declare_dram_parameter("out", [s, e], F16, isOutput=True)
    if debug:
        dbg_q = nc.declare_dram_parameter("dbg_q", [dq, s], F16, isOutput=True)
        dbg_k = nc.declare_dram_parameter("dbg_k", [dq, s], F16, isOutput=True)
        dbg_v = nc.declare_dram_parameter(
            "dbg_v", [s, hpc * (d + 1)], BF16, isOutput=True
        )
        dbg_at = nc.declare_dram_parameter("dbg_at", [s, dq], F16, isOutput=True)
        dbg_pt = nc.declare_dram_parameter("dbg_pt", [128, 17408], BF16,
                                           isOutput=True)
        dbg_rc = nc.declare_dram_parameter("dbg_rc", [128, 16], F32,
                                           isOutput=True)
        dbg_off = [0]

    with tile.TileContext(nc) as tc, contextlib.ExitStack() as ctx:
        pers = ctx.enter_context(tc.tile_pool(name="pers", bufs=1))
        xpool = ctx.enter_context(tc.tile_pool(name="xp", bufs=3))
        ppool = ctx.enter_context(tc.tile_pool(name="ppl", bufs=4))
        atn = ctx.enter_context(tc.tile_pool(name="atn", bufs=4))
        att = ctx.enter_context(tc.tile_pool(name="att", bufs=4))
        nrm = ctx.enter_context(tc.tile_pool(name="nrm", bufs=4))
        opool = ctx.enter_context(tc.tile_pool(name="opl", bufs=3))
        pp = ctx.enter_context(tc.tile_pool(name="pp", bufs=2, space="PSUM"))
        sp = ctx.enter_context(tc.tile_pool(name="sp", bufs=2, space="PSUM"))
        vp = ctx.enter_context(tc.tile_pool(name="vp", bufs=2, space="PSUM"))

        # ---- persistent tensors ----
        cf_sb = pers.tile([128, 2 * ndq + dq], F32, name="cf_sb")
        ch_sb = pers.tile([128, 256], F16, name="ch_sb")
        bq_sb = cf_sb[:, 0:ndq]
        bk_sb = cf_sb[:, ndq : 2 * ndq]
        bv_sb = cf_sb[:, 2 * ndq : 2 * ndq + dq]
        id_sb = ch_sb[:, 0:128]
        mk_sb = ch_sb[:, 128:256]
        q_sb = [
            [pers.tile([128, SQ], F16, name=f"q_sb{c}_{w}") for w in range(nwin)]
            for c in range(ndq)
        ]
        k_sb = [
            [pers.tile([128, SQ], F16, name=f"k_sb{c}_{w}") for w in range(nwin)]
            for c in range(ndq)
        ]
        v_sb = [
            pers.tile([128, hpc * (d + 1)], BF16, name=f"v_sb{i}")
            for i in range(nsc)
        ]
        wq_sb = pers.tile([128, nec * dq], F16, name="wq_sb")
        wk_sb = pers.tile([128, nec * dq], F16, name="wk_sb")
        wv_sb = pers.tile([128, nec * dq], F16, name="wv_sb")
        wo_sb = pers.tile([128, ndq * e], F16, name="wo_sb")

        # ---- DMA helpers (SP engine -> one HWDGE queue, program order) ----
        def load_w_part(wt, dst, part, nparts=2):
            # e-chunk group `part` of [e, dq] -> dst cols
            g = nec // nparts
            src = wt.rearrange("(n p) m -> p n m", p=128)
            nc.sync.dma_start(
                out=dst.rearrange("p (n m) -> p n m", m=dq)[
                    :, part * g : (part + 1) * g, :
                ],
                in_=src[:, part * g : (part + 1) * g, :],
            )

        def load_x_slab(xt, dst, sb, part=None, nparts=2):
            # dst: [128, nec*512] tile; cols [sb*512,(sb+1)*512) of [e, s]
            src = xt.rearrange("(n p) m -> p n m", p=128)
            d3 = dst.rearrange("p (n m) -> p n m", m=SQ)
            if part is None:
                nc.sync.dma_start(
                    out=d3[:, :, :],
                    in_=src[:, :, sb * SQ : (sb + 1) * SQ],
                )
            else:
                g = nec // nparts
                nc.sync.dma_start(
                    out=d3[:, part * g : (part + 1) * g, :],
                    in_=src[:, part * g : (part + 1) * g,
                            sb * SQ : (sb + 1) * SQ],
                )



        x_t = {}  # (tensor, sb) -> slab tile
        for t, xd in (("q", xq), ("k", xk), ("v", xv)):
            x_t[t, 0] = xpool.tile([128, nec * SQ], F16, tag=f"x{t}",
                                   name=f"x{t}0", bufs=3)
        # slab 0 interleaved with weight pieces for earliest unblock;
        # wq/xq0 in quarters so the first projection matmuls start ASAP
        for part in range(4):
            load_w_part(wqd, wq_sb, part, nparts=4)
            load_x_slab(xq, x_t["q", 0], 0, part=part, nparts=4)
        # packed constants (biases for the first bias-add, mask for h0 scores)
        nc.sync.dma_start(out=cf_sb[:, :], in_=cfd[:, :])
        nc.sync.dma_start(out=ch_sb[:, :], in_=chd[:, :])
        load_w_part(wkd, wk_sb, 0)
        load_x_slab(xk, x_t["k", 0], 0, part=0)
        load_w_part(wkd, wk_sb, 1)
        load_x_slab(xk, x_t["k", 0], 0, part=1)
        load_w_part(wvd, wv_sb, 0)
        load_x_slab(xv, x_t["v", 0], 0, part=0)
        load_w_part(wvd, wv_sb, 1)
        load_x_slab(xv, x_t["v", 0], 0, part=1)
        x_t["q", 1] = xpool.tile([128, nec * SQ], F16, tag="xq",
                                 name="xq1", bufs=3)
        load_x_slab(xq, x_t["q", 1], 1)
        for sb in range(1, nwin):
            for t, xd in (("q", xq), ("k", xk), ("v", xv)):
                if (t, sb) in x_t:
                    continue
                x_t[t, sb] = xpool.tile([128, nec * SQ], F16, tag=f"x{t}",
                                        name=f"x{t}{sb}", bufs=3)
                load_x_slab(xd, x_t[t, sb], sb)
            if sb == 1:
                nc.sync.dma_start(
                    out=wo_sb.rearrange("p (n m) -> p n m", m=e),
                    in_=wod.rearrange("(n p) m -> p n m", p=128),
                )

        # ones columns of v_sb, once, on the idle gpsimd engine
        for i in range(nsc):
            v3 = v_sb[i].rearrange("p (h t) -> p h t", t=d + 1)
            nc.gpsimd.memset(v3[:, :, d], 1.0)

        # ---- compute unit generators ----
        def w3(wt):
            return wt.rearrange("p (n m) -> p n m", m=dq)

        open_ps = {}

        def proj_qk_phase(w_sb_t, xt, dst, bias, sb, c, phase):
            """Half-contraction phase of a q/k projection unit. Phase 0
            allocates the psum tile and contracts ec 0..3; phase 1 finishes
            ec 4..7 and applies the bias. Between a unit's phases at most one
            other pp allocation may occur (pp bufs=2)."""
            key = ("qk", xt, sb, c)
            if phase == 0:
                ps = pp.tile([128, SQ], F32, tag="pp", name="ps_pj")
                open_ps[key] = ps
                ecs = range(0, nec // 2)
            else:
                ps = open_ps.pop(key)
                ecs = range(nec // 2, nec)
            for ec in ecs:
                nc.tensor.matmul(
                    ps[:, :],
                    w3(w_sb_t)[:, ec, c * 128 : (c + 1) * 128],
                    x_t[xt, sb][:, ec * SQ : (ec + 1) * SQ],
                    start=(ec == 0),
                    stop=(ec == nec - 1),
                )
            pe_rows(nec * SQ // 2)
            if phase == 1:
                nc.vector.tensor_scalar_add(
                    dst[c][sb][:, :], ps[:, :], bias[:, c : c + 1]
                )

        def proj_v_phase(sb, ii, phase):
            key = ("v", sb, ii)
            if phase == 0:
                ps = pp.tile([128, dq], F32, tag="pp", name="ps_v")
                open_ps[key] = ps
                ecs = range(0, nec // 2)
            else:
                ps = open_ps.pop(key)
                ecs = range(nec // 2, nec)
            wv_ = w3(wv_sb)
            for ec in ecs:
                nc.tensor.matmul(
                    ps[:, :],
                    x_t["v", sb][:, ec * SQ + ii * 128 : ec * SQ + ii * 128 + 128],
                    wv_[:, ec, :],
                    start=(ec == 0),
                    stop=(ec == nec - 1),
                )
            pe_rows(nec * SQ // 2)
            if phase == 1:
                i = sb * 4 + ii
                v3 = v_sb[i].rearrange("p (h t) -> p h t", t=d + 1)
                nc.vector.tensor_add(
                    v3[:, :, 0:d],
                    ps[:, :].rearrange("p (h t) -> p h t", t=d),
                    bv_sb[:, :].rearrange("p (h t) -> p h t", t=d),
                )

        def proj_qk_unit(w_sb_t, xt, dst, bias, sb, c):
            """One [128,512] slab-column of a transposed projection."""
            ps = pp.tile([128, SQ], F32, tag="pp", name="ps_pj")
            wv_ = w3(w_sb_t)
            for ec in range(nec):
                nc.tensor.matmul(
                    ps[:, :],
                    wv_[:, ec, c * 128 : (c + 1) * 128],
                    x_t[xt, sb][:, ec * SQ : (ec + 1) * SQ],
                    start=(ec == 0),
                    stop=(ec == nec - 1),
                )
            nc.vector.tensor_scalar_add(
                dst[c][sb][:, :], ps[:, :], bias[:, c : c + 1]
            )

        def proj_v_unit(sb, ii):
            """One [128(s), dq] natural-layout V chunk (i = sb*4+ii)."""
            i = sb * 4 + ii
            ps = pp.tile([128, dq], F32, tag="pp", name="ps_v")
            wv_ = w3(wv_sb)
            for ec in range(nec):
                nc.tensor.matmul(
                    ps[:, :],
                    x_t["v", sb][:, ec * SQ + ii * 128 : ec * SQ + ii * 128 + 128],
                    wv_[:, ec, :],
                    start=(ec == 0),
                    stop=(ec == nec - 1),
                )
            v3 = v_sb[i].rearrange("p (h t) -> p h t", t=d + 1)
            nc.vector.tensor_add(
                v3[:, :, 0:d],
                ps[:, :].rearrange("p (h t) -> p h t", t=d),
                bv_sb[:, :].rearrange("p (h t) -> p h t", t=d),
            )

        # static PE/ACT occupancy estimate driving filler insertion
        eng_ns = {"pe": 0.0, "act": 0.0}

        def pe_rows(n):
            eng_ns["pe"] += n * 0.4167

        def act_cols(n):
            eng_ns["act"] += 1.25 * (n * 0.8333 + 185.0)  # 1.25: tuned filler bias

        def attention_head(qb, h, att_tiles, pre_last_cb=None,
                           act_norm=False):
            """scores+exp+PV+normalize for one (window, head).

            Generator: yields after each score-group / PV emission so the
            driver can interleave PE filler while ACT churns through exps.
            pre_last_cb: emitted right after the last score group (tail
            shortening for the final head). act_norm: do half the normalize
            multiplies on ACT (only sensible when ACT is idle afterwards).
            """
            c, hp = h // 2, (h % 2) * 64
            nkb = 4 * qb + 4 if causal else nsc
            # segments: (kb, qstart_global, width)
            segs = []
            for kb in range(nkb):
                if causal and kb >= 4 * qb:
                    qs = kb * 128
                else:
                    qs = qb * SQ
                segs.append((kb, qs, (qb + 1) * SQ - qs))
            # greedy-pack into exp groups of width <= GW
            groups, cur, curw = [], [], 0
            for seg in segs:
                if curw + seg[2] > GW:
                    groups.append(cur)
                    cur, curw = [], 0
                cur.append(seg)
                curw += seg[2]
            if cur:
                groups.append(cur)
            if len(groups) > 1:
                # smallest group first: its short exp lands while ACT still
                # drains the previous head, instead of bubbling at head end
                groups = groups[-2:] + groups[:-2]

            vpa = vp.tile([128, 4 * (d + 1)], F32, tag="vo", name="vpa")
            last_kb = nkb - 1
            npv = sum(
                1 for kb in range(nkb) for qcl in range(4)
                if not (causal and 4 * qb + qcl < kb))
            pv_n = [0]

            def emit_scores(grp):
                gw = sum(g[2] for g in grp)
                scp = sp.tile([128, GW], F32, tag="sc", name="scp")
                off = 0
                for kb, qs, w in grp:
                    ks = k_sb[c][kb // 4][hp : hp + d,
                                          (kb % 4) * 128 : (kb % 4) * 128 + 128]
                    qw_ = q_sb[c][qs // SQ]
                    if causal and kb >= 4 * qb:
                        # additive mask for the diagonal-crossing sub-block
                        nc.tensor.matmul(scp[:, off : off + 128], id_sb[:, :],
                                         mk_sb[:, :], start=True, stop=False)
                        nc.tensor.matmul(
                            scp[:, off : off + 128], ks,
                            qw_[hp : hp + d, qs % SQ : qs % SQ + 128],
                            start=False, stop=True,
                        )
                        pe_rows(256)
                        if w > 128:
                            nc.tensor.matmul(
                                scp[:, off + 128 : off + w], ks,
                                qw_[hp : hp + d, qs % SQ + 128 : qs % SQ + w],
                                start=True, stop=True,
                            )
                            pe_rows(w - 128)
                    else:
                        nc.tensor.matmul(
                            scp[:, off : off + w], ks,
                            qw_[hp : hp + d, qs % SQ : qs % SQ + w],
                            start=True, stop=True,
                        )
                        pe_rows(w)
                    off += w
                pt = ppool.tile([128, GW], BF16, tag="pt", name="pt")
                nc.scalar.activation(
                    pt[:, 0:gw], scp[:, 0:gw], AF.Exp,
                    scale=float(1.0 / np.sqrt(d)),
                )
                act_cols(gw)
                if debug and h == 0:
                    nc.sync.dma_start(
                        out=dbg_pt[:, dbg_off[0] : dbg_off[0] + gw],
                        in_=pt[:, 0:gw])
                    dbg_off[0] += gw
                return pt

            def emit_pv(grp, pt):
                # One psum accumulation group for the whole vpa bank: a
                # start marks the full 2KB zero-region pending-zero, so only
                # the first matmul may carry start and only the last stop;
                # each sub-region auto-initializes on its first write.
                off = 0
                for kb, qs, w in grp:
                    for qcl in range(4):
                        qg = 4 * qb + qcl           # global q chunk
                        if causal and qg < kb:
                            continue                 # fully masked block
                        boff = off + qcl * 128 + qb * SQ - qs
                        nc.tensor.matmul(
                            vpa[:, qcl * (d + 1) : (qcl + 1) * (d + 1)],
                            pt[:, boff : boff + 128],
                            v_sb[kb][:, h * (d + 1) : (h + 1) * (d + 1)],
                            start=(pv_n[0] == 0),
                            stop=(pv_n[0] == npv - 1),
                        )
                        pv_n[0] += 1
                        pe_rows(d + 1)
                    off += w

            # lag-1 software pipeline: scores g+1 overlaps exp g
            prev = None
            for gi, grp in enumerate(groups):
                pt = emit_scores(grp)
                if pre_last_cb is not None and gi == len(groups) - 1:
                    pre_last_cb()
                yield
                if prev is not None:
                    emit_pv(*prev)
                    yield
                prev = (grp, pt)
            emit_pv(*prev)

            v4 = vpa.rearrange("p (qc t) -> p qc t", t=d + 1)
            rcp = nrm.tile([128, 4], F32, tag="rcp", name="rcp")
            nc.vector.reciprocal(rcp[:, :], v4[:, :, d])
            if debug and h == 0:
                nc.sync.dma_start(out=dbg_rc[:, qb * 4 : qb * 4 + 4],
                                  in_=rcp[:, :])
            for qcl in range(4):
                if act_norm and qcl >= 2:
                    nc.scalar.activation(
                        att_tiles[qcl][:, h * d : (h + 1) * d],
                        v4[:, qcl, 0:d],
                        AF.Copy,
                        scale=rcp[:, qcl : qcl + 1],
                    )
                else:
                    nc.vector.tensor_scalar_mul(
                        att_tiles[qcl][:, h * d : (h + 1) * d],
                        v4[:, qcl, 0:d],
                        rcp[:, qcl : qcl + 1],
                    )

        def wo_transpose_unit(att_tiles, cc, at_store, copy_eng=None):
            """Transpose attn chunk cc (heads 2cc, 2cc+1) -> at_store[cc]."""
            tp = pp.tile([128, SQ], F16, tag="pp", name="tp")
            for qcl in range(4):
                nc.tensor.transpose(
                    tp[:, qcl * 128 : (qcl + 1) * 128],
                    att_tiles[qcl][:, cc * 128 : (cc + 1) * 128],
                    id_sb[:, :],
                )
                pe_rows(128)
            at_ = att.tile([128, SQ], F16, tag=f"at{cc}", name="at_")
            if copy_eng is None:
                nc.vector.tensor_copy(at_[:, :], tp[:, :])
            else:
                copy_eng.copy(at_[:, :], tp[:, :])
            at_store[cc] = at_

        def wo_matmul_unit(at_store, qb, i, copy_eng=None):
            """Output projection + store for s-chunk i of window qb."""
            wo3 = wo_sb.rearrange("p (n m) -> p n m", m=e)
            ot = opool.tile([128, e], F16, tag="ot", name="ot")
            si = qb * 4 + i
            for ob in range(2):
                ps = pp.tile([128, 512], F32, tag="pp", name="ps_o")
                for cc in range(ndq):
                    nc.tensor.matmul(
                        ps[:, :],
                        at_store[cc][:, i * 128 : (i + 1) * 128],
                        wo3[:, cc, ob * 512 : (ob + 1) * 512],
                        start=(cc == 0),
                        stop=(cc == ndq - 1),
                    )
                    pe_rows(512)
                if copy_eng is None:
                    nc.vector.tensor_copy(
                        ot[:, ob * 512 : (ob + 1) * 512], ps[:, :])
                else:
                    copy_eng.copy(ot[:, ob * 512 : (ob + 1) * 512], ps[:, :])
                nc.sync.dma_start(
                    out=out[si * 128 : (si + 1) * 128,
                            ob * 512 : (ob + 1) * 512],
                    in_=ot[:, ob * 512 : (ob + 1) * 512],
                )

        # ---- projection queue, deadline-ordered ----
        # Per window sb: q/k chunk c due just before head 2c; v slab due
        # during head 0's score groups (its diag PV needs it). Deadline key:
        # (sb, h_due) with v at h_due=1 (forced explicitly at h0's yields).
        proj_queue = []
        for sb in range(nwin):
            proj_queue.append((sb, 0, "q", sb, 0))
            proj_queue.append((sb, 0, "k", sb, 0))
            for ii in range(4):
                proj_queue.append((sb, 1, "v", sb, ii))
            for c in range(1, ndq):
                proj_queue.append((sb, 2 * c, "q", sb, c))
                proj_queue.append((sb, 2 * c, "k", sb, c))
        wo_queue = []

        def emit_proj_unit():
            _, _, kind, sb, j = proj_queue.pop(0)
            if kind == "q":
                proj_qk_unit(wq_sb, "q", q_sb, bq_sb, sb, j)
            elif kind == "k":
                proj_qk_unit(wk_sb, "k", k_sb, bk_sb, sb, j)
            else:
                proj_v_unit(sb, j)
            pe_rows(nec * SQ)

        def balance_filler(qb):
            # Keep PE fed while ACT is the pacing engine — but don't consume
            # units whose deadline lets them fill a FUTURE window's ACT-bound
            # stretch (they are the only legal filler there).
            if open_ps:
                return  # a phase-split unit owns a pp slot; don't rotate pp
            while eng_ns["pe"] < eng_ns["act"]:
                if proj_queue and (
                    (proj_queue[0][0], proj_queue[0][1]) < (qb + 1, 1)
                ):
                    emit_proj_unit()
                elif wo_queue:
                    wo_queue.pop(0)()
                else:
                    return

        def force_due(qb, h):
            while proj_queue and (proj_queue[0][0], proj_queue[0][1]) <= (qb, h):
                emit_proj_unit()

        def wo_full(qb, att_tiles, last=False):
            at_store = [None] * ndq
            for cc in range(ndq):
                wo_transpose_unit(att_tiles, cc, at_store)
            if debug:
                for qcl in range(4):
                    nc.sync.dma_start(
                        out=dbg_at[(qb * 4 + qcl) * 128 :
                                   (qb * 4 + qcl + 1) * 128, :],
                        in_=att_tiles[qcl][:, :],
                    )
            for i in range(4):
                # final window: ACT is idle by now, DVE is not
                wo_matmul_unit(at_store, qb, i,
                               copy_eng=nc.scalar if last else None)

        # ---- emission ----
        # bootstrap: the startup is DMA-bound; emit phase-split units in
        # A,A,B,B order so every unit's first contraction half runs while
        # the second DMA halves are still in flight
        boot = {("q", 0, 0), ("q", 0, 1), ("q", 0, 2), ("q", 0, 3),
                ("k", 0, 0), ("k", 0, 1), ("v", 0, 0), ("v", 0, 1),
                ("v", 0, 2), ("v", 0, 3)}
        for c0, c1 in ((0, 1), (2, 3)):
            proj_qk_phase(wq_sb, "q", q_sb, bq_sb, 0, c0, 0)
            proj_qk_phase(wq_sb, "q", q_sb, bq_sb, 0, c1, 0)
            proj_qk_phase(wq_sb, "q", q_sb, bq_sb, 0, c0, 1)
            proj_qk_phase(wq_sb, "q", q_sb, bq_sb, 0, c1, 1)
        proj_qk_phase(wk_sb, "k", k_sb, bk_sb, 0, 0, 0)
        proj_qk_phase(wk_sb, "k", k_sb, bk_sb, 0, 1, 0)
        proj_qk_phase(wk_sb, "k", k_sb, bk_sb, 0, 0, 1)
        proj_qk_phase(wk_sb, "k", k_sb, bk_sb, 0, 1, 1)
        proj_queue = [u for u in proj_queue if (u[2], u[3], u[4]) not in boot]

        prev = None  # deferred (qb, att_tiles, at_store) for wo
        last_store = [None] * ndq
        for qb in range(nwin):
            att_tiles = [
                atn.tile([128, dq], F16, tag=f"an{qcl}", name=f"an{qcl}_{qb}")
                for qcl in range(4)
            ]
            for h in range(hpc):
                force_due(qb, h)
                yi = 0
                for _ in attention_head(qb, h, att_tiles):
                    yi += 1
                    if h == 0 and qb == 0:
                        # window 0's v slab is still streaming in: run the
                        # first contraction halves while the rest arrives
                        if yi == 1:
                            proj_v_phase(0, 0, 0)
                            proj_v_phase(0, 1, 0)
                        elif yi == 2:
                            proj_v_phase(0, 0, 1)
                            proj_v_phase(0, 1, 1)
                        elif yi == 3:
                            proj_v_phase(0, 2, 0)
                            proj_v_phase(0, 3, 0)
                            proj_v_phase(0, 2, 1)
                            proj_v_phase(0, 3, 1)
                    elif h == 0 and yi <= 2:
                        # v slab for this window's diagonal, 2 units per yield
                        for _ in range(2):
                            if proj_queue and proj_queue[0][2] == "v" \
                                    and proj_queue[0][3] == qb:
                                emit_proj_unit()
                    balance_filler(qb)
            # defer this window's Wo into the balance queue: it is the only
            # PE work with no deadline, so it belongs in the late ACT-bound
            # holes (atn/att bufs=4 make any emission order inversion-free)
            pqb, ptiles, pstore = qb, att_tiles, [None] * ndq

            def mk_tr(ptiles=ptiles, pstore=pstore, pqb=pqb):
                for cc in range(ndq):
                    wo_transpose_unit(ptiles, cc, pstore)
                if debug:
                    for qcl in range(4):
                        nc.sync.dma_start(
                            out=dbg_at[(pqb * 4 + qcl) * 128 :
                                       (pqb * 4 + qcl + 1) * 128, :],
                            in_=ptiles[qcl][:, :],
                        )

            if qb < nwin - 1:
                wo_queue.append(mk_tr)
                for i in range(4):
                    wo_queue.append(
                        lambda st=pstore, w=pqb, j=i: wo_matmul_unit(st, w, j))
            else:
                prev = (qb, att_tiles)
        while proj_queue:
            emit_proj_unit()
        while wo_queue:
            wo_queue.pop(0)()
        wo_full(*prev, last=True)

        if debug:
            for c in range(ndq):
                for w in range(nwin):
                    cs = slice(c * 128, (c + 1) * 128)
                    ws = slice(w * SQ, (w + 1) * SQ)
                    nc.sync.dma_start(out=dbg_q[cs, ws], in_=q_sb[c][w][:, :])
                    nc.sync.dma_start(out=dbg_k[cs, ws], in_=k_sb[c][w][:, :])
            for i in range(nsc):
                nc.sync.dma_start(
                    out=dbg_v[i * 128 : (i + 1) * 128, :], in_=v_sb[i][:, :]
                )

    if split_waits:
        split_excess_waits(nc)
    return nc


def make_crossmask():
    kk = np.arange(128)[:, None]
    qq = np.arange(128)[None, :]
    return np.where(kk <= qq, 0.0, NEG).astype(np.float16)


def classify_mask(mask):
    m = np.asarray(mask).reshape(S, S)
    if np.array_equal(m, np.tril(np.ones((S, S), bool))):
        return "causal"
    if m.all():
        return "dense"
    return "generic"


def prep_core_inputs(query, key, value, Wq, bq, Wk, bk, Wv, bv, Wo, bo, mask):
    """Shard + lay out host-side numpy inputs for the 8 cores."""
    kind = classify_mask(mask)
    maps = []
    for core in range(NCORES):
        b, gi = core // NGROUPS, core % NGROUPS
        gs = slice(gi * DQ, (gi + 1) * DQ)
        im = {
            "xq_t": np.ascontiguousarray(
                np.asarray(query[b]).T.astype(np.float16)),
            "xk_t": np.ascontiguousarray(
                np.asarray(key[b]).T.astype(np.float16)),
            "xv_t": np.ascontiguousarray(
                np.asarray(value[b]).T.astype(np.float16)),
            "wq_t": np.ascontiguousarray(
                np.asarray(Wq)[gs, :].T.astype(np.float16)),
            "wk_t": np.ascontiguousarray(
                np.asarray(Wk)[gs, :].T.astype(np.float16)),
            "wv_t": np.ascontiguousarray(
                np.asarray(Wv)[gs, :].T.astype(np.float16)),
            "wo_t": np.ascontiguousarray(
                np.asarray(Wo)[:, gs].T.astype(np.float16)),
            "consts_f32": np.ascontiguousarray(np.concatenate([
                np.asarray(bq)[gs].astype(np.float32).reshape(-1, 128).T,
                np.asarray(bk)[gs].astype(np.float32).reshape(-1, 128).T,
                np.broadcast_to(
                    np.asarray(bv)[gs].astype(np.float32), (128, DQ)),
            ], axis=1)),
            "consts_f16": np.ascontiguousarray(np.concatenate([
                np.eye(128, dtype=np.float16), make_crossmask()
            ], axis=1)),
        }
        maps.append(im)
    return maps, kind


def make_runner(nc, n_cores=NCORES):
    """Build a reusable jitted SPMD executor for `nc` on cores 0..n_cores-1."""
    import jax
    from jax.experimental.shard_map import shard_map
    from jax.sharding import Mesh, PartitionSpec

    from concourse import bass2jax, mybir as _mybir

    bass2jax.install_neuronx_cc_hook()

    partition_name = (
        nc.partition_id_tensor.name if nc.partition_id_tensor else None
    )
    in_names, out_names, out_avals, zero_shapes = [], [], [], []
    for alloc in nc.m.functions[0].allocations:
        if not isinstance(alloc, _mybir.MemoryLocationSet):
            continue
        name = alloc.memorylocations[0].name
        if alloc.kind == "ExternalInput":
            if name != partition_name:
                in_names.append(name)
        elif alloc.kind == "ExternalOutput":
            out_names.append(name)
            shape = tuple(alloc.tensor_shape)
            dtype = _mybir.dt.np(alloc.dtype)
            out_avals.append(jax.core.ShapedArray(shape, dtype))
            zero_shapes.append((shape, dtype))
    n_params = len(in_names)
    all_in = list(in_names) + list(out_names)
    if partition_name is not None:
        all_in.append(partition_name)

    def _body(*args):
        operands = list(args)
        if partition_name is not None:
            operands.append(bass2jax.partition_id_tensor())
        outs = bass2jax._bass_exec_p.bind(
            *operands,
            out_avals=tuple(out_avals),
            in_names=tuple(all_in),
            out_names=tuple(out_names),
            lowering_input_output_aliases=(),
            sim_require_finite=True,
            sim_require_nnan=True,
            nc=nc,
        )
        return tuple(outs)

    devices = jax.devices()[:n_cores]
    assert len(devices) == n_cores
    mesh = Mesh(np.asarray(devices), ("core",))
    in_specs = (PartitionSpec("core"),) * (n_params + len(out_names))
    out_specs = (PartitionSpec("core"),) * len(out_names)
    sharded = jax.jit(
        shard_map(
            _body,
            mesh=mesh,
            in_specs=in_specs,
            out_specs=out_specs,
            check_rep=False,
        ),
        keep_unused=True,
    )
    zeros = [
        np.zeros((n_cores * sh[0], *sh[1:]), dt) for sh, dt in zero_shapes
    ]

    def concat_inputs(in_maps):
        return [
            np.concatenate(
                [np.asarray(in_maps[c][n]) for c in range(n_cores)], axis=0
            )
            for n in in_names
        ]

    def run(in_maps):
        out_arrs = sharded(*concat_inputs(in_maps), *zeros)
        return [
            {
                name: np.asarray(out_arrs[i]).reshape(
                    n_cores, *out_avals[i].shape
                )[c]
                for i, name in enumerate(out_names)
            }
            for c in range(n_cores)
        ]

    run.sharded = sharded
    run.concat_inputs = concat_inputs
    run.zeros = zeros
    run.out_names = out_names
    run.out_avals = out_avals
    return run


_CACHE = {}


def get_runner(kind="causal"):
    if kind not in _CACHE:
        nc = build_kernel(causal=(kind == "causal"))
        _CACHE[kind] = make_runner(nc)
    return _CACHE[kind]


def _numpy_reference(query, key, value, Wq, bq, Wk, bk, Wv, bv, Wo, bo, mask):
    q = (query @ Wq.T + bq).reshape(B, S, H, D).transpose(0, 2, 1, 3)
    k = (key @ Wk.T + bk).reshape(B, S, H, D).transpose(0, 2, 1, 3)
    v = (value @ Wv.T + bv).reshape(B, S, H, D).transpose(0, 2, 1, 3)
    sc = np.einsum("bhqd,bhkd->bhqk", q, k) / np.sqrt(D)
    sc = np.where(np.asarray(mask).reshape(1, 1, S, S), sc, -np.inf)
    sc -= sc.max(axis=-1, keepdims=True)
    p = np.exp(sc)
    p /= p.sum(axis=-1, keepdims=True)
    o = np.einsum("bhqk,bhkd->bhqd", p, v)
    o = o.transpose(0, 2, 1, 3).reshape(B, S, E)
    return o @ Wo.T + bo


def kernel(**inputs) -> np.ndarray:
    kind = classify_mask(inputs["mask"])
    if kind == "generic":
        fp = {k: np.asarray(v, np.float32) for k, v in inputs.items()
              if k != "mask"}
        return _numpy_reference(mask=inputs["mask"], **fp).astype(np.float32)
    in_maps, kind = prep_core_inputs(**inputs)
    run = get_runner(kind)
    results = run(in_maps)
    bo = np.asarray(inputs["bo"], dtype=np.float32)
    out = np.empty((B, S, E), dtype=np.float32)
    for b in range(B):
        acc = results[b * NGROUPS]["out"].astype(np.float32)
        for gi in range(1, NGROUPS):
            acc = acc + results[b * NGROUPS + gi]["out"].astype(np.float32)
        out[b] = acc + bo[None, :]
    return out


# revision 9
# speedup vs baseline: 1.0740x; 1.0047x over previous
"""Trainium2 Bass kernel: 16-head causal attention (B=4, S=2048, E=1024).

Sharding: 8 cores = 4 batches x 2 head-groups (8 heads each); host sums the
two head-group partials (fp32) and adds bo.

Per-core pipeline (fp16/bf16 matmul operands; PSUM accumulates fp32):
  - q^T = Wq_g X^T, k^T = Wk_g X^T    (transposed projections, [dq, S] f16)
  - V   = X^T.T Wv_g^T                (natural [S, dv] bf16, +ones column per
                                       head so PV also yields denominators)
  - scores^T[k, q] at 128x128 causal granularity: fully-masked sub-blocks are
    skipped; each diagonal-crossing sub-block gets one [128,128] additive mask
    matmul (identity stationary, f16 mask moving, NEG=-60000).
  - P^T = exp(scores^T/8) on ACT -> bf16 (range-safe: exp can reach ~1.3e8,
    which overflows f16; masked lanes underflow to exactly 0)
  - PV: out[q, 65] += P^T_block^T V_aug: stationary = P^T [128,128], moving =
    V_aug [128,65] bf16 -> full 128 output partitions at 65 rows/block. One
    PSUM accumulation group per vpa bank (single start/stop; sub-regions
    auto-initialize via the pending-zero mechanism).
  - normalize: DVE reciprocal of the denominator column + tensor_scalar_mul
  - attn [q, dq] f16 -> PE-transpose [dq, q] -> Wo matmul -> f16 partials
Scheduling: the emitter interleaves projection/output-projection work into the
ACT-bound attention windows (deadline queue + PE-vs-ACT balance heuristic),
batches DMAs into ~45 large transfers, and software-pipelines scores/exp/PV
with a lag of one exp group.
"""

import contextlib

import numpy as np

import bass_rust
import concourse.bass as bass
import concourse.mybir as mybir
import concourse.tile as tile

F32 = mybir.dt.float32
F16 = mybir.dt.float16
BF16 = mybir.dt.bfloat16
AF = mybir.ActivationFunctionType

B, S, E = 4, 2048, 1024
H, D = 16, 64
NCORES = 8
NGROUPS = 2            # head groups (tensor parallel)
HPC = H // NGROUPS     # heads per core
DQ = HPC * D           # per-core projection width = 512
NEG = -60000.0         # f16-representable; exp(NEG/8) == 0.0 in fp32

SK = 128               # k sub-block (partition dim of scores^T)
SQ = 512               # q window
GW = 1024              # exp group width (psum [128, GW])


def split_excess_waits(nc, maxw=1):
    """This container's walrus supports one sem wait per instruction;
    hoist extras onto same-engine nops just before the instruction."""
    n_new = 0
    for bb in nc.main_func.blocks:
        new_list = []
        changed = False
        for inst in list(bb.instructions):
            si = inst.sync_info
            waits = list(si.on_wait) if si and si.on_wait else []
            if len(waits) > maxw:
                changed = True
                extra, keep = waits[:-maxw], waits[-maxw:]
                for ci in range(0, len(extra), maxw):
                    nop = bass_rust.InstNoOp(
                        name=f"I-waitsplit-{n_new}", ins=[], outs=[]
                    )
                    n_new += 1
                    nop.engine = inst.engine
                    nop.sync_info = mybir.SyncInfo(
                        on_wait=extra[ci : ci + maxw], on_update=[]
                    )
                    new_list.append(nop)
                inst.sync_info = mybir.SyncInfo(
                    on_wait=keep,
                    on_update=list(si.on_update) if si.on_update else [],
                )
            new_list.append(inst)
        if changed:
            bb.instructions = new_list
    return n_new


def build_kernel(causal=True, split_waits=True, debug=False):
    s, e, hpc, d = S, E, HPC, D
    dq = hpc * d              # 512
    nec = e // 128            # 8 input-feature chunks
    ndq = dq // 128           # 4 projection partition chunks
    nwin = s // SQ            # 4 q windows
    nsc = s // 128            # 16 s chunks

    nc = bass.Bass()

    xq = nc.declare_dram_parameter("xq_t", [e, s], F16, isOutput=False)
    xk = nc.declare_dram_parameter("xk_t", [e, s], F16, isOutput=False)
    xv = nc.declare_dram_parameter("xv_t", [e, s], F16, isOutput=False)
    wqd = nc.declare_dram_parameter("wq_t", [e, dq], F16, isOutput=False)
    wkd = nc.declare_dram_parameter("wk_t", [e, dq], F16, isOutput=False)
    wvd = nc.declare_dram_parameter("wv_t", [e, dq], F16, isOutput=False)
    wod = nc.declare_dram_parameter("wo_t", [dq, e], F16, isOutput=False)
    # packed constants: [bq(4) | bk(4) | bv_b(512)] f32, [ident | crossmask] f16
    cfd = nc.declare_dram_parameter("consts_f32", [128, 2 * ndq + dq], F32,
                                    isOutput=False)
    chd = nc.declare_dram_parameter("consts_f16", [128, 256], F16,
                                    isOutput=False)
    out = nc.declare_dram_parameter("out", [s, e], F16, isOutput=True)
    if debug:
        dbg_q = nc.declare_dram_parameter("dbg_q", [dq, s], F16, isOutput=True)
        dbg_k = nc.declare_dram_parameter("dbg_k", [dq, s], F16, isOutput=True)
        dbg_v = nc.declare_dram_parameter(
            "dbg_v", [s, hpc * (d + 1)], BF16, isOutput=True
        )
        dbg_at = nc.declare_dram_parameter("dbg_at", [s, dq], F16, isOutput=True)
        dbg_pt = nc.declare_dram_parameter("dbg_pt", [128, 17408], BF16,
                                           isOutput=True)
        dbg_rc = nc.declare_dram_parameter("dbg_rc", [128, 16], F32,
                                           isOutput=True)
        dbg_off = [0]

    with tile.TileContext(nc) as tc, contextlib.ExitStack() as ctx:
        pers = ctx.enter_context(tc.tile_pool(name="pers", bufs=1))
        xpool = ctx.enter_context(tc.tile_pool(name="xp", bufs=3))
        ppool = ctx.enter_context(tc.tile_pool(name="ppl", bufs=4))
        atn = ctx.enter_context(tc.tile_pool(name="atn", bufs=4))
        att = ctx.enter_context(tc.tile_pool(name="att", bufs=4))
        nrm = ctx.enter_context(tc.tile_pool(name="nrm", bufs=4))
        opool = ctx.enter_context(tc.tile_pool(name="opl", bufs=3))
        pp = ctx.enter_context(tc.tile_pool(name="pp", bufs=2, space="PSUM"))
        sp = ctx.enter_context(tc.tile_pool(name="sp", bufs=2, space="PSUM"))
        vp = ctx.enter_context(tc.tile_pool(name="vp", bufs=2, space="PSUM"))

        # ---- persistent tensors ----
        cf_sb = pers.tile([128, 2 * ndq + dq], F32, name="cf_sb")
        ch_sb = pers.tile([128, 256], F16, name="ch_sb")
        bq_sb = cf_sb[:, 0:ndq]
        bk_sb = cf_sb[:, ndq : 2 * ndq]
        bv_sb = cf_sb[:, 2 * ndq : 2 * ndq + dq]
        id_sb = ch_sb[:, 0:128]
        mk_sb = ch_sb[:, 128:256]
        q_sb = [
            [pers.tile([128, SQ], F16, name=f"q_sb{c}_{w}") for w in range(nwin)]
            for c in range(ndq)
        ]
        k_sb = [
            [pers.tile([128, SQ], F16, name=f"k_sb{c}_{w}") for w in range(nwin)]
            for c in range(ndq)
        ]
        v_sb = [
            pers.tile([128, hpc * (d + 1)], BF16, name=f"v_sb{i}")
            for i in range(nsc)
        ]
        wq_sb = pers.tile([128, nec * dq], F16, name="wq_sb")
        wk_sb = pers.tile([128, nec * dq], F16, name="wk_sb")
        wv_sb = pers.tile([128, nec * dq], F16, name="wv_sb")
        wo_sb = pers.tile([128, ndq * e], F16, name="wo_sb")

        # ---- DMA helpers (SP engine -> one HWDGE queue, program order) ----
        def load_w_part(wt, dst, part, nparts=2):
            # e-chunk group `part` of [e, dq] -> dst cols
            g = nec // nparts
            src = wt.rearrange("(n p) m -> p n m", p=128)
            nc.sync.dma_start(
                out=dst.rearrange("p (n m) -> p n m", m=dq)[
                    :, part * g : (part + 1) * g, :
                ],
                in_=src[:, part * g : (part + 1) * g, :],
            )

        def load_x_slab(xt, dst, sb, part=None, nparts=2):
            # dst: [128, nec*512] tile; cols [sb*512,(sb+1)*512) of [e, s]
            src = xt.rearrange("(n p) m -> p n m", p=128)
            d3 = dst.rearrange("p (n m) -> p n m", m=SQ)
            if part is None:
                nc.sync.dma_start(
                    out=d3[:, :, :],
                    in_=src[:, :, sb * SQ : (sb + 1) * SQ],
                )
            else:
                g = nec // nparts
                nc.sync.dma_start(
                    out=d3[:, part * g : (part + 1) * g, :],
                    in_=src[:, part * g : (part + 1) * g,
                            sb * SQ : (sb + 1) * SQ],
                )



        x_t = {}  # (tensor, sb) -> slab tile
        for t, xd in (("q", xq), ("k", xk), ("v", xv)):
            x_t[t, 0] = xpool.tile([128, nec * SQ], F16, tag=f"x{t}",
                                   name=f"x{t}0", bufs=3)
        # slab 0 interleaved with weight pieces for earliest unblock;
        # wq/xq0 in quarters so the first projection matmuls start ASAP
        for part in range(4):
            load_w_part(wqd, wq_sb, part, nparts=4)
            load_x_slab(xq, x_t["q", 0], 0, part=part, nparts=4)
        # packed constants (biases for the first bias-add, mask for h0 scores)
        nc.sync.dma_start(out=cf_sb[:, :], in_=cfd[:, :])
        nc.sync.dma_start(out=ch_sb[:, :], in_=chd[:, :])
        load_w_part(wkd, wk_sb, 0)
        load_x_slab(xk, x_t["k", 0], 0, part=0)
        load_w_part(wkd, wk_sb, 1)
        load_x_slab(xk, x_t["k", 0], 0, part=1)
        load_w_part(wvd, wv_sb, 0)
        load_x_slab(xv, x_t["v", 0], 0, part=0)
        load_w_part(wvd, wv_sb, 1)
        load_x_slab(xv, x_t["v", 0], 0, part=1)
        x_t["q", 1] = xpool.tile([128, nec * SQ], F16, tag="xq",
                                 name="xq1", bufs=3)
        load_x_slab(xq, x_t["q", 1], 1)
        for sb in range(1, nwin):
            for t, xd in (("q", xq), ("k", xk), ("v", xv)):
                if (t, sb) in x_t:
                    continue
                x_t[t, sb] = xpool.tile([128, nec * SQ], F16, tag=f"x{t}",
                                        name=f"x{t}{sb}", bufs=3)
                load_x_slab(xd, x_t[t, sb], sb)
            if sb == 1:
                nc.sync.dma_start(
                    out=wo_sb.rearrange("p (n m) -> p n m", m=e),
                    in_=wod.rearrange("(n p) m -> p n m", p=128),
                )

        # ones columns of v_sb, once, on the idle gpsimd engine
        for i in range(nsc):
            v3 = v_sb[i].rearrange("p (h t) -> p h t", t=d + 1)
            nc.gpsimd.memset(v3[:, :, d], 1.0)

        # ---- compute unit generators ----
        def w3(wt):
            return wt.rearrange("p (n m) -> p n m", m=dq)

        open_ps = {}

        def proj_qk_phase(w_sb_t, xt, dst, bias, sb, c, phase):
            """Half-contraction phase of a q/k projection unit. Phase 0
            allocates the psum tile and contracts ec 0..3; phase 1 finishes
            ec 4..7 and applies the bias. Between a unit's phases at most one
            other pp allocation may occur (pp bufs=2)."""
            key = ("qk", xt, sb, c)
            if phase == 0:
                ps = pp.tile([128, SQ], F32, tag="pp", name="ps_pj")
                open_ps[key] = ps
                ecs = range(0, nec // 2)
            else:
                ps = open_ps.pop(key)
                ecs = range(nec // 2, nec)
            for ec in ecs:
                nc.tensor.matmul(
                    ps[:, :],
                    w3(w_sb_t)[:, ec, c * 128 : (c + 1) * 128],
                    x_t[xt, sb][:, ec * SQ : (ec + 1) * SQ],
                    start=(ec == 0),
                    stop=(ec == nec - 1),
                )
            pe_rows(nec * SQ // 2)
            if phase == 1:
                nc.vector.tensor_scalar_add(
                    dst[c][sb][:, :], ps[:, :], bias[:, c : c + 1]
                )

        def proj_v_phase(sb, ii, phase):
            key = ("v", sb, ii)
            if phase == 0:
                ps = pp.tile([128, dq], F32, tag="pp", name="ps_v")
                open_ps[key] = ps
                ecs = range(0, nec // 2)
            else:
                ps = open_ps.pop(key)
                ecs = range(nec // 2, nec)
            wv_ = w3(wv_sb)
            for ec in ecs:
                nc.tensor.matmul(
                    ps[:, :],
                    x_t["v", sb][:, ec * SQ + ii * 128 : ec * SQ + ii * 128 + 128],
                    wv_[:, ec, :],
                    start=(ec == 0),
                    stop=(ec == nec - 1),
                )
            pe_rows(nec * SQ // 2)
            if phase == 1:
                i = sb * 4 + ii
                v3 = v_sb[i].rearrange("p (h t) -> p h t", t=d + 1)
                nc.vector.tensor_add(
                    v3[:, :, 0:d],
                    ps[:, :].rearrange("p (h t) -> p h t", t=d),
                    bv_sb[:, :].rearrange("p (h t) -> p h t", t=d),
                )

        def proj_qk_unit(w_sb_t, xt, dst, bias, sb, c):
            """One [128,512] slab-column of a transposed projection."""
            ps = pp.tile([128, SQ], F32, tag="pp", name="ps_pj")
            wv_ = w3(w_sb_t)
            for ec in range(nec):
                nc.tensor.matmul(
                    ps[:, :],
                    wv_[:, ec, c * 128 : (c + 1) * 128],
                    x_t[xt, sb][:, ec * SQ : (ec + 1) * SQ],
                    start=(ec == 0),
                    stop=(ec == nec - 1),
                )
            nc.vector.tensor_scalar_add(
                dst[c][sb][:, :], ps[:, :], bias[:, c : c + 1]
            )

        def proj_v_unit(sb, ii):
            """One [128(s), dq] natural-layout V chunk (i = sb*4+ii)."""
            i = sb * 4 + ii
            ps = pp.tile([128, dq], F32, tag="pp", name="ps_v")
            wv_ = w3(wv_sb)
            for ec in range(nec):
                nc.tensor.matmul(
                    ps[:, :],
                    x_t["v", sb][:, ec * SQ + ii * 128 : ec * SQ + ii * 128 + 128],
                    wv_[:, ec, :],
                    start=(ec == 0),
                    stop=(ec == nec - 1),
                )
            v3 = v_sb[i].rearrange("p (h t) -> p h t", t=d + 1)
            nc.vector.tensor_add(
                v3[:, :, 0:d],
                ps[:, :].rearrange("p (h t) -> p h t", t=d),
                bv_sb[:, :].rearrange("p (h t) -> p h t", t=d),
            )

        # static PE/ACT occupancy estimate driving filler insertion
        eng_ns = {"pe": 0.0, "act": 0.0}

        def pe_rows(n):
            eng_ns["pe"] += n * 0.4167

        def act_cols(n):
            eng_ns["act"] += 1.33 * (n * 0.8333 + 185.0)  # 1.33: tuned filler bias

        def attention_head(qb, h, att_tiles, pre_last_cb=None,
                           act_norm=False):
            """scores+exp+PV+normalize for one (window, head).

            Generator: yields after each score-group / PV emission so the
            driver can interleave PE filler while ACT churns through exps.
            pre_last_cb: emitted right after the last score group (tail
            shortening for the final head). act_norm: do half the normalize
            multiplies on ACT (only sensible when ACT is idle afterwards).
            """
            c, hp = h // 2, (h % 2) * 64
            nkb = 4 * qb + 4 if causal else nsc
            # segments: (kb, qstart_global, width)
            segs = []
            for kb in range(nkb):
                if causal and kb >= 4 * qb:
                    qs = kb * 128
                else:
                    qs = qb * SQ
                segs.append((kb, qs, (qb + 1) * SQ - qs))
            # greedy-pack into exp groups of width <= GW
            groups, cur, curw = [], [], 0
            for seg in segs:
                if curw + seg[2] > GW:
                    groups.append(cur)
                    cur, curw = [], 0
                cur.append(seg)
                curw += seg[2]
            if cur:
                groups.append(cur)
            if len(groups) > 1:
                # smallest group first: its short exp lands while ACT still
                # drains the previous head, instead of bubbling at head end
                groups = groups[-2:] + groups[:-2]

            vpa = vp.tile([128, 4 * (d + 1)], F32, tag="vo", name="vpa")
            last_kb = nkb - 1
            npv = sum(
                1 for kb in range(nkb) for qcl in range(4)
                if not (causal and 4 * qb + qcl < kb))
            pv_n = [0]

            def emit_scores(grp):
                gw = sum(g[2] for g in grp)
                scp = sp.tile([128, GW], F32, tag="sc", name="scp")
                off = 0
                for kb, qs, w in grp:
                    ks = k_sb[c][kb // 4][hp : hp + d,
                                          (kb % 4) * 128 : (kb % 4) * 128 + 128]
                    qw_ = q_sb[c][qs // SQ]
                    if causal and kb >= 4 * qb:
                        # additive mask for the diagonal-crossing sub-block
                        nc.tensor.matmul(scp[:, off : off + 128], id_sb[:, :],
                                         mk_sb[:, :], start=True, stop=False)
                        nc.tensor.matmul(
                            scp[:, off : off + 128], ks,
                            qw_[hp : hp + d, qs % SQ : qs % SQ + 128],
                            start=False, stop=True,
                        )
                        pe_rows(256)
                        if w > 128:
                            nc.tensor.matmul(
                                scp[:, off + 128 : off + w], ks,
                                qw_[hp : hp + d, qs % SQ + 128 : qs % SQ + w],
                                start=True, stop=True,
                            )
                            pe_rows(w - 128)
                    else:
                        nc.tensor.matmul(
                            scp[:, off : off + w], ks,
                            qw_[hp : hp + d, qs % SQ : qs % SQ + w],
                            start=True, stop=True,
                        )
                        pe_rows(w)
                    off += w
                pt = ppool.tile([128, GW], BF16, tag="pt", name="pt")
                nc.scalar.activation(
                    pt[:, 0:gw], scp[:, 0:gw], AF.Exp,
                    scale=float(1.0 / np.sqrt(d)),
                )
                act_cols(gw)
                if debug and h == 0:
                    nc.sync.dma_start(
                        out=dbg_pt[:, dbg_off[0] : dbg_off[0] + gw],
                        in_=pt[:, 0:gw])
                    dbg_off[0] += gw
                return pt

            def emit_pv(grp, pt):
                # One psum accumulation group for the whole vpa bank: a
                # start marks the full 2KB zero-region pending-zero, so only
                # the first matmul may carry start and only the last stop;
                # each sub-region auto-initializes on its first write.
                off = 0
                for kb, qs, w in grp:
                    for qcl in range(4):
                        qg = 4 * qb + qcl           # global q chunk
                        if causal and qg < kb:
                            continue                 # fully masked block
                        boff = off + qcl * 128 + qb * SQ - qs
                        nc.tensor.matmul(
                            vpa[:, qcl * (d + 1) : (qcl + 1) * (d + 1)],
                            pt[:, boff : boff + 128],
                            v_sb[kb][:, h * (d + 1) : (h + 1) * (d + 1)],
                            start=(pv_n[0] == 0),
                            stop=(pv_n[0] == npv - 1),
                        )
                        pv_n[0] += 1
                        pe_rows(d + 1)
                    off += w

            # lag-1 software pipeline: scores g+1 overlaps exp g
            prev = None
            for gi, grp in enumerate(groups):
                pt = emit_scores(grp)
                if pre_last_cb is not None and gi == len(groups) - 1:
                    pre_last_cb()
                yield
                if prev is not None:
                    emit_pv(*prev)
                    yield
                prev = (grp, pt)
            emit_pv(*prev)

            v4 = vpa.rearrange("p (qc t) -> p qc t", t=d + 1)
            rcp = nrm.tile([128, 4], F32, tag="rcp", name="rcp")
            nc.vector.reciprocal(rcp[:, :], v4[:, :, d])
            if debug and h == 0:
                nc.sync.dma_start(out=dbg_rc[:, qb * 4 : qb * 4 + 4],
                                  in_=rcp[:, :])
            for qcl in range(4):
                if act_norm and qcl >= 2:
                    nc.scalar.activation(
                        att_tiles[qcl][:, h * d : (h + 1) * d],
                        v4[:, qcl, 0:d],
                        AF.Copy,
                        scale=rcp[:, qcl : qcl + 1],
                    )
                else:
                    nc.vector.tensor_scalar_mul(
                        att_tiles[qcl][:, h * d : (h + 1) * d],
                        v4[:, qcl, 0:d],
                        rcp[:, qcl : qcl + 1],
                    )

        def wo_transpose_unit(att_tiles, cc, at_store, copy_eng=None):
            """Transpose attn chunk cc (heads 2cc, 2cc+1) -> at_store[cc]."""
            tp = pp.tile([128, SQ], F16, tag="pp", name="tp")
            for qcl in range(4):
                nc.tensor.transpose(
                    tp[:, qcl * 128 : (qcl + 1) * 128],
                    att_tiles[qcl][:, cc * 128 : (cc + 1) * 128],
                    id_sb[:, :],
                )
                pe_rows(128)
            at_ = att.tile([128, SQ], F16, tag=f"at{cc}", name="at_")
            if copy_eng is None:
                nc.vector.tensor_copy(at_[:, :], tp[:, :])
            else:
                copy_eng.copy(at_[:, :], tp[:, :])
            at_store[cc] = at_

        def wo_matmul_unit(at_store, qb, i, copy_eng=None):
            """Output projection + store for s-chunk i of window qb."""
            wo3 = wo_sb.rearrange("p (n m) -> p n m", m=e)
            ot = opool.tile([128, e], F16, tag="ot", name="ot")
            si = qb * 4 + i
            for ob in range(2):
                ps = pp.tile([128, 512], F32, tag="pp", name="ps_o")
                for cc in range(ndq):
                    nc.tensor.matmul(
                        ps[:, :],
                        at_store[cc][:, i * 128 : (i + 1) * 128],
                        wo3[:, cc, ob * 512 : (ob + 1) * 512],
                        start=(cc == 0),
                        stop=(cc == ndq - 1),
                    )
                    pe_rows(512)
                if copy_eng is None:
                    nc.vector.tensor_copy(
                        ot[:, ob * 512 : (ob + 1) * 512], ps[:, :])
                else:
                    copy_eng.copy(ot[:, ob * 512 : (ob + 1) * 512], ps[:, :])
                nc.sync.dma_start(
                    out=out[si * 128 : (si + 1) * 128,
                            ob * 512 : (ob + 1) * 512],
                    in_=ot[:, ob * 512 : (ob + 1) * 512],
                )

        # ---- projection queue, deadline-ordered ----
        # Per window sb: q/k chunk c due just before head 2c; v slab due
        # during head 0's score groups (its diag PV needs it). Deadline key:
        # (sb, h_due) with v at h_due=1 (forced explicitly at h0's yields).
        proj_queue = []
        for sb in range(nwin):
            proj_queue.append((sb, 0, "q", sb, 0))
            proj_queue.append((sb, 0, "k", sb, 0))
            for ii in range(4):
                proj_queue.append((sb, 1, "v", sb, ii))
            for c in range(1, ndq):
                proj_queue.append((sb, 2 * c, "q", sb, c))
                proj_queue.append((sb, 2 * c, "k", sb, c))
        wo_queue = []

        def emit_proj_unit():
            _, _, kind, sb, j = proj_queue.pop(0)
            if kind == "q":
                proj_qk_unit(wq_sb, "q", q_sb, bq_sb, sb, j)
            elif kind == "k":
                proj_qk_unit(wk_sb, "k", k_sb, bk_sb, sb, j)
            else:
                proj_v_unit(sb, j)
            pe_rows(nec * SQ)

        def balance_filler(qb):
            # Keep PE fed while ACT is the pacing engine — but don't consume
            # units whose deadline lets them fill a FUTURE window's ACT-bound
            # stretch (they are the only legal filler there).
            if open_ps:
                return  # a phase-split unit owns a pp slot; don't rotate pp
            while eng_ns["pe"] < eng_ns["act"]:
                if proj_queue and (
                    (proj_queue[0][0], proj_queue[0][1]) < (qb + 1, 1)
                ):
                    emit_proj_unit()
                elif wo_queue:
                    wo_queue.pop(0)()
                else:
                    return

        def force_due(qb, h):
            while proj_queue and (proj_queue[0][0], proj_queue[0][1]) <= (qb, h):
                emit_proj_unit()

        def wo_full(qb, att_tiles, last=False):
            at_store = [None] * ndq
            for cc in range(ndq):
                wo_transpose_unit(att_tiles, cc, at_store)
            if debug:
                for qcl in range(4):
                    nc.sync.dma_start(
                        out=dbg_at[(qb * 4 + qcl) * 128 :
                                   (qb * 4 + qcl + 1) * 128, :],
                        in_=att_tiles[qcl][:, :],
                    )
            for i in range(4):
                # final window: ACT is idle by now, DVE is not
                wo_matmul_unit(at_store, qb, i,
                               copy_eng=nc.scalar if last else None)

        # ---- emission ----
        # bootstrap: the startup is DMA-bound; emit phase-split units in
        # A,A,B,B order so every unit's first contraction half runs while
        # the second DMA halves are still in flight
        boot = {("q", 0, 0), ("q", 0, 1), ("q", 0, 2), ("q", 0, 3),
                ("k", 0, 0), ("k", 0, 1), ("v", 0, 0), ("v", 0, 1),
                ("v", 0, 2), ("v", 0, 3)}
        for c0, c1 in ((0, 1), (2, 3)):
            proj_qk_phase(wq_sb, "q", q_sb, bq_sb, 0, c0, 0)
            proj_qk_phase(wq_sb, "q", q_sb, bq_sb, 0, c1, 0)
            proj_qk_phase(wq_sb, "q", q_sb, bq_sb, 0, c0, 1)
            proj_qk_phase(wq_sb, "q", q_sb, bq_sb, 0, c1, 1)
        proj_qk_phase(wk_sb, "k", k_sb, bk_sb, 0, 0, 0)
        proj_qk_phase(wk_sb, "k", k_sb, bk_sb, 0, 1, 0)
        proj_qk_phase(wk_sb, "k", k_sb, bk_sb, 0, 0, 1)
        proj_qk_phase(wk_sb, "k", k_sb, bk_sb, 0, 1, 1)
        proj_queue = [u for u in proj_queue if (u[2], u[3], u[4]) not in boot]

        prev = None  # deferred (qb, att_tiles, at_store) for wo
        last_store = [None] * ndq
        for qb in range(nwin):
            att_tiles = [
                atn.tile([128, dq], F16, tag=f"an{qcl}", name=f"an{qcl}_{qb}")
                for qcl in range(4)
            ]
            for h in range(hpc):
                force_due(qb, h)
                yi = 0
                for _ in attention_head(qb, h, att_tiles):
                    yi += 1
                    if h == 0 and qb == 0:
                        # window 0's v slab is still streaming in: run the
                        # first contraction halves while the rest arrives
                        if yi == 1:
                            proj_v_phase(0, 0, 0)
                            proj_v_phase(0, 1, 0)
                        elif yi == 2:
                            proj_v_phase(0, 0, 1)
                            proj_v_phase(0, 1, 1)
                        elif yi == 3:
                            proj_v_phase(0, 2, 0)
                            proj_v_phase(0, 3, 0)
                            proj_v_phase(0, 2, 1)
                            proj_v_phase(0, 3, 1)
                    elif h == 0 and yi <= 2:
                        # v slab for this window's diagonal, 2 units per yield
                        for _ in range(2):
                            if proj_queue and proj_queue[0][2] == "v" \
                                    and proj_queue[0][3] == qb:
                                emit_proj_unit()
                    balance_filler(qb)
            # defer this window's Wo into the balance queue: it is the only
            # PE work with no deadline, so it belongs in the late ACT-bound
            # holes (atn/att bufs=4 make any emission order inversion-free)
            pqb, ptiles, pstore = qb, att_tiles, [None] * ndq

            def mk_tr(ptiles=ptiles, pstore=pstore, pqb=pqb):
                for cc in range(ndq):
                    wo_transpose_unit(ptiles, cc, pstore)
                if debug:
                    for qcl in range(4):
                        nc.sync.dma_start(
                            out=dbg_at[(pqb * 4 + qcl) * 128 :
                                       (pqb * 4 + qcl + 1) * 128, :],
                            in_=ptiles[qcl][:, :],
                        )

            if qb < nwin - 1:
                wo_queue.append(mk_tr)
                for i in range(4):
                    wo_queue.append(
                        lambda st=pstore, w=pqb, j=i: wo_matmul_unit(st, w, j))
            else:
                prev = (qb, att_tiles)
        while proj_queue:
            emit_proj_unit()
        while wo_queue:
            wo_queue.pop(0)()
        wo_full(*prev, last=True)

        if debug:
            for c in range(ndq):
                for w in range(nwin):
                    cs = slice(c * 128, (c + 1) * 128)
                    ws = slice(w * SQ, (w + 1) * SQ)
                    nc.sync.dma_start(out=dbg_q[cs, ws], in_=q_sb[c][w][:, :])
                    nc.sync.dma_start(out=dbg_k[cs, ws], in_=k_sb[c][w][:, :])
            for i in range(nsc):
                nc.sync.dma_start(
                    out=dbg_v[i * 128 : (i + 1) * 128, :], in_=v_sb[i][:, :]
                )

    if split_waits:
        split_excess_waits(nc)
    return nc


def make_crossmask():
    kk = np.arange(128)[:, None]
    qq = np.arange(128)[None, :]
    return np.where(kk <= qq, 0.0, NEG).astype(np.float16)


def classify_mask(mask):
    m = np.asarray(mask).reshape(S, S)
    if np.array_equal(m, np.tril(np.ones((S, S), bool))):
        return "causal"
    if m.all():
        return "dense"
    return "generic"


def prep_core_inputs(query, key, value, Wq, bq, Wk, bk, Wv, bv, Wo, bo, mask):
    """Shard + lay out host-side numpy inputs for the 8 cores."""
    kind = classify_mask(mask)
    maps = []
    for core in range(NCORES):
        b, gi = core // NGROUPS, core % NGROUPS
        gs = slice(gi * DQ, (gi + 1) * DQ)
        im = {
            "xq_t": np.ascontiguousarray(
                np.asarray(query[b]).T.astype(np.float16)),
            "xk_t": np.ascontiguousarray(
                np.asarray(key[b]).T.astype(np.float16)),
            "xv_t": np.ascontiguousarray(
                np.asarray(value[b]).T.astype(np.float16)),
            "wq_t": np.ascontiguousarray(
                np.asarray(Wq)[gs, :].T.astype(np.float16)),
            "wk_t": np.ascontiguousarray(
                np.asarray(Wk)[gs, :].T.astype(np.float16)),
            "wv_t": np.ascontiguousarray(
                np.asarray(Wv)[gs, :].T.astype(np.float16)),
            "wo_t": np.ascontiguousarray(
                np.asarray(Wo)[:, gs].T.astype(np.float16)),
            "consts_f32": np.ascontiguousarray(np.concatenate([
                np.asarray(bq)[gs].astype(np.float32).reshape(-1, 128).T,
                np.asarray(bk)[gs].astype(np.float32).reshape(-1, 128).T,
                np.broadcast_to(
                    np.asarray(bv)[gs].astype(np.float32), (128, DQ)),
            ], axis=1)),
            "consts_f16": np.ascontiguousarray(np.concatenate([
                np.eye(128, dtype=np.float16), make_crossmask()
            ], axis=1)),
        }
        maps.append(im)
    return maps, kind


def make_runner(nc, n_cores=NCORES):
    """Build a reusable jitted SPMD executor for `nc` on cores 0..n_cores-1."""
    import jax
    from jax.experimental.shard_map import shard_map
    from jax.sharding import Mesh, PartitionSpec

    from concourse import bass2jax, mybir as _mybir

    bass2jax.install_neuronx_cc_hook()

    partition_name = (
        nc.partition_id_tensor.name if nc.partition_id_tensor else None
    )
    in_names, out_names, out_avals, zero_shapes = [], [], [], []
    for alloc in nc.m.functions[0].allocations:
        if not isinstance(alloc, _mybir.MemoryLocationSet):
            continue
        name = alloc.memorylocations[0].name
        if alloc.kind == "ExternalInput":
            if name != partition_name:
                in_names.append(name)
        elif alloc.kind == "ExternalOutput":
            out_names.append(name)
            shape = tuple(alloc.tensor_shape)
            dtype = _mybir.dt.np(alloc.dtype)
            out_avals.append(jax.core.ShapedArray(shape, dtype))
            zero_shapes.append((shape, dtype))
    n_params = len(in_names)
    all_in = list(in_names) + list(out_names)
    if partition_name is not None:
        all_in.append(partition_name)

    def _body(*args):
        operands = list(args)
        if partition_name is not None:
            operands.append(bass2jax.partition_id_tensor())
        outs = bass2jax._bass_exec_p.bind(
            *operands,
            out_avals=tuple(out_avals),
            in_names=tuple(all_in),
            out_names=tuple(out_names),
            lowering_input_output_aliases=(),
            sim_require_finite=True,
            sim_require_nnan=True,
            nc=nc,
        )
        return tuple(outs)

    devices = jax.devices()[:n_cores]
    assert len(devices) == n_cores
    mesh = Mesh(np.asarray(devices), ("core",))
    in_specs = (PartitionSpec("core"),) * (n_params + len(out_names))
    out_specs = (PartitionSpec("core"),) * len(out_names)
    sharded = jax.jit(
        shard_map(
            _body,
            mesh=mesh,
            in_specs=in_specs,
            out_specs=out_specs,
            check_rep=False,
        ),
        keep_unused=True,
    )
    zeros = [
        np.zeros((n_cores * sh[0], *sh[1:]), dt) for sh, dt in zero_shapes
    ]

    def concat_inputs(in_maps):
        return [
            np.concatenate(
                [np.asarray(in_maps[c][n]) for c in range(n_cores)], axis=0
            )
            for n in in_names
        ]

    def run(in_maps):
        out_arrs = sharded(*concat_inputs(in_maps), *zeros)
        return [
            {
                name: np.asarray(out_arrs[i]).reshape(
                    n_cores, *out_avals[i].shape
                )[c]
                for i, name in enumerate(out_names)
            }
            for c in range(n_cores)
        ]

    run.sharded = sharded
    run.concat_inputs = concat_inputs
    run.zeros = zeros
    run.out_names = out_names
    run.out_avals = out_avals
    return run


_CACHE = {}


def get_runner(kind="causal"):
    if kind not in _CACHE:
        nc = build_kernel(causal=(kind == "causal"))
        _CACHE[kind] = make_runner(nc)
    return _CACHE[kind]


def _numpy_reference(query, key, value, Wq, bq, Wk, bk, Wv, bv, Wo, bo, mask):
    q = (query @ Wq.T + bq).reshape(B, S, H, D).transpose(0, 2, 1, 3)
    k = (key @ Wk.T + bk).reshape(B, S, H, D).transpose(0, 2, 1, 3)
    v = (value @ Wv.T + bv).reshape(B, S, H, D).transpose(0, 2, 1, 3)
    sc = np.einsum("bhqd,bhkd->bhqk", q, k) / np.sqrt(D)
    sc = np.where(np.asarray(mask).reshape(1, 1, S, S), sc, -np.inf)
    sc -= sc.max(axis=-1, keepdims=True)
    p = np.exp(sc)
    p /= p.sum(axis=-1, keepdims=True)
    o = np.einsum("bhqk,bhkd->bhqd", p, v)
    o = o.transpose(0, 2, 1, 3).reshape(B, S, E)
    return o @ Wo.T + bo


def kernel(**inputs) -> np.ndarray:
    kind = classify_mask(inputs["mask"])
    if kind == "generic":
        fp = {k: np.asarray(v, np.float32) for k, v in inputs.items()
              if k != "mask"}
        return _numpy_reference(mask=inputs["mask"], **fp).astype(np.float32)
    in_maps, kind = prep_core_inputs(**inputs)
    run = get_runner(kind)
    results = run(in_maps)
    bo = np.asarray(inputs["bo"], dtype=np.float32)
    out = np.empty((B, S, E), dtype=np.float32)
    for b in range(B):
        acc = results[b * NGROUPS]["out"].astype(np.float32)
        for gi in range(1, NGROUPS):
            acc = acc + results[b * NGROUPS + gi]["out"].astype(np.float32)
        out[b] = acc + bo[None, :]
    return out


# revision 10
# speedup vs baseline: 1.0740x; 1.0000x over previous
"""Trainium2 Bass kernel: 16-head causal attention (B=4, S=2048, E=1024).

Sharding: 8 cores = 4 batches x 2 head-groups (8 heads each); host sums the
two head-group partials (fp32) and adds bo.

Per-core pipeline (fp16/bf16 matmul operands; PSUM accumulates fp32):
  - q^T = Wq_g X^T, k^T = Wk_g X^T    (transposed projections, [dq, S] f16)
  - V   = X^T.T Wv_g^T                (natural [S, dv] bf16, +ones column per
                                       head so PV also yields denominators)
  - scores^T[k, q] at 128x128 causal granularity: fully-masked sub-blocks are
    skipped; each diagonal-crossing sub-block gets one [128,128] additive mask
    matmul (identity stationary, f16 mask moving, NEG=-60000).
  - P^T = exp(scores^T/8) on ACT -> bf16 (range-safe: exp can reach ~1.3e8,
    which overflows f16; masked lanes underflow to exactly 0)
  - PV: out[q, 65] += P^T_block^T V_aug: stationary = P^T [128,128], moving =
    V_aug [128,65] bf16 -> full 128 output partitions at 65 rows/block. One
    PSUM accumulation group per vpa bank (single start/stop; sub-regions
    auto-initialize via the pending-zero mechanism).
  - normalize: DVE reciprocal of the denominator column + tensor_scalar_mul
  - attn [q, dq] f16 -> PE-transpose [dq, q] -> Wo matmul -> f16 partials
Scheduling: the emitter interleaves projection/output-projection work into the
ACT-bound attention windows (deadline queue + PE-vs-ACT balance heuristic),
batches DMAs into ~45 large transfers, and software-pipelines scores/exp/PV
with a lag of one exp group.
"""

import contextlib

import numpy as np

import bass_rust
import concourse.bass as bass
import concourse.mybir as mybir
import concourse.tile as tile

F32 = mybir.dt.float32
F16 = mybir.dt.float16
BF16 = mybir.dt.bfloat16
AF = mybir.ActivationFunctionType

B, S, E = 4, 2048, 1024
H, D = 16, 64
NCORES = 8
NGROUPS = 2            # head groups (tensor parallel)
HPC = H // NGROUPS     # heads per core
DQ = HPC * D           # per-core projection width = 512
NEG = -60000.0         # f16-representable; exp(NEG/8) == 0.0 in fp32

SK = 128               # k sub-block (partition dim of scores^T)
SQ = 512               # q window
GW = 1024              # exp group width (psum [128, GW])


def split_excess_waits(nc, maxw=1):
    """This container's walrus supports one sem wait per instruction;
    hoist extras onto same-engine nops just before the instruction."""
    n_new = 0
    for bb in nc.main_func.blocks:
        new_list = []
        changed = False
        for inst in list(bb.instructions):
            si = inst.sync_info
            waits = list(si.on_wait) if si and si.on_wait else []
            if len(waits) > maxw:
                changed = True
                extra, keep = waits[:-maxw], waits[-maxw:]
                for ci in range(0, len(extra), maxw):
                    nop = bass_rust.InstNoOp(
                        name=f"I-waitsplit-{n_new}", ins=[], outs=[]
                    )
                    n_new += 1
                    nop.engine = inst.engine
                    nop.sync_info = mybir.SyncInfo(
                        on_wait=extra[ci : ci + maxw], on_update=[]
                    )
                    new_list.append(nop)
                inst.sync_info = mybir.SyncInfo(
                    on_wait=keep,
                    on_update=list(si.on_update) if si.on_update else [],
                )
            new_list.append(inst)
        if changed:
            bb.instructions = new_list
    return n_new


def build_kernel(causal=True, split_waits=True, debug=False):
    s, e, hpc, d = S, E, HPC, D
    dq = hpc * d              # 512
    nec = e // 128            # 8 input-feature chunks
    ndq = dq // 128           # 4 projection partition chunks
    nwin = s // SQ            # 4 q windows
    nsc = s // 128            # 16 s chunks

    nc = bass.Bass()

    xq = nc.declare_dram_parameter("xq_t", [e, s], F16, isOutput=False)
    xk = nc.declare_dram_parameter("xk_t", [e, s], F16, isOutput=False)
    xv = nc.declare_dram_parameter("xv_t", [e, s], F16, isOutput=False)
    wqd = nc.declare_dram_parameter("wq_t", [e, dq], F16, isOutput=False)
    wkd = nc.declare_dram_parameter("wk_t", [e, dq], F16, isOutput=False)
    wvd = nc.declare_dram_parameter("wv_t", [e, dq], F16, isOutput=False)
    wod = nc.declare_dram_parameter("wo_t", [dq, e], F16, isOutput=False)
    # packed constants: [bq(4) | bk(4) | bv_b(512)] f32, [ident | crossmask] f16
    cfd = nc.declare_dram_parameter("consts_f32", [128, 2 * ndq + dq], F32,
                                    isOutput=False)
    chd = nc.declare_dram_parameter("consts_f16", [128, 256], F16,
                                    isOutput=False)
    out = nc.declare_dram_parameter("out", [s, e], F16, isOutput=True)
    if debug:
        dbg_q = nc.declare_dram_parameter("dbg_q", [dq, s], F16, isOutput=True)
        dbg_k = nc.declare_dram_parameter("dbg_k", [dq, s], F16, isOutput=True)
        dbg_v = nc.declare_dram_parameter(
            "dbg_v", [s, hpc * (d + 1)], BF16, isOutput=True
        )
        dbg_at = nc.declare_dram_parameter("dbg_at", [s, dq], F16, isOutput=True)
        dbg_pt = nc.declare_dram_parameter("dbg_pt", [128, 17408], BF16,
                                           isOutput=True)
        dbg_rc = nc.declare_dram_parameter("dbg_rc", [128, 16], F32,
                                           isOutput=True)
        dbg_off = [0]

    with tile.TileContext(nc) as tc, contextlib.ExitStack() as ctx:
        pers = ctx.enter_context(tc.tile_pool(name="pers", bufs=1))
        xpool = ctx.enter_context(tc.tile_pool(name="xp", bufs=3))
        ppool = ctx.enter_context(tc.tile_pool(name="ppl", bufs=4))
        atn = ctx.enter_context(tc.tile_pool(name="atn", bufs=4))
        att = ctx.enter_context(tc.tile_pool(name="att", bufs=4))
        nrm = ctx.enter_context(tc.tile_pool(name="nrm", bufs=4))
        opool = ctx.enter_context(tc.tile_pool(name="opl", bufs=3))
        pp = ctx.enter_context(tc.tile_pool(name="pp", bufs=2, space="PSUM"))
        sp = ctx.enter_context(tc.tile_pool(name="sp", bufs=2, space="PSUM"))
        vp = ctx.enter_context(tc.tile_pool(name="vp", bufs=2, space="PSUM"))

        # ---- persistent tensors ----
        cf_sb = pers.tile([128, 2 * ndq + dq], F32, name="cf_sb")
        ch_sb = pers.tile([128, 256], F16, name="ch_sb")
        bq_sb = cf_sb[:, 0:ndq]
        bk_sb = cf_sb[:, ndq : 2 * ndq]
        bv_sb = cf_sb[:, 2 * ndq : 2 * ndq + dq]
        id_sb = ch_sb[:, 0:128]
        mk_sb = ch_sb[:, 128:256]
        q_sb = [
            [pers.tile([128, SQ], F16, name=f"q_sb{c}_{w}") for w in range(nwin)]
            for c in range(ndq)
        ]
        k_sb = [
            [pers.tile([128, SQ], F16, name=f"k_sb{c}_{w}") for w in range(nwin)]
            for c in range(ndq)
        ]
        v_sb = [
            pers.tile([128, hpc * (d + 1)], BF16, name=f"v_sb{i}")
            for i in range(nsc)
        ]
        wq_sb = pers.tile([128, nec * dq], F16, name="wq_sb")
        wk_sb = pers.tile([128, nec * dq], F16, name="wk_sb")
        wv_sb = pers.tile([128, nec * dq], F16, name="wv_sb")
        wo_sb = pers.tile([128, ndq * e], F16, name="wo_sb")

        # ---- DMA helpers (SP engine -> one HWDGE queue, program order) ----
        def load_w_part(wt, dst, part, nparts=2):
            # e-chunk group `part` of [e, dq] -> dst cols
            g = nec // nparts
            src = wt.rearrange("(n p) m -> p n m", p=128)
            nc.sync.dma_start(
                out=dst.rearrange("p (n m) -> p n m", m=dq)[
                    :, part * g : (part + 1) * g, :
                ],
                in_=src[:, part * g : (part + 1) * g, :],
            )

        def load_x_slab(xt, dst, sb, part=None, nparts=2):
            # dst: [128, nec*512] tile; cols [sb*512,(sb+1)*512) of [e, s]
            src = xt.rearrange("(n p) m -> p n m", p=128)
            d3 = dst.rearrange("p (n m) -> p n m", m=SQ)
            if part is None:
                nc.sync.dma_start(
                    out=d3[:, :, :],
                    in_=src[:, :, sb * SQ : (sb + 1) * SQ],
                )
            else:
                g = nec // nparts
                nc.sync.dma_start(
                    out=d3[:, part * g : (part + 1) * g, :],
                    in_=src[:, part * g : (part + 1) * g,
                            sb * SQ : (sb + 1) * SQ],
                )



        x_t = {}  # (tensor, sb) -> slab tile
        for t, xd in (("q", xq), ("k", xk), ("v", xv)):
            x_t[t, 0] = xpool.tile([128, nec * SQ], F16, tag=f"x{t}",
                                   name=f"x{t}0", bufs=3)
        # slab 0 interleaved with weight pieces for earliest unblock;
        # wq/xq0 in quarters so the first projection matmuls start ASAP
        for part in range(4):
            load_w_part(wqd, wq_sb, part, nparts=4)
            load_x_slab(xq, x_t["q", 0], 0, part=part, nparts=4)
        # packed constants (biases for the first bias-add, mask for h0 scores)
        nc.sync.dma_start(out=cf_sb[:, :], in_=cfd[:, :])
        nc.sync.dma_start(out=ch_sb[:, :], in_=chd[:, :])
        load_w_part(wkd, wk_sb, 0)
        load_x_slab(xk, x_t["k", 0], 0, part=0)
        load_w_part(wkd, wk_sb, 1)
        load_x_slab(xk, x_t["k", 0], 0, part=1)
        load_w_part(wvd, wv_sb, 0)
        load_x_slab(xv, x_t["v", 0], 0, part=0)
        load_w_part(wvd, wv_sb, 1)
        load_x_slab(xv, x_t["v", 0], 0, part=1)
        x_t["q", 1] = xpool.tile([128, nec * SQ], F16, tag="xq",
                                 name="xq1", bufs=3)
        load_x_slab(xq, x_t["q", 1], 1)
        for sb in range(1, nwin):
            for t, xd in (("q", xq), ("k", xk), ("v", xv)):
                if (t, sb) in x_t:
                    continue
                x_t[t, sb] = xpool.tile([128, nec * SQ], F16, tag=f"x{t}",
                                        name=f"x{t}{sb}", bufs=3)
                load_x_slab(xd, x_t[t, sb], sb)
            if sb == 1:
                nc.sync.dma_start(
                    out=wo_sb.rearrange("p (n m) -> p n m", m=e),
                    in_=wod.rearrange("(n p) m -> p n m", p=128),
                )

        # ones columns of v_sb, once, on the idle gpsimd engine
        for i in range(nsc):
            v3 = v_sb[i].rearrange("p (h t) -> p h t", t=d + 1)
            nc.gpsimd.memset(v3[:, :, d], 1.0)

        # ---- compute unit generators ----
        def w3(wt):
            return wt.rearrange("p (n m) -> p n m", m=dq)

        open_ps = {}

        def proj_qk_phase(w_sb_t, xt, dst, bias, sb, c, phase):
            """Half-contraction phase of a q/k projection unit. Phase 0
            allocates the psum tile and contracts ec 0..3; phase 1 finishes
            ec 4..7 and applies the bias. Between a unit's phases at most one
            other pp allocation may occur (pp bufs=2)."""
            key = ("qk", xt, sb, c)
            if phase == 0:
                ps = pp.tile([128, SQ], F32, tag="pp", name="ps_pj")
                open_ps[key] = ps
                ecs = range(0, nec // 2)
            else:
                ps = open_ps.pop(key)
                ecs = range(nec // 2, nec)
            for ec in ecs:
                nc.tensor.matmul(
                    ps[:, :],
                    w3(w_sb_t)[:, ec, c * 128 : (c + 1) * 128],
                    x_t[xt, sb][:, ec * SQ : (ec + 1) * SQ],
                    start=(ec == 0),
                    stop=(ec == nec - 1),
                )
            pe_rows(nec * SQ // 2)
            if phase == 1:
                nc.vector.tensor_scalar_add(
                    dst[c][sb][:, :], ps[:, :], bias[:, c : c + 1]
                )

        def proj_v_phase(sb, ii, phase):
            key = ("v", sb, ii)
            if phase == 0:
                ps = pp.tile([128, dq], F32, tag="pp", name="ps_v")
                open_ps[key] = ps
                ecs = range(0, nec // 2)
            else:
                ps = open_ps.pop(key)
                ecs = range(nec // 2, nec)
            wv_ = w3(wv_sb)
            for ec in ecs:
                nc.tensor.matmul(
                    ps[:, :],
                    x_t["v", sb][:, ec * SQ + ii * 128 : ec * SQ + ii * 128 + 128],
                    wv_[:, ec, :],
                    start=(ec == 0),
                    stop=(ec == nec - 1),
                )
            pe_rows(nec * SQ // 2)
            if phase == 1:
                i = sb * 4 + ii
                v3 = v_sb[i].rearrange("p (h t) -> p h t", t=d + 1)
                nc.vector.tensor_add(
                    v3[:, :, 0:d],
                    ps[:, :].rearrange("p (h t) -> p h t", t=d),
                    bv_sb[:, :].rearrange("p (h t) -> p h t", t=d),
                )

        def proj_qk_unit(w_sb_t, xt, dst, bias, sb, c):
            """One [128,512] slab-column of a transposed projection."""
            ps = pp.tile([128, SQ], F32, tag="pp", name="ps_pj")
            wv_ = w3(w_sb_t)
            for ec in range(nec):
                nc.tensor.matmul(
                    ps[:, :],
                    wv_[:, ec, c * 128 : (c + 1) * 128],
                    x_t[xt, sb][:, ec * SQ : (ec + 1) * SQ],
                    start=(ec == 0),
                    stop=(ec == nec - 1),
                )
            nc.vector.tensor_scalar_add(
                dst[c][sb][:, :], ps[:, :], bias[:, c : c + 1]
            )

        def proj_v_unit(sb, ii):
            """One [128(s), dq] natural-layout V chunk (i = sb*4+ii)."""
            i = sb * 4 + ii
            ps = pp.tile([128, dq], F32, tag="pp", name="ps_v")
            wv_ = w3(wv_sb)
            for ec in range(nec):
                nc.tensor.matmul(
                    ps[:, :],
                    x_t["v", sb][:, ec * SQ + ii * 128 : ec * SQ + ii * 128 + 128],
                    wv_[:, ec, :],
                    start=(ec == 0),
                    stop=(ec == nec - 1),
                )
            v3 = v_sb[i].rearrange("p (h t) -> p h t", t=d + 1)
            nc.vector.tensor_add(
                v3[:, :, 0:d],
                ps[:, :].rearrange("p (h t) -> p h t", t=d),
                bv_sb[:, :].rearrange("p (h t) -> p h t", t=d),
            )

        # static PE/ACT occupancy estimate driving filler insertion
        eng_ns = {"pe": 0.0, "act": 0.0}

        def pe_rows(n):
            eng_ns["pe"] += n * 0.4167

        def act_cols(n):
            eng_ns["act"] += 1.33 * (n * 0.8333 + 185.0)  # 1.33: tuned filler bias

        def attention_head(qb, h, att_tiles, pre_last_cb=None,
                           act_norm=False):
            """scores+exp+PV+normalize for one (window, head).

            Generator: yields after each score-group / PV emission so the
            driver can interleave PE filler while ACT churns through exps.
            pre_last_cb: emitted right after the last score group (tail
            shortening for the final head). act_norm: do half the normalize
            multiplies on ACT (only sensible when ACT is idle afterwards).
            """
            c, hp = h // 2, (h % 2) * 64
            nkb = 4 * qb + 4 if causal else nsc
            # segments: (kb, qstart_global, width)
            segs = []
            for kb in range(nkb):
                if causal and kb >= 4 * qb:
                    qs = kb * 128
                else:
                    qs = qb * SQ
                segs.append((kb, qs, (qb + 1) * SQ - qs))
            # greedy-pack into exp groups of width <= GW
            groups, cur, curw = [], [], 0
            for seg in segs:
                if curw + seg[2] > GW:
                    groups.append(cur)
                    cur, curw = [], 0
                cur.append(seg)
                curw += seg[2]
            if cur:
                groups.append(cur)
            if len(groups) > 1:
                # smallest group first: its short exp lands while ACT still
                # drains the previous head, instead of bubbling at head end
                groups = groups[-2:] + groups[:-2]

            vpa = vp.tile([128, 4 * (d + 1)], F32, tag="vo", name="vpa")
            last_kb = nkb - 1
            npv = sum(
                1 for kb in range(nkb) for qcl in range(4)
                if not (causal and 4 * qb + qcl < kb))
            pv_n = [0]

            def emit_scores(grp):
                gw = sum(g[2] for g in grp)
                scp = sp.tile([128, GW], F32, tag="sc", name="scp")
                off = 0
                for kb, qs, w in grp:
                    ks = k_sb[c][kb // 4][hp : hp + d,
                                          (kb % 4) * 128 : (kb % 4) * 128 + 128]
                    qw_ = q_sb[c][qs // SQ]
                    if causal and kb >= 4 * qb:
                        # additive mask for the diagonal-crossing sub-block
                        nc.tensor.matmul(scp[:, off : off + 128], id_sb[:, :],
                                         mk_sb[:, :], start=True, stop=False)
                        nc.tensor.matmul(
                            scp[:, off : off + 128], ks,
                            qw_[hp : hp + d, qs % SQ : qs % SQ + 128],
                            start=False, stop=True,
                        )
                        pe_rows(256)
                        if w > 128:
                            nc.tensor.matmul(
                                scp[:, off + 128 : off + w], ks,
                                qw_[hp : hp + d, qs % SQ + 128 : qs % SQ + w],
                                start=True, stop=True,
                            )
                            pe_rows(w - 128)
                    else:
                        nc.tensor.matmul(
                            scp[:, off : off + w], ks,
                            qw_[hp : hp + d, qs % SQ : qs % SQ + w],
                            start=True, stop=True,
                        )
                        pe_rows(w)
                    off += w
                pt = ppool.tile([128, GW], BF16, tag="pt", name="pt")
                nc.scalar.activation(
                    pt[:, 0:gw], scp[:, 0:gw], AF.Exp,
                    scale=float(1.0 / np.sqrt(d)),
                )
                act_cols(gw)
                if debug and h == 0:
                    nc.sync.dma_start(
                        out=dbg_pt[:, dbg_off[0] : dbg_off[0] + gw],
                        in_=pt[:, 0:gw])
                    dbg_off[0] += gw
                return pt

            def emit_pv(grp, pt):
                # One psum accumulation group for the whole vpa bank: a
                # start marks the full 2KB zero-region pending-zero, so only
                # the first matmul may carry start and only the last stop;
                # each sub-region auto-initializes on its first write.
                off = 0
                for kb, qs, w in grp:
                    for qcl in range(4):
                        qg = 4 * qb + qcl           # global q chunk
                        if causal and qg < kb:
                            continue                 # fully masked block
                        boff = off + qcl * 128 + qb * SQ - qs
                        nc.tensor.matmul(
                            vpa[:, qcl * (d + 1) : (qcl + 1) * (d + 1)],
                            pt[:, boff : boff + 128],
                            v_sb[kb][:, h * (d + 1) : (h + 1) * (d + 1)],
                            start=(pv_n[0] == 0),
                            stop=(pv_n[0] == npv - 1),
                        )
                        pv_n[0] += 1
                        pe_rows(d + 1)
                    off += w

            # lag-1 software pipeline: scores g+1 overlaps exp g
            prev = None
            for gi, grp in enumerate(groups):
                pt = emit_scores(grp)
                if pre_last_cb is not None and gi == len(groups) - 1:
                    pre_last_cb()
                yield
                if prev is not None:
                    emit_pv(*prev)
                    yield
                prev = (grp, pt)
            emit_pv(*prev)

            v4 = vpa.rearrange("p (qc t) -> p qc t", t=d + 1)
            rcp = nrm.tile([128, 4], F32, tag="rcp", name="rcp")
            nc.vector.reciprocal(rcp[:, :], v4[:, :, d])
            if debug and h == 0:
                nc.sync.dma_start(out=dbg_rc[:, qb * 4 : qb * 4 + 4],
                                  in_=rcp[:, :])
            for qcl in range(4):
                if act_norm and qcl >= 2:
                    nc.scalar.activation(
                        att_tiles[qcl][:, h * d : (h + 1) * d],
                        v4[:, qcl, 0:d],
                        AF.Copy,
                        scale=rcp[:, qcl : qcl + 1],
                    )
                else:
                    nc.vector.tensor_scalar_mul(
                        att_tiles[qcl][:, h * d : (h + 1) * d],
                        v4[:, qcl, 0:d],
                        rcp[:, qcl : qcl + 1],
                    )

        def wo_transpose_unit(att_tiles, cc, at_store, copy_eng=None):
            """Transpose attn chunk cc (heads 2cc, 2cc+1) -> at_store[cc]."""
            tp = pp.tile([128, SQ], F16, tag="pp", name="tp")
            for qcl in range(4):
                nc.tensor.transpose(
                    tp[:, qcl * 128 : (qcl + 1) * 128],
                    att_tiles[qcl][:, cc * 128 : (cc + 1) * 128],
                    id_sb[:, :],
                )
                pe_rows(128)
            at_ = att.tile([128, SQ], F16, tag=f"at{cc}", name="at_")
            if copy_eng is None:
                nc.vector.tensor_copy(at_[:, :], tp[:, :])
            else:
                copy_eng.copy(at_[:, :], tp[:, :])
            at_store[cc] = at_

        def wo_matmul_unit(at_store, qb, i, copy_eng=None):
            """Output projection + store for s-chunk i of window qb."""
            wo3 = wo_sb.rearrange("p (n m) -> p n m", m=e)
            ot = opool.tile([128, e], F16, tag="ot", name="ot")
            si = qb * 4 + i
            for ob in range(2):
                ps = pp.tile([128, 512], F32, tag="pp", name="ps_o")
                for cc in range(ndq):
                    nc.tensor.matmul(
                        ps[:, :],
                        at_store[cc][:, i * 128 : (i + 1) * 128],
                        wo3[:, cc, ob * 512 : (ob + 1) * 512],
                        start=(cc == 0),
                        stop=(cc == ndq - 1),
                    )
                    pe_rows(512)
                if copy_eng is None:
                    nc.vector.tensor_copy(
                        ot[:, ob * 512 : (ob + 1) * 512], ps[:, :])
                else:
                    copy_eng.copy(ot[:, ob * 512 : (ob + 1) * 512], ps[:, :])
                nc.sync.dma_start(
                    out=out[si * 128 : (si + 1) * 128,
                            ob * 512 : (ob + 1) * 512],
                    in_=ot[:, ob * 512 : (ob + 1) * 512],
                )

        # ---- projection queue, deadline-ordered ----
        # Per window sb: q/k chunk c due just before head 2c; v slab due
        # during head 0's score groups (its diag PV needs it). Deadline key:
        # (sb, h_due) with v at h_due=1 (forced explicitly at h0's yields).
        proj_queue = []
        for sb in range(nwin):
            proj_queue.append((sb, 0, "q", sb, 0))
            proj_queue.append((sb, 0, "k", sb, 0))
            for ii in range(4):
                proj_queue.append((sb, 1, "v", sb, ii))
            for c in range(1, ndq):
                proj_queue.append((sb, 2 * c, "q", sb, c))
                proj_queue.append((sb, 2 * c, "k", sb, c))
        wo_queue = []

        def emit_proj_unit():
            _, _, kind, sb, j = proj_queue.pop(0)
            if kind == "q":
                proj_qk_unit(wq_sb, "q", q_sb, bq_sb, sb, j)
            elif kind == "k":
                proj_qk_unit(wk_sb, "k", k_sb, bk_sb, sb, j)
            else:
                proj_v_unit(sb, j)
            pe_rows(nec * SQ)

        def balance_filler(qb):
            # Keep PE fed while ACT is the pacing engine — but don't consume
            # units whose deadline lets them fill a FUTURE window's ACT-bound
            # stretch (they are the only legal filler there).
            if open_ps:
                return  # a phase-split unit owns a pp slot; don't rotate pp
            while eng_ns["pe"] < eng_ns["act"]:
                if proj_queue and (
                    (proj_queue[0][0], proj_queue[0][1]) < (qb + 1, 1)
                ):
                    emit_proj_unit()
                elif wo_queue:
                    wo_queue.pop(0)()
                else:
                    return

        def force_due(qb, h):
            while proj_queue and (proj_queue[0][0], proj_queue[0][1]) <= (qb, h):
                emit_proj_unit()

        def wo_full(qb, att_tiles, last=False):
            at_store = [None] * ndq
            for cc in range(ndq):
                wo_transpose_unit(att_tiles, cc, at_store)
            if debug:
                for qcl in range(4):
                    nc.sync.dma_start(
                        out=dbg_at[(qb * 4 + qcl) * 128 :
                                   (qb * 4 + qcl + 1) * 128, :],
                        in_=att_tiles[qcl][:, :],
                    )
            for i in range(4):
                # final window: ACT is idle by now, DVE is not
                wo_matmul_unit(at_store, qb, i,
                               copy_eng=nc.scalar if last else None)

        # ---- emission ----
        # bootstrap: the startup is DMA-bound; emit phase-split units in
        # A,A,B,B order so every unit's first contraction half runs while
        # the second DMA halves are still in flight
        boot = {("q", 0, 0), ("q", 0, 1), ("q", 0, 2), ("q", 0, 3),
                ("k", 0, 0), ("k", 0, 1), ("v", 0, 0), ("v", 0, 1),
                ("v", 0, 2), ("v", 0, 3)}
        for c0, c1 in ((0, 1), (2, 3)):
            proj_qk_phase(wq_sb, "q", q_sb, bq_sb, 0, c0, 0)
            proj_qk_phase(wq_sb, "q", q_sb, bq_sb, 0, c1, 0)
            proj_qk_phase(wq_sb, "q", q_sb, bq_sb, 0, c0, 1)
            proj_qk_phase(wq_sb, "q", q_sb, bq_sb, 0, c1, 1)
        proj_qk_phase(wk_sb, "k", k_sb, bk_sb, 0, 0, 0)
        proj_qk_phase(wk_sb, "k", k_sb, bk_sb, 0, 1, 0)
        proj_qk_phase(wk_sb, "k", k_sb, bk_sb, 0, 0, 1)
        proj_qk_phase(wk_sb, "k", k_sb, bk_sb, 0, 1, 1)
        proj_queue = [u for u in proj_queue if (u[2], u[3], u[4]) not in boot]

        prev = None  # deferred (qb, att_tiles, at_store) for wo
        last_store = [None] * ndq
        for qb in range(nwin):
            att_tiles = [
                atn.tile([128, dq], F16, tag=f"an{qcl}", name=f"an{qcl}_{qb}")
                for qcl in range(4)
            ]
            for h in range(hpc):
                force_due(qb, h)
                if h == 6 and qb + 1 < nwin:
                    # pre-force next window's first q/k chunks: the boundary
                    # head's scores start with zero projection latency
                    force_due(qb + 1, 0)
                yi = 0
                for _ in attention_head(qb, h, att_tiles):
                    yi += 1
                    if h == 0 and qb == 0:
                        # window 0's v slab is still streaming in: run the
                        # first contraction halves while the rest arrives
                        if yi == 1:
                            proj_v_phase(0, 0, 0)
                            proj_v_phase(0, 1, 0)
                        elif yi == 2:
                            proj_v_phase(0, 0, 1)
                            proj_v_phase(0, 1, 1)
                        elif yi == 3:
                            proj_v_phase(0, 2, 0)
                            proj_v_phase(0, 3, 0)
                            proj_v_phase(0, 2, 1)
                            proj_v_phase(0, 3, 1)
                    elif h == 0 and yi <= 2:
                        # v slab for this window's diagonal, 2 units per yield
                        for _ in range(2):
                            if proj_queue and proj_queue[0][2] == "v" \
                                    and proj_queue[0][3] == qb:
                                emit_proj_unit()
                    balance_filler(qb)
            # defer this window's Wo into the balance queue: it is the only
            # PE work with no deadline, so it belongs in the late ACT-bound
            # holes (atn/att bufs=4 make any emission order inversion-free)
            pqb, ptiles, pstore = qb, att_tiles, [None] * ndq

            def mk_tr(ptiles=ptiles, pstore=pstore, pqb=pqb):
                for cc in range(ndq):
                    wo_transpose_unit(ptiles, cc, pstore)
                if debug:
                    for qcl in range(4):
                        nc.sync.dma_start(
                            out=dbg_at[(pqb * 4 + qcl) * 128 :
                                       (pqb * 4 + qcl + 1) * 128, :],
                            in_=ptiles[qcl][:, :],
                        )

            if qb < nwin - 1:
                wo_queue.append(mk_tr)
                for i in range(4):
                    wo_queue.append(
                        lambda st=pstore, w=pqb, j=i: wo_matmul_unit(st, w, j))
            else:
                prev = (qb, att_tiles)
        while proj_queue:
            emit_proj_unit()
        while wo_queue:
            wo_queue.pop(0)()
        wo_full(*prev, last=True)

        if debug:
            for c in range(ndq):
                for w in range(nwin):
                    cs = slice(c * 128, (c + 1) * 128)
                    ws = slice(w * SQ, (w + 1) * SQ)
                    nc.sync.dma_start(out=dbg_q[cs, ws], in_=q_sb[c][w][:, :])
                    nc.sync.dma_start(out=dbg_k[cs, ws], in_=k_sb[c][w][:, :])
            for i in range(nsc):
                nc.sync.dma_start(
                    out=dbg_v[i * 128 : (i + 1) * 128, :], in_=v_sb[i][:, :]
                )

    if split_waits:
        split_excess_waits(nc)
    return nc


def make_crossmask():
    kk = np.arange(128)[:, None]
    qq = np.arange(128)[None, :]
    return np.where(kk <= qq, 0.0, NEG).astype(np.float16)


def classify_mask(mask):
    m = np.asarray(mask).reshape(S, S)
    if np.array_equal(m, np.tril(np.ones((S, S), bool))):
        return "causal"
    if m.all():
        return "dense"
    return "generic"


def prep_core_inputs(query, key, value, Wq, bq, Wk, bk, Wv, bv, Wo, bo, mask):
    """Shard + lay out host-side numpy inputs for the 8 cores."""
    kind = classify_mask(mask)
    maps = []
    for core in range(NCORES):
        b, gi = core // NGROUPS, core % NGROUPS
        gs = slice(gi * DQ, (gi + 1) * DQ)
        im = {
            "xq_t": np.ascontiguousarray(
                np.asarray(query[b]).T.astype(np.float16)),
            "xk_t": np.ascontiguousarray(
                np.asarray(key[b]).T.astype(np.float16)),
            "xv_t": np.ascontiguousarray(
                np.asarray(value[b]).T.astype(np.float16)),
            "wq_t": np.ascontiguousarray(
                np.asarray(Wq)[gs, :].T.astype(np.float16)),
            "wk_t": np.ascontiguousarray(
                np.asarray(Wk)[gs, :].T.astype(np.float16)),
            "wv_t": np.ascontiguousarray(
                np.asarray(Wv)[gs, :].T.astype(np.float16)),
            "wo_t": np.ascontiguousarray(
                np.asarray(Wo)[:, gs].T.astype(np.float16)),
            "consts_f32": np.ascontiguousarray(np.concatenate([
                np.asarray(bq)[gs].astype(np.float32).reshape(-1, 128).T,
                np.asarray(bk)[gs].astype(np.float32).reshape(-1, 128).T,
                np.broadcast_to(
                    np.asarray(bv)[gs].astype(np.float32), (128, DQ)),
            ], axis=1)),
            "consts_f16": np.ascontiguousarray(np.concatenate([
                np.eye(128, dtype=np.float16), make_crossmask()
            ], axis=1)),
        }
        maps.append(im)
    return maps, kind


def make_runner(nc, n_cores=NCORES):
    """Build a reusable jitted SPMD executor for `nc` on cores 0..n_cores-1."""
    import jax
    from jax.experimental.shard_map import shard_map
    from jax.sharding import Mesh, PartitionSpec

    from concourse import bass2jax, mybir as _mybir

    bass2jax.install_neuronx_cc_hook()

    partition_name = (
        nc.partition_id_tensor.name if nc.partition_id_tensor else None
    )
    in_names, out_names, out_avals, zero_shapes = [], [], [], []
    for alloc in nc.m.functions[0].allocations:
        if not isinstance(alloc, _mybir.MemoryLocationSet):
            continue
        name = alloc.memorylocations[0].name
        if alloc.kind == "ExternalInput":
            if name != partition_name:
                in_names.append(name)
        elif alloc.kind == "ExternalOutput":
            out_names.append(name)
            shape = tuple(alloc.tensor_shape)
            dtype = _mybir.dt.np(alloc.dtype)
            out_avals.append(jax.core.ShapedArray(shape, dtype))
            zero_shapes.append((shape, dtype))
    n_params = len(in_names)
    all_in = list(in_names) + list(out_names)
    if partition_name is not None:
        all_in.append(partition_name)

    def _body(*args):
        operands = list(args)
        if partition_name is not None:
            operands.append(bass2jax.partition_id_tensor())
        outs = bass2jax._bass_exec_p.bind(
            *operands,
            out_avals=tuple(out_avals),
            in_names=tuple(all_in),
            out_names=tuple(out_names),
            lowering_input_output_aliases=(),
            sim_require_finite=True,
            sim_require_nnan=True,
            nc=nc,
        )
        return tuple(outs)

    devices = jax.devices()[:n_cores]
    assert len(devices) == n_cores
    mesh = Mesh(np.asarray(devices), ("core",))
    in_specs = (PartitionSpec("core"),) * (n_params + len(out_names))
    out_specs = (PartitionSpec("core"),) * len(out_names)
    sharded = jax.jit(
        shard_map(
            _body,
            mesh=mesh,
            in_specs=in_specs,
            out_specs=out_specs,
            check_rep=False,
        ),
        keep_unused=True,
    )
    zeros = [
        np.zeros((n_cores * sh[0], *sh[1:]), dt) for sh, dt in zero_shapes
    ]

    def concat_inputs(in_maps):
        return [
            np.concatenate(
                [np.asarray(in_maps[c][n]) for c in range(n_cores)], axis=0
            )
            for n in in_names
        ]

    def run(in_maps):
        out_arrs = sharded(*concat_inputs(in_maps), *zeros)
        return [
            {
                name: np.asarray(out_arrs[i]).reshape(
                    n_cores, *out_avals[i].shape
                )[c]
                for i, name in enumerate(out_names)
            }
            for c in range(n_cores)
        ]

    run.sharded = sharded
    run.concat_inputs = concat_inputs
    run.zeros = zeros
    run.out_names = out_names
    run.out_avals = out_avals
    return run


_CACHE = {}


def get_runner(kind="causal"):
    if kind not in _CACHE:
        nc = build_kernel(causal=(kind == "causal"))
        _CACHE[kind] = make_runner(nc)
    return _CACHE[kind]


def _numpy_reference(query, key, value, Wq, bq, Wk, bk, Wv, bv, Wo, bo, mask):
    q = (query @ Wq.T + bq).reshape(B, S, H, D).transpose(0, 2, 1, 3)
    k = (key @ Wk.T + bk).reshape(B, S, H, D).transpose(0, 2, 1, 3)
    v = (value @ Wv.T + bv).reshape(B, S, H, D).transpose(0, 2, 1, 3)
    sc = np.einsum("bhqd,bhkd->bhqk", q, k) / np.sqrt(D)
    sc = np.where(np.asarray(mask).reshape(1, 1, S, S), sc, -np.inf)
    sc -= sc.max(axis=-1, keepdims=True)
    p = np.exp(sc)
    p /= p.sum(axis=-1, keepdims=True)
    o = np.einsum("bhqk,bhkd->bhqd", p, v)
    o = o.transpose(0, 2, 1, 3).reshape(B, S, E)
    return o @ Wo.T + bo


def kernel(**inputs) -> np.ndarray:
    kind = classify_mask(inputs["mask"])
    if kind == "generic":
        fp = {k: np.asarray(v, np.float32) for k, v in inputs.items()
              if k != "mask"}
        return _numpy_reference(mask=inputs["mask"], **fp).astype(np.float32)
    in_maps, kind = prep_core_inputs(**inputs)
    run = get_runner(kind)
    results = run(in_maps)
    bo = np.asarray(inputs["bo"], dtype=np.float32)
    out = np.empty((B, S, E), dtype=np.float32)
    for b in range(B):
        acc = results[b * NGROUPS]["out"].astype(np.float32)
        for gi in range(1, NGROUPS):
            acc = acc + results[b * NGROUPS + gi]["out"].astype(np.float32)
        out[b] = acc + bo[None, :]
    return out


# revision 11
# speedup vs baseline: 1.0741x; 1.0001x over previous
"""Trainium2 Bass kernel: 16-head causal attention (B=4, S=2048, E=1024).

Sharding: 8 cores = 4 batches x 2 head-groups (8 heads each); host sums the
two head-group partials (fp32) and adds bo.

Per-core pipeline (fp16/bf16 matmul operands; PSUM accumulates fp32):
  - q^T = Wq_g X^T, k^T = Wk_g X^T    (transposed projections, [dq, S] f16)
  - V   = X^T.T Wv_g^T                (natural [S, dv] bf16, +ones column per
                                       head so PV also yields denominators)
  - scores^T[k, q] at 128x128 causal granularity: fully-masked sub-blocks are
    skipped; each diagonal-crossing sub-block gets one [128,128] additive mask
    matmul (identity stationary, f16 mask moving, NEG=-60000).
  - P^T = exp(scores^T/8) on ACT -> bf16 (range-safe: exp can reach ~1.3e8,
    which overflows f16; masked lanes underflow to exactly 0)
  - PV: out[q, 65] += P^T_block^T V_aug: stationary = P^T [128,128], moving =
    V_aug [128,65] bf16 -> full 128 output partitions at 65 rows/block. One
    PSUM accumulation group per vpa bank (single start/stop; sub-regions
    auto-initialize via the pending-zero mechanism).
  - normalize: DVE reciprocal of the denominator column + tensor_scalar_mul
  - attn [q, dq] f16 -> PE-transpose [dq, q] -> Wo matmul -> f16 partials
Scheduling: the emitter interleaves projection/output-projection work into the
ACT-bound attention windows (deadline queue + PE-vs-ACT balance heuristic),
batches DMAs into ~45 large transfers, and software-pipelines scores/exp/PV
with a lag of one exp group.
"""

import contextlib

import numpy as np

import bass_rust
import concourse.bass as bass
import concourse.mybir as mybir
import concourse.tile as tile

F32 = mybir.dt.float32
F16 = mybir.dt.float16
BF16 = mybir.dt.bfloat16
AF = mybir.ActivationFunctionType

B, S, E = 4, 2048, 1024
H, D = 16, 64
NCORES = 8
NGROUPS = 2            # head groups (tensor parallel)
HPC = H // NGROUPS     # heads per core
DQ = HPC * D           # per-core projection width = 512
NEG = -60000.0         # f16-representable; exp(NEG/8) == 0.0 in fp32

SK = 128               # k sub-block (partition dim of scores^T)
SQ = 512               # q window
GW = 1024              # exp group width (psum [128, GW])


def split_excess_waits(nc, maxw=1):
    """This container's walrus supports one sem wait per instruction;
    hoist extras onto same-engine nops just before the instruction."""
    n_new = 0
    for bb in nc.main_func.blocks:
        new_list = []
        changed = False
        for inst in list(bb.instructions):
            si = inst.sync_info
            waits = list(si.on_wait) if si and si.on_wait else []
            if len(waits) > maxw:
                changed = True
                extra, keep = waits[:-maxw], waits[-maxw:]
                for ci in range(0, len(extra), maxw):
                    nop = bass_rust.InstNoOp(
                        name=f"I-waitsplit-{n_new}", ins=[], outs=[]
                    )
                    n_new += 1
                    nop.engine = inst.engine
                    nop.sync_info = mybir.SyncInfo(
                        on_wait=extra[ci : ci + maxw], on_update=[]
                    )
                    new_list.append(nop)
                inst.sync_info = mybir.SyncInfo(
                    on_wait=keep,
                    on_update=list(si.on_update) if si.on_update else [],
                )
            new_list.append(inst)
        if changed:
            bb.instructions = new_list
    return n_new


def build_kernel(causal=True, split_waits=True, debug=False):
    s, e, hpc, d = S, E, HPC, D
    dq = hpc * d              # 512
    nec = e // 128            # 8 input-feature chunks
    ndq = dq // 128           # 4 projection partition chunks
    nwin = s // SQ            # 4 q windows
    nsc = s // 128            # 16 s chunks

    nc = bass.Bass()

    xq = nc.declare_dram_parameter("xq_t", [e, s], F16, isOutput=False)
    xk = nc.declare_dram_parameter("xk_t", [e, s], F16, isOutput=False)
    xv = nc.declare_dram_parameter("xv_t", [e, s], F16, isOutput=False)
    wqd = nc.declare_dram_parameter("wq_t", [e, dq], F16, isOutput=False)
    wkd = nc.declare_dram_parameter("wk_t", [e, dq], F16, isOutput=False)
    wvd = nc.declare_dram_parameter("wv_t", [e, dq], F16, isOutput=False)
    wod = nc.declare_dram_parameter("wo_t", [dq, e], F16, isOutput=False)
    # packed constants: [bq(4) | bk(4) | bv_b(512)] f32, [ident | crossmask] f16
    cfd = nc.declare_dram_parameter("consts_f32", [128, 2 * ndq + dq], F32,
                                    isOutput=False)
    chd = nc.declare_dram_parameter("consts_f16", [128, 256], F16,
                                    isOutput=False)
    out = nc.declare_dram_parameter("out", [s, e], F16, isOutput=True)
    if debug:
        dbg_q = nc.declare_dram_parameter("dbg_q", [dq, s], F16, isOutput=True)
        dbg_k = nc.declare_dram_parameter("dbg_k", [dq, s], F16, isOutput=True)
        dbg_v = nc.declare_dram_parameter(
            "dbg_v", [s, hpc * (d + 1)], BF16, isOutput=True
        )
        dbg_at = nc.declare_dram_parameter("dbg_at", [s, dq], F16, isOutput=True)
        dbg_pt = nc.declare_dram_parameter("dbg_pt", [128, 17408], BF16,
                                           isOutput=True)
        dbg_rc = nc.declare_dram_parameter("dbg_rc", [128, 16], F32,
                                           isOutput=True)
        dbg_off = [0]

    with tile.TileContext(nc) as tc, contextlib.ExitStack() as ctx:
        pers = ctx.enter_context(tc.tile_pool(name="pers", bufs=1))
        xpool = ctx.enter_context(tc.tile_pool(name="xp", bufs=3))
        ppool = ctx.enter_context(tc.tile_pool(name="ppl", bufs=4))
        atn = ctx.enter_context(tc.tile_pool(name="atn", bufs=4))
        att = ctx.enter_context(tc.tile_pool(name="att", bufs=4))
        nrm = ctx.enter_context(tc.tile_pool(name="nrm", bufs=4))
        opool = ctx.enter_context(tc.tile_pool(name="opl", bufs=3))
        pp = ctx.enter_context(tc.tile_pool(name="pp", bufs=2, space="PSUM"))
        sp = ctx.enter_context(tc.tile_pool(name="sp", bufs=2, space="PSUM"))
        vp = ctx.enter_context(tc.tile_pool(name="vp", bufs=2, space="PSUM"))

        # ---- persistent tensors ----
        cf_sb = pers.tile([128, 2 * ndq + dq], F32, name="cf_sb")
        ch_sb = pers.tile([128, 256], F16, name="ch_sb")
        bq_sb = cf_sb[:, 0:ndq]
        bk_sb = cf_sb[:, ndq : 2 * ndq]
        bv_sb = cf_sb[:, 2 * ndq : 2 * ndq + dq]
        id_sb = ch_sb[:, 0:128]
        mk_sb = ch_sb[:, 128:256]
        q_sb = [
            [pers.tile([128, SQ], F16, name=f"q_sb{c}_{w}") for w in range(nwin)]
            for c in range(ndq)
        ]
        k_sb = [
            [pers.tile([128, SQ], F16, name=f"k_sb{c}_{w}") for w in range(nwin)]
            for c in range(ndq)
        ]
        v_sb = [
            pers.tile([128, hpc * (d + 1)], BF16, name=f"v_sb{i}")
            for i in range(nsc)
        ]
        wq_sb = pers.tile([128, nec * dq], F16, name="wq_sb")
        wk_sb = pers.tile([128, nec * dq], F16, name="wk_sb")
        wv_sb = pers.tile([128, nec * dq], F16, name="wv_sb")
        wo_sb = pers.tile([128, ndq * e], F16, name="wo_sb")

        # ---- DMA helpers (SP engine -> one HWDGE queue, program order) ----
        def load_w_part(wt, dst, part, nparts=2):
            # e-chunk group `part` of [e, dq] -> dst cols
            g = nec // nparts
            src = wt.rearrange("(n p) m -> p n m", p=128)
            nc.sync.dma_start(
                out=dst.rearrange("p (n m) -> p n m", m=dq)[
                    :, part * g : (part + 1) * g, :
                ],
                in_=src[:, part * g : (part + 1) * g, :],
            )

        def load_x_slab(xt, dst, sb, part=None, nparts=2):
            # dst: [128, nec*512] tile; cols [sb*512,(sb+1)*512) of [e, s]
            src = xt.rearrange("(n p) m -> p n m", p=128)
            d3 = dst.rearrange("p (n m) -> p n m", m=SQ)
            if part is None:
                nc.sync.dma_start(
                    out=d3[:, :, :],
                    in_=src[:, :, sb * SQ : (sb + 1) * SQ],
                )
            else:
                g = nec // nparts
                nc.sync.dma_start(
                    out=d3[:, part * g : (part + 1) * g, :],
                    in_=src[:, part * g : (part + 1) * g,
                            sb * SQ : (sb + 1) * SQ],
                )



        x_t = {}  # (tensor, sb) -> slab tile
        for t, xd in (("q", xq), ("k", xk), ("v", xv)):
            x_t[t, 0] = xpool.tile([128, nec * SQ], F16, tag=f"x{t}",
                                   name=f"x{t}0", bufs=3)
        # slab 0 interleaved with weight pieces for earliest unblock;
        # wq/xq0 in quarters so the first projection matmuls start ASAP
        for part in range(4):
            load_w_part(wqd, wq_sb, part, nparts=4)
            load_x_slab(xq, x_t["q", 0], 0, part=part, nparts=4)
        # packed constants (biases for the first bias-add, mask for h0 scores)
        nc.sync.dma_start(out=cf_sb[:, :], in_=cfd[:, :])
        nc.sync.dma_start(out=ch_sb[:, :], in_=chd[:, :])
        load_w_part(wkd, wk_sb, 0)
        load_x_slab(xk, x_t["k", 0], 0, part=0)
        load_w_part(wkd, wk_sb, 1)
        load_x_slab(xk, x_t["k", 0], 0, part=1)
        load_w_part(wvd, wv_sb, 0)
        load_x_slab(xv, x_t["v", 0], 0, part=0)
        load_w_part(wvd, wv_sb, 1)
        load_x_slab(xv, x_t["v", 0], 0, part=1)
        x_t["q", 1] = xpool.tile([128, nec * SQ], F16, tag="xq",
                                 name="xq1", bufs=3)
        load_x_slab(xq, x_t["q", 1], 1)
        for sb in range(1, nwin):
            for t, xd in (("q", xq), ("k", xk), ("v", xv)):
                if (t, sb) in x_t:
                    continue
                x_t[t, sb] = xpool.tile([128, nec * SQ], F16, tag=f"x{t}",
                                        name=f"x{t}{sb}", bufs=3)
                load_x_slab(xd, x_t[t, sb], sb)
            if sb == 1:
                nc.sync.dma_start(
                    out=wo_sb.rearrange("p (n m) -> p n m", m=e),
                    in_=wod.rearrange("(n p) m -> p n m", p=128),
                )

        # ones columns of v_sb, once, on the idle gpsimd engine
        for i in range(nsc):
            v3 = v_sb[i].rearrange("p (h t) -> p h t", t=d + 1)
            nc.gpsimd.memset(v3[:, :, d], 1.0)

        # ---- compute unit generators ----
        def w3(wt):
            return wt.rearrange("p (n m) -> p n m", m=dq)

        open_ps = {}

        def proj_qk_phase(w_sb_t, xt, dst, bias, sb, c, phase):
            """Half-contraction phase of a q/k projection unit. Phase 0
            allocates the psum tile and contracts ec 0..3; phase 1 finishes
            ec 4..7 and applies the bias. Between a unit's phases at most one
            other pp allocation may occur (pp bufs=2)."""
            key = ("qk", xt, sb, c)
            if phase == 0:
                ps = pp.tile([128, SQ], F32, tag="pp", name="ps_pj")
                open_ps[key] = ps
                ecs = range(0, nec // 2)
            else:
                ps = open_ps.pop(key)
                ecs = range(nec // 2, nec)
            for ec in ecs:
                nc.tensor.matmul(
                    ps[:, :],
                    w3(w_sb_t)[:, ec, c * 128 : (c + 1) * 128],
                    x_t[xt, sb][:, ec * SQ : (ec + 1) * SQ],
                    start=(ec == 0),
                    stop=(ec == nec - 1),
                )
            pe_rows(nec * SQ // 2)
            if phase == 1:
                nc.vector.tensor_scalar_add(
                    dst[c][sb][:, :], ps[:, :], bias[:, c : c + 1]
                )

        def proj_v_phase(sb, ii, phase):
            key = ("v", sb, ii)
            if phase == 0:
                ps = pp.tile([128, dq], F32, tag="pp", name="ps_v")
                open_ps[key] = ps
                ecs = range(0, nec // 2)
            else:
                ps = open_ps.pop(key)
                ecs = range(nec // 2, nec)
            wv_ = w3(wv_sb)
            for ec in ecs:
                nc.tensor.matmul(
                    ps[:, :],
                    x_t["v", sb][:, ec * SQ + ii * 128 : ec * SQ + ii * 128 + 128],
                    wv_[:, ec, :],
                    start=(ec == 0),
                    stop=(ec == nec - 1),
                )
            pe_rows(nec * SQ // 2)
            if phase == 1:
                i = sb * 4 + ii
                v3 = v_sb[i].rearrange("p (h t) -> p h t", t=d + 1)
                nc.vector.tensor_add(
                    v3[:, :, 0:d],
                    ps[:, :].rearrange("p (h t) -> p h t", t=d),
                    bv_sb[:, :].rearrange("p (h t) -> p h t", t=d),
                )

        def proj_qk_unit(w_sb_t, xt, dst, bias, sb, c):
            """One [128,512] slab-column of a transposed projection."""
            ps = pp.tile([128, SQ], F32, tag="pp", name="ps_pj")
            wv_ = w3(w_sb_t)
            for ec in range(nec):
                nc.tensor.matmul(
                    ps[:, :],
                    wv_[:, ec, c * 128 : (c + 1) * 128],
                    x_t[xt, sb][:, ec * SQ : (ec + 1) * SQ],
                    start=(ec == 0),
                    stop=(ec == nec - 1),
                )
            nc.vector.tensor_scalar_add(
                dst[c][sb][:, :], ps[:, :], bias[:, c : c + 1]
            )

        def proj_v_unit(sb, ii):
            """One [128(s), dq] natural-layout V chunk (i = sb*4+ii)."""
            i = sb * 4 + ii
            ps = pp.tile([128, dq], F32, tag="pp", name="ps_v")
            wv_ = w3(wv_sb)
            for ec in range(nec):
                nc.tensor.matmul(
                    ps[:, :],
                    x_t["v", sb][:, ec * SQ + ii * 128 : ec * SQ + ii * 128 + 128],
                    wv_[:, ec, :],
                    start=(ec == 0),
                    stop=(ec == nec - 1),
                )
            v3 = v_sb[i].rearrange("p (h t) -> p h t", t=d + 1)
            nc.vector.tensor_add(
                v3[:, :, 0:d],
                ps[:, :].rearrange("p (h t) -> p h t", t=d),
                bv_sb[:, :].rearrange("p (h t) -> p h t", t=d),
            )

        # static PE/ACT occupancy estimate driving filler insertion
        eng_ns = {"pe": 0.0, "act": 0.0}

        def pe_rows(n):
            eng_ns["pe"] += n * 0.4167

        def act_cols(n):
            eng_ns["act"] += 1.325 * (n * 0.8333 + 185.0)  # tuned filler bias

        def attention_head(qb, h, att_tiles, pre_last_cb=None,
                           act_norm=False):
            """scores+exp+PV+normalize for one (window, head).

            Generator: yields after each score-group / PV emission so the
            driver can interleave PE filler while ACT churns through exps.
            pre_last_cb: emitted right after the last score group (tail
            shortening for the final head). act_norm: do half the normalize
            multiplies on ACT (only sensible when ACT is idle afterwards).
            """
            c, hp = h // 2, (h % 2) * 64
            nkb = 4 * qb + 4 if causal else nsc
            # segments: (kb, qstart_global, width)
            segs = []
            for kb in range(nkb):
                if causal and kb >= 4 * qb:
                    qs = kb * 128
                else:
                    qs = qb * SQ
                segs.append((kb, qs, (qb + 1) * SQ - qs))
            # greedy-pack into exp groups of width <= GW
            groups, cur, curw = [], [], 0
            for seg in segs:
                if curw + seg[2] > GW:
                    groups.append(cur)
                    cur, curw = [], 0
                cur.append(seg)
                curw += seg[2]
            if cur:
                groups.append(cur)
            if len(groups) > 1:
                # smallest group first: its short exp lands while ACT still
                # drains the previous head, instead of bubbling at head end
                groups = groups[-2:] + groups[:-2]

            vpa = vp.tile([128, 4 * (d + 1)], F32, tag="vo", name="vpa")
            last_kb = nkb - 1
            npv = sum(
                1 for kb in range(nkb) for qcl in range(4)
                if not (causal and 4 * qb + qcl < kb))
            pv_n = [0]

            def emit_scores(grp):
                gw = sum(g[2] for g in grp)
                scp = sp.tile([128, GW], F32, tag="sc", name="scp")
                off = 0
                for kb, qs, w in grp:
                    ks = k_sb[c][kb // 4][hp : hp + d,
                                          (kb % 4) * 128 : (kb % 4) * 128 + 128]
                    qw_ = q_sb[c][qs // SQ]
                    if causal and kb >= 4 * qb:
                        # additive mask for the diagonal-crossing sub-block
                        nc.tensor.matmul(scp[:, off : off + 128], id_sb[:, :],
                                         mk_sb[:, :], start=True, stop=False)
                        nc.tensor.matmul(
                            scp[:, off : off + 128], ks,
                            qw_[hp : hp + d, qs % SQ : qs % SQ + 128],
                            start=False, stop=True,
                        )
                        pe_rows(256)
                        if w > 128:
                            nc.tensor.matmul(
                                scp[:, off + 128 : off + w], ks,
                                qw_[hp : hp + d, qs % SQ + 128 : qs % SQ + w],
                                start=True, stop=True,
                            )
                            pe_rows(w - 128)
                    else:
                        nc.tensor.matmul(
                            scp[:, off : off + w], ks,
                            qw_[hp : hp + d, qs % SQ : qs % SQ + w],
                            start=True, stop=True,
                        )
                        pe_rows(w)
                    off += w
                pt = ppool.tile([128, GW], BF16, tag="pt", name="pt")
                nc.scalar.activation(
                    pt[:, 0:gw], scp[:, 0:gw], AF.Exp,
                    scale=float(1.0 / np.sqrt(d)),
                )
                act_cols(gw)
                if debug and h == 0:
                    nc.sync.dma_start(
                        out=dbg_pt[:, dbg_off[0] : dbg_off[0] + gw],
                        in_=pt[:, 0:gw])
                    dbg_off[0] += gw
                return pt

            def emit_pv(grp, pt):
                # One psum accumulation group for the whole vpa bank: a
                # start marks the full 2KB zero-region pending-zero, so only
                # the first matmul may carry start and only the last stop;
                # each sub-region auto-initializes on its first write.
                off = 0
                for kb, qs, w in grp:
                    for qcl in range(4):
                        qg = 4 * qb + qcl           # global q chunk
                        if causal and qg < kb:
                            continue                 # fully masked block
                        boff = off + qcl * 128 + qb * SQ - qs
                        nc.tensor.matmul(
                            vpa[:, qcl * (d + 1) : (qcl + 1) * (d + 1)],
                            pt[:, boff : boff + 128],
                            v_sb[kb][:, h * (d + 1) : (h + 1) * (d + 1)],
                            start=(pv_n[0] == 0),
                            stop=(pv_n[0] == npv - 1),
                        )
                        pv_n[0] += 1
                        pe_rows(d + 1)
                    off += w

            # lag-1 software pipeline: scores g+1 overlaps exp g
            prev = None
            for gi, grp in enumerate(groups):
                pt = emit_scores(grp)
                if pre_last_cb is not None and gi == len(groups) - 1:
                    pre_last_cb()
                yield
                if prev is not None:
                    emit_pv(*prev)
                    yield
                prev = (grp, pt)
            emit_pv(*prev)

            v4 = vpa.rearrange("p (qc t) -> p qc t", t=d + 1)
            rcp = nrm.tile([128, 4], F32, tag="rcp", name="rcp")
            nc.vector.reciprocal(rcp[:, :], v4[:, :, d])
            if debug and h == 0:
                nc.sync.dma_start(out=dbg_rc[:, qb * 4 : qb * 4 + 4],
                                  in_=rcp[:, :])
            for qcl in range(4):
                if act_norm and qcl >= 2:
                    nc.scalar.activation(
                        att_tiles[qcl][:, h * d : (h + 1) * d],
                        v4[:, qcl, 0:d],
                        AF.Copy,
                        scale=rcp[:, qcl : qcl + 1],
                    )
                else:
                    nc.vector.tensor_scalar_mul(
                        att_tiles[qcl][:, h * d : (h + 1) * d],
                        v4[:, qcl, 0:d],
                        rcp[:, qcl : qcl + 1],
                    )

        def wo_transpose_unit(att_tiles, cc, at_store, copy_eng=None):
            """Transpose attn chunk cc (heads 2cc, 2cc+1) -> at_store[cc]."""
            tp = pp.tile([128, SQ], F16, tag="pp", name="tp")
            for qcl in range(4):
                nc.tensor.transpose(
                    tp[:, qcl * 128 : (qcl + 1) * 128],
                    att_tiles[qcl][:, cc * 128 : (cc + 1) * 128],
                    id_sb[:, :],
                )
                pe_rows(128)
            at_ = att.tile([128, SQ], F16, tag=f"at{cc}", name="at_")
            if copy_eng is None:
                nc.vector.tensor_copy(at_[:, :], tp[:, :])
            else:
                copy_eng.copy(at_[:, :], tp[:, :])
            at_store[cc] = at_

        def wo_matmul_unit(at_store, qb, i, copy_eng=None):
            """Output projection + store for s-chunk i of window qb."""
            wo3 = wo_sb.rearrange("p (n m) -> p n m", m=e)
            ot = opool.tile([128, e], F16, tag="ot", name="ot")
            si = qb * 4 + i
            for ob in range(2):
                ps = pp.tile([128, 512], F32, tag="pp", name="ps_o")
                for cc in range(ndq):
                    nc.tensor.matmul(
                        ps[:, :],
                        at_store[cc][:, i * 128 : (i + 1) * 128],
                        wo3[:, cc, ob * 512 : (ob + 1) * 512],
                        start=(cc == 0),
                        stop=(cc == ndq - 1),
                    )
                    pe_rows(512)
                if copy_eng is None:
                    nc.vector.tensor_copy(
                        ot[:, ob * 512 : (ob + 1) * 512], ps[:, :])
                else:
                    copy_eng.copy(ot[:, ob * 512 : (ob + 1) * 512], ps[:, :])
                nc.sync.dma_start(
                    out=out[si * 128 : (si + 1) * 128,
                            ob * 512 : (ob + 1) * 512],
                    in_=ot[:, ob * 512 : (ob + 1) * 512],
                )

        # ---- projection queue, deadline-ordered ----
        # Per window sb: q/k chunk c due just before head 2c; v slab due
        # during head 0's score groups (its diag PV needs it). Deadline key:
        # (sb, h_due) with v at h_due=1 (forced explicitly at h0's yields).
        proj_queue = []
        for sb in range(nwin):
            proj_queue.append((sb, 0, "q", sb, 0))
            proj_queue.append((sb, 0, "k", sb, 0))
            for ii in range(4):
                proj_queue.append((sb, 1, "v", sb, ii))
            for c in range(1, ndq):
                proj_queue.append((sb, 2 * c, "q", sb, c))
                proj_queue.append((sb, 2 * c, "k", sb, c))
        wo_queue = []

        def emit_proj_unit():
            _, _, kind, sb, j = proj_queue.pop(0)
            if kind == "q":
                proj_qk_unit(wq_sb, "q", q_sb, bq_sb, sb, j)
            elif kind == "k":
                proj_qk_unit(wk_sb, "k", k_sb, bk_sb, sb, j)
            else:
                proj_v_unit(sb, j)
            pe_rows(nec * SQ)

        def balance_filler(qb):
            # Keep PE fed while ACT is the pacing engine — but don't consume
            # units whose deadline lets them fill a FUTURE window's ACT-bound
            # stretch (they are the only legal filler there).
            if open_ps:
                return  # a phase-split unit owns a pp slot; don't rotate pp
            while eng_ns["pe"] < eng_ns["act"]:
                if proj_queue and (
                    (proj_queue[0][0], proj_queue[0][1]) < (qb + 1, 1)
                ):
                    emit_proj_unit()
                elif wo_queue:
                    wo_queue.pop(0)()
                else:
                    return

        def force_due(qb, h):
            while proj_queue and (proj_queue[0][0], proj_queue[0][1]) <= (qb, h):
                emit_proj_unit()

        def wo_full(qb, att_tiles, last=False):
            at_store = [None] * ndq
            for cc in range(ndq):
                wo_transpose_unit(att_tiles, cc, at_store)
            if debug:
                for qcl in range(4):
                    nc.sync.dma_start(
                        out=dbg_at[(qb * 4 + qcl) * 128 :
                                   (qb * 4 + qcl + 1) * 128, :],
                        in_=att_tiles[qcl][:, :],
                    )
            for i in range(4):
                # final window: ACT is idle by now, DVE is not
                wo_matmul_unit(at_store, qb, i,
                               copy_eng=nc.scalar if last else None)

        # ---- emission ----
        # bootstrap: the startup is DMA-bound; emit phase-split units in
        # A,A,B,B order so every unit's first contraction half runs while
        # the second DMA halves are still in flight
        boot = {("q", 0, 0), ("q", 0, 1), ("q", 0, 2), ("q", 0, 3),
                ("k", 0, 0), ("k", 0, 1), ("v", 0, 0), ("v", 0, 1),
                ("v", 0, 2), ("v", 0, 3)}
        for c0, c1 in ((0, 1), (2, 3)):
            proj_qk_phase(wq_sb, "q", q_sb, bq_sb, 0, c0, 0)
            proj_qk_phase(wq_sb, "q", q_sb, bq_sb, 0, c1, 0)
            proj_qk_phase(wq_sb, "q", q_sb, bq_sb, 0, c0, 1)
            proj_qk_phase(wq_sb, "q", q_sb, bq_sb, 0, c1, 1)
        proj_qk_phase(wk_sb, "k", k_sb, bk_sb, 0, 0, 0)
        proj_qk_phase(wk_sb, "k", k_sb, bk_sb, 0, 1, 0)
        proj_qk_phase(wk_sb, "k", k_sb, bk_sb, 0, 0, 1)
        proj_qk_phase(wk_sb, "k", k_sb, bk_sb, 0, 1, 1)
        proj_queue = [u for u in proj_queue if (u[2], u[3], u[4]) not in boot]

        prev = None  # deferred (qb, att_tiles, at_store) for wo
        last_store = [None] * ndq
        for qb in range(nwin):
            att_tiles = [
                atn.tile([128, dq], F16, tag=f"an{qcl}", name=f"an{qcl}_{qb}")
                for qcl in range(4)
            ]
            for h in range(hpc):
                force_due(qb, h)
                if h == 6 and qb + 1 < nwin:
                    # pre-force next window's first q/k chunks: the boundary
                    # head's scores start with zero projection latency
                    force_due(qb + 1, 0)
                yi = 0
                for _ in attention_head(qb, h, att_tiles):
                    yi += 1
                    if h == 0 and qb == 0:
                        # window 0's v slab is still streaming in: run the
                        # first contraction halves while the rest arrives
                        if yi == 1:
                            proj_v_phase(0, 0, 0)
                            proj_v_phase(0, 1, 0)
                        elif yi == 2:
                            proj_v_phase(0, 0, 1)
                            proj_v_phase(0, 1, 1)
                        elif yi == 3:
                            proj_v_phase(0, 2, 0)
                            proj_v_phase(0, 3, 0)
                            proj_v_phase(0, 2, 1)
                            proj_v_phase(0, 3, 1)
                    elif h == 0 and yi <= 2:
                        # v slab for this window's diagonal, 2 units per yield
                        for _ in range(2):
                            if proj_queue and proj_queue[0][2] == "v" \
                                    and proj_queue[0][3] == qb:
                                emit_proj_unit()
                    balance_filler(qb)
            # defer this window's Wo into the balance queue: it is the only
            # PE work with no deadline, so it belongs in the late ACT-bound
            # holes (atn/att bufs=4 make any emission order inversion-free)
            pqb, ptiles, pstore = qb, att_tiles, [None] * ndq

            def mk_tr(ptiles=ptiles, pstore=pstore, pqb=pqb):
                for cc in range(ndq):
                    wo_transpose_unit(ptiles, cc, pstore)
                if debug:
                    for qcl in range(4):
                        nc.sync.dma_start(
                            out=dbg_at[(pqb * 4 + qcl) * 128 :
                                       (pqb * 4 + qcl + 1) * 128, :],
                            in_=ptiles[qcl][:, :],
                        )

            if qb < nwin - 1:
                wo_queue.append(mk_tr)
                for i in range(4):
                    wo_queue.append(
                        lambda st=pstore, w=pqb, j=i: wo_matmul_unit(st, w, j))
            else:
                prev = (qb, att_tiles)
        while proj_queue:
            emit_proj_unit()
        while wo_queue:
            wo_queue.pop(0)()
        wo_full(*prev, last=True)

        if debug:
            for c in range(ndq):
                for w in range(nwin):
                    cs = slice(c * 128, (c + 1) * 128)
                    ws = slice(w * SQ, (w + 1) * SQ)
                    nc.sync.dma_start(out=dbg_q[cs, ws], in_=q_sb[c][w][:, :])
                    nc.sync.dma_start(out=dbg_k[cs, ws], in_=k_sb[c][w][:, :])
            for i in range(nsc):
                nc.sync.dma_start(
                    out=dbg_v[i * 128 : (i + 1) * 128, :], in_=v_sb[i][:, :]
                )

    if split_waits:
        split_excess_waits(nc)
    return nc


def make_crossmask():
    kk = np.arange(128)[:, None]
    qq = np.arange(128)[None, :]
    return np.where(kk <= qq, 0.0, NEG).astype(np.float16)


def classify_mask(mask):
    m = np.asarray(mask).reshape(S, S)
    if np.array_equal(m, np.tril(np.ones((S, S), bool))):
        return "causal"
    if m.all():
        return "dense"
    return "generic"


def prep_core_inputs(query, key, value, Wq, bq, Wk, bk, Wv, bv, Wo, bo, mask):
    """Shard + lay out host-side numpy inputs for the 8 cores."""
    kind = classify_mask(mask)
    maps = []
    for core in range(NCORES):
        b, gi = core // NGROUPS, core % NGROUPS
        gs = slice(gi * DQ, (gi + 1) * DQ)
        im = {
            "xq_t": np.ascontiguousarray(
                np.asarray(query[b]).T.astype(np.float16)),
            "xk_t": np.ascontiguousarray(
                np.asarray(key[b]).T.astype(np.float16)),
            "xv_t": np.ascontiguousarray(
                np.asarray(value[b]).T.astype(np.float16)),
            "wq_t": np.ascontiguousarray(
                np.asarray(Wq)[gs, :].T.astype(np.float16)),
            "wk_t": np.ascontiguousarray(
                np.asarray(Wk)[gs, :].T.astype(np.float16)),
            "wv_t": np.ascontiguousarray(
                np.asarray(Wv)[gs, :].T.astype(np.float16)),
            "wo_t": np.ascontiguousarray(
                np.asarray(Wo)[:, gs].T.astype(np.float16)),
            "consts_f32": np.ascontiguousarray(np.concatenate([
                np.asarray(bq)[gs].astype(np.float32).reshape(-1, 128).T,
                np.asarray(bk)[gs].astype(np.float32).reshape(-1, 128).T,
                np.broadcast_to(
                    np.asarray(bv)[gs].astype(np.float32), (128, DQ)),
            ], axis=1)),
            "consts_f16": np.ascontiguousarray(np.concatenate([
                np.eye(128, dtype=np.float16), make_crossmask()
            ], axis=1)),
        }
        maps.append(im)
    return maps, kind


def make_runner(nc, n_cores=NCORES):
    """Build a reusable jitted SPMD executor for `nc` on cores 0..n_cores-1."""
    import jax
    from jax.experimental.shard_map import shard_map
    from jax.sharding import Mesh, PartitionSpec

    from concourse import bass2jax, mybir as _mybir

    bass2jax.install_neuronx_cc_hook()

    partition_name = (
        nc.partition_id_tensor.name if nc.partition_id_tensor else None
    )
    in_names, out_names, out_avals, zero_shapes = [], [], [], []
    for alloc in nc.m.functions[0].allocations:
        if not isinstance(alloc, _mybir.MemoryLocationSet):
            continue
        name = alloc.memorylocations[0].name
        if alloc.kind == "ExternalInput":
            if name != partition_name:
                in_names.append(name)
        elif alloc.kind == "ExternalOutput":
            out_names.append(name)
            shape = tuple(alloc.tensor_shape)
            dtype = _mybir.dt.np(alloc.dtype)
            out_avals.append(jax.core.ShapedArray(shape, dtype))
            zero_shapes.append((shape, dtype))
    n_params = len(in_names)
    all_in = list(in_names) + list(out_names)
    if partition_name is not None:
        all_in.append(partition_name)

    def _body(*args):
        operands = list(args)
        if partition_name is not None:
            operands.append(bass2jax.partition_id_tensor())
        outs = bass2jax._bass_exec_p.bind(
            *operands,
            out_avals=tuple(out_avals),
            in_names=tuple(all_in),
            out_names=tuple(out_names),
            lowering_input_output_aliases=(),
            sim_require_finite=True,
            sim_require_nnan=True,
            nc=nc,
        )
        return tuple(outs)

    devices = jax.devices()[:n_cores]
    assert len(devices) == n_cores
    mesh = Mesh(np.asarray(devices), ("core",))
    in_specs = (PartitionSpec("core"),) * (n_params + len(out_names))
    out_specs = (PartitionSpec("core"),) * len(out_names)
    sharded = jax.jit(
        shard_map(
            _body,
            mesh=mesh,
            in_specs=in_specs,
            out_specs=out_specs,
            check_rep=False,
        ),
        keep_unused=True,
    )
    zeros = [
        np.zeros((n_cores * sh[0], *sh[1:]), dt) for sh, dt in zero_shapes
    ]

    def concat_inputs(in_maps):
        return [
            np.concatenate(
                [np.asarray(in_maps[c][n]) for c in range(n_cores)], axis=0
            )
            for n in in_names
        ]

    def run(in_maps):
        out_arrs = sharded(*concat_inputs(in_maps), *zeros)
        return [
            {
                name: np.asarray(out_arrs[i]).reshape(
                    n_cores, *out_avals[i].shape
                )[c]
                for i, name in enumerate(out_names)
            }
            for c in range(n_cores)
        ]

    run.sharded = sharded
    run.concat_inputs = concat_inputs
    run.zeros = zeros
    run.out_names = out_names
    run.out_avals = out_avals
    return run


_CACHE = {}


def get_runner(kind="causal"):
    if kind not in _CACHE:
        nc = build_kernel(causal=(kind == "causal"))
        _CACHE[kind] = make_runner(nc)
    return _CACHE[kind]


def _numpy_reference(query, key, value, Wq, bq, Wk, bk, Wv, bv, Wo, bo, mask):
    q = (query @ Wq.T + bq).reshape(B, S, H, D).transpose(0, 2, 1, 3)
    k = (key @ Wk.T + bk).reshape(B, S, H, D).transpose(0, 2, 1, 3)
    v = (value @ Wv.T + bv).reshape(B, S, H, D).transpose(0, 2, 1, 3)
    sc = np.einsum("bhqd,bhkd->bhqk", q, k) / np.sqrt(D)
    sc = np.where(np.asarray(mask).reshape(1, 1, S, S), sc, -np.inf)
    sc -= sc.max(axis=-1, keepdims=True)
    p = np.exp(sc)
    p /= p.sum(axis=-1, keepdims=True)
    o = np.einsum("bhqk,bhkd->bhqd", p, v)
    o = o.transpose(0, 2, 1, 3).reshape(B, S, E)
    return o @ Wo.T + bo


def kernel(**inputs) -> np.ndarray:
    kind = classify_mask(inputs["mask"])
    if kind == "generic":
        fp = {k: np.asarray(v, np.float32) for k, v in inputs.items()
              if k != "mask"}
        return _numpy_reference(mask=inputs["mask"], **fp).astype(np.float32)
    in_maps, kind = prep_core_inputs(**inputs)
    run = get_runner(kind)
    results = run(in_maps)
    bo = np.asarray(inputs["bo"], dtype=np.float32)
    out = np.empty((B, S, E), dtype=np.float32)
    for b in range(B):
        acc = results[b * NGROUPS]["out"].astype(np.float32)
        for gi in range(1, NGROUPS):
            acc = acc + results[b * NGROUPS + gi]["out"].astype(np.float32)
        out[b] = acc + bo[None, :]
    return out


# revision 12
# speedup vs baseline: 1.0753x; 1.0012x over previous
"""Trainium2 Bass kernel: 16-head causal attention (B=4, S=2048, E=1024).

Sharding: 8 cores = 4 batches x 2 head-groups (8 heads each); host sums the
two head-group partials (fp32) and adds bo.

Per-core pipeline (fp16/bf16 matmul operands; PSUM accumulates fp32):
  - q^T = Wq_g X^T, k^T = Wk_g X^T    (transposed projections, [dq, S] f16)
  - V   = X^T.T Wv_g^T                (natural [S, dv] bf16, +ones column per
                                       head so PV also yields denominators)
  - scores^T[k, q] at 128x128 causal granularity: fully-masked sub-blocks are
    skipped; each diagonal-crossing sub-block gets one [128,128] additive mask
    matmul (identity stationary, f16 mask moving, NEG=-60000).
  - P^T = exp(scores^T/8) on ACT -> bf16 (range-safe: exp can reach ~1.3e8,
    which overflows f16; masked lanes underflow to exactly 0)
  - PV: out[q, 65] += P^T_block^T V_aug: stationary = P^T [128,128], moving =
    V_aug [128,65] bf16 -> full 128 output partitions at 65 rows/block. One
    PSUM accumulation group per vpa bank (single start/stop; sub-regions
    auto-initialize via the pending-zero mechanism).
  - normalize: DVE reciprocal of the denominator column + tensor_scalar_mul
  - attn [q, dq] f16 -> PE-transpose [dq, q] -> Wo matmul -> f16 partials
Scheduling: the emitter interleaves projection/output-projection work into the
ACT-bound attention windows (deadline queue + PE-vs-ACT balance heuristic),
batches DMAs into ~45 large transfers, and software-pipelines scores/exp/PV
with a lag of one exp group.
"""

import contextlib

import numpy as np

import bass_rust
import concourse.bass as bass
import concourse.mybir as mybir
import concourse.tile as tile

F32 = mybir.dt.float32
F16 = mybir.dt.float16
BF16 = mybir.dt.bfloat16
AF = mybir.ActivationFunctionType

B, S, E = 4, 2048, 1024
H, D = 16, 64
NCORES = 8
NGROUPS = 2            # head groups (tensor parallel)
HPC = H // NGROUPS     # heads per core
DQ = HPC * D           # per-core projection width = 512
NEG = -60000.0         # f16-representable; exp(NEG/8) == 0.0 in fp32

SK = 128               # k sub-block (partition dim of scores^T)
SQ = 512               # q window
GW = 1024              # exp group width (psum [128, GW])


def split_excess_waits(nc, maxw=1):
    """This container's walrus supports one sem wait per instruction;
    hoist extras onto same-engine nops just before the instruction."""
    n_new = 0
    for bb in nc.main_func.blocks:
        new_list = []
        changed = False
        for inst in list(bb.instructions):
            si = inst.sync_info
            waits = list(si.on_wait) if si and si.on_wait else []
            if len(waits) > maxw:
                changed = True
                extra, keep = waits[:-maxw], waits[-maxw:]
                for ci in range(0, len(extra), maxw):
                    nop = bass_rust.InstNoOp(
                        name=f"I-waitsplit-{n_new}", ins=[], outs=[]
                    )
                    n_new += 1
                    nop.engine = inst.engine
                    nop.sync_info = mybir.SyncInfo(
                        on_wait=extra[ci : ci + maxw], on_update=[]
                    )
                    new_list.append(nop)
                inst.sync_info = mybir.SyncInfo(
                    on_wait=keep,
                    on_update=list(si.on_update) if si.on_update else [],
                )
            new_list.append(inst)
        if changed:
            bb.instructions = new_list
    return n_new


def build_kernel(causal=True, split_waits=True, debug=False):
    s, e, hpc, d = S, E, HPC, D
    dq = hpc * d              # 512
    nec = e // 128            # 8 input-feature chunks
    ndq = dq // 128           # 4 projection partition chunks
    nwin = s // SQ            # 4 q windows
    nsc = s // 128            # 16 s chunks

    nc = bass.Bass()

    xq = nc.declare_dram_parameter("xq_t", [e, s], F16, isOutput=False)
    xk = nc.declare_dram_parameter("xk_t", [e, s], F16, isOutput=False)
    xv = nc.declare_dram_parameter("xv_t", [e, s], F16, isOutput=False)
    wqd = nc.declare_dram_parameter("wq_t", [e, dq], F16, isOutput=False)
    wkd = nc.declare_dram_parameter("wk_t", [e, dq], F16, isOutput=False)
    wvd = nc.declare_dram_parameter("wv_t", [e, dq], F16, isOutput=False)
    wod = nc.declare_dram_parameter("wo_t", [dq, e], F16, isOutput=False)
    # packed constants: [bq(4) | bk(4) | bv_b(512)] f32, [ident | crossmask] f16
    cfd = nc.declare_dram_parameter("consts_f32", [128, 2 * ndq + dq], F32,
                                    isOutput=False)
    chd = nc.declare_dram_parameter("consts_f16", [128, 256], F16,
                                    isOutput=False)
    out = nc.declare_dram_parameter("out", [s, e], F16, isOutput=True)
    if debug:
        dbg_q = nc.declare_dram_parameter("dbg_q", [dq, s], F16, isOutput=True)
        dbg_k = nc.declare_dram_parameter("dbg_k", [dq, s], F16, isOutput=True)
        dbg_v = nc.declare_dram_parameter(
            "dbg_v", [s, hpc * (d + 1)], BF16, isOutput=True
        )
        dbg_at = nc.declare_dram_parameter("dbg_at", [s, dq], F16, isOutput=True)
        dbg_pt = nc.declare_dram_parameter("dbg_pt", [128, 17408], BF16,
                                           isOutput=True)
        dbg_rc = nc.declare_dram_parameter("dbg_rc", [128, 16], F32,
                                           isOutput=True)
        dbg_off = [0]

    with tile.TileContext(nc) as tc, contextlib.ExitStack() as ctx:
        pers = ctx.enter_context(tc.tile_pool(name="pers", bufs=1))
        xpool = ctx.enter_context(tc.tile_pool(name="xp", bufs=3))
        ppool = ctx.enter_context(tc.tile_pool(name="ppl", bufs=4))
        atn = ctx.enter_context(tc.tile_pool(name="atn", bufs=4))
        att = ctx.enter_context(tc.tile_pool(name="att", bufs=4))
        nrm = ctx.enter_context(tc.tile_pool(name="nrm", bufs=4))
        opool = ctx.enter_context(tc.tile_pool(name="opl", bufs=3))
        pp = ctx.enter_context(tc.tile_pool(name="pp", bufs=2, space="PSUM"))
        sp = ctx.enter_context(tc.tile_pool(name="sp", bufs=2, space="PSUM"))
        vp = ctx.enter_context(tc.tile_pool(name="vp", bufs=2, space="PSUM"))

        # ---- persistent tensors ----
        cf_sb = pers.tile([128, 2 * ndq + dq], F32, name="cf_sb")
        ch_sb = pers.tile([128, 256], F16, name="ch_sb")
        bq_sb = cf_sb[:, 0:ndq]
        bk_sb = cf_sb[:, ndq : 2 * ndq]
        bv_sb = cf_sb[:, 2 * ndq : 2 * ndq + dq]
        id_sb = ch_sb[:, 0:128]
        mk_sb = ch_sb[:, 128:256]
        q_sb = [
            [pers.tile([128, SQ], F16, name=f"q_sb{c}_{w}") for w in range(nwin)]
            for c in range(ndq)
        ]
        k_sb = [
            [pers.tile([128, SQ], F16, name=f"k_sb{c}_{w}") for w in range(nwin)]
            for c in range(ndq)
        ]
        v_sb = [
            pers.tile([128, hpc * (d + 1)], BF16, name=f"v_sb{i}")
            for i in range(nsc)
        ]
        wq_sb = pers.tile([128, nec * dq], F16, name="wq_sb")
        wk_sb = pers.tile([128, nec * dq], F16, name="wk_sb")
        wv_sb = pers.tile([128, nec * dq], F16, name="wv_sb")
        wo_sb = pers.tile([128, ndq * e], F16, name="wo_sb")

        # ---- DMA helpers (SP engine -> one HWDGE queue, program order) ----
        def load_w_part(wt, dst, part, nparts=2):
            # e-chunk group `part` of [e, dq] -> dst cols
            g = nec // nparts
            src = wt.rearrange("(n p) m -> p n m", p=128)
            nc.sync.dma_start(
                out=dst.rearrange("p (n m) -> p n m", m=dq)[
                    :, part * g : (part + 1) * g, :
                ],
                in_=src[:, part * g : (part + 1) * g, :],
            )

        def load_x_slab(xt, dst, sb, part=None, nparts=2):
            # dst: [128, nec*512] tile; cols [sb*512,(sb+1)*512) of [e, s]
            src = xt.rearrange("(n p) m -> p n m", p=128)
            d3 = dst.rearrange("p (n m) -> p n m", m=SQ)
            if part is None:
                nc.sync.dma_start(
                    out=d3[:, :, :],
                    in_=src[:, :, sb * SQ : (sb + 1) * SQ],
                )
            else:
                g = nec // nparts
                nc.sync.dma_start(
                    out=d3[:, part * g : (part + 1) * g, :],
                    in_=src[:, part * g : (part + 1) * g,
                            sb * SQ : (sb + 1) * SQ],
                )



        x_t = {}  # (tensor, sb) -> slab tile
        for t, xd in (("q", xq), ("k", xk), ("v", xv)):
            x_t[t, 0] = xpool.tile([128, nec * SQ], F16, tag=f"x{t}",
                                   name=f"x{t}0", bufs=3)
        # slab 0 interleaved with weight pieces for earliest unblock;
        # wq/xq0 in quarters so the first projection matmuls start ASAP
        for part in range(4):
            load_w_part(wqd, wq_sb, part, nparts=4)
            load_x_slab(xq, x_t["q", 0], 0, part=part, nparts=4)
        # packed constants (biases for the first bias-add, mask for h0 scores)
        nc.sync.dma_start(out=cf_sb[:, :], in_=cfd[:, :])
        nc.sync.dma_start(out=ch_sb[:, :], in_=chd[:, :])
        load_w_part(wkd, wk_sb, 0)
        load_x_slab(xk, x_t["k", 0], 0, part=0)
        load_w_part(wkd, wk_sb, 1)
        load_x_slab(xk, x_t["k", 0], 0, part=1)
        load_w_part(wvd, wv_sb, 0)
        load_x_slab(xv, x_t["v", 0], 0, part=0)
        load_w_part(wvd, wv_sb, 1)
        load_x_slab(xv, x_t["v", 0], 0, part=1)
        x_t["q", 1] = xpool.tile([128, nec * SQ], F16, tag="xq",
                                 name="xq1", bufs=3)
        load_x_slab(xq, x_t["q", 1], 1)
        for sb in range(1, nwin):
            for t, xd in (("q", xq), ("k", xk), ("v", xv)):
                if (t, sb) in x_t:
                    continue
                x_t[t, sb] = xpool.tile([128, nec * SQ], F16, tag=f"x{t}",
                                        name=f"x{t}{sb}", bufs=3)
                load_x_slab(xd, x_t[t, sb], sb)
            if sb == 1:
                nc.sync.dma_start(
                    out=wo_sb.rearrange("p (n m) -> p n m", m=e),
                    in_=wod.rearrange("(n p) m -> p n m", p=128),
                )

        # ones columns of v_sb, once, on the idle gpsimd engine
        for i in range(nsc):
            v3 = v_sb[i].rearrange("p (h t) -> p h t", t=d + 1)
            nc.gpsimd.memset(v3[:, :, d], 1.0)

        # ---- compute unit generators ----
        def w3(wt):
            return wt.rearrange("p (n m) -> p n m", m=dq)

        open_ps = {}

        def proj_qk_phase(w_sb_t, xt, dst, bias, sb, c, phase):
            """Half-contraction phase of a q/k projection unit. Phase 0
            allocates the psum tile and contracts ec 0..3; phase 1 finishes
            ec 4..7 and applies the bias. Between a unit's phases at most one
            other pp allocation may occur (pp bufs=2)."""
            key = ("qk", xt, sb, c)
            if phase == 0:
                ps = pp.tile([128, SQ], F32, tag="pp", name="ps_pj")
                open_ps[key] = ps
                ecs = range(0, nec // 2)
            else:
                ps = open_ps.pop(key)
                ecs = range(nec // 2, nec)
            for ec in ecs:
                nc.tensor.matmul(
                    ps[:, :],
                    w3(w_sb_t)[:, ec, c * 128 : (c + 1) * 128],
                    x_t[xt, sb][:, ec * SQ : (ec + 1) * SQ],
                    start=(ec == 0),
                    stop=(ec == nec - 1),
                )
            pe_rows(nec * SQ // 2)
            if phase == 1:
                nc.vector.tensor_scalar_add(
                    dst[c][sb][:, :], ps[:, :], bias[:, c : c + 1]
                )

        def proj_v_phase(sb, ii, phase):
            key = ("v", sb, ii)
            if phase == 0:
                ps = pp.tile([128, dq], F32, tag="pp", name="ps_v")
                open_ps[key] = ps
                ecs = range(0, nec // 2)
            else:
                ps = open_ps.pop(key)
                ecs = range(nec // 2, nec)
            wv_ = w3(wv_sb)
            for ec in ecs:
                nc.tensor.matmul(
                    ps[:, :],
                    x_t["v", sb][:, ec * SQ + ii * 128 : ec * SQ + ii * 128 + 128],
                    wv_[:, ec, :],
                    start=(ec == 0),
                    stop=(ec == nec - 1),
                )
            pe_rows(nec * SQ // 2)
            if phase == 1:
                i = sb * 4 + ii
                v3 = v_sb[i].rearrange("p (h t) -> p h t", t=d + 1)
                nc.vector.tensor_add(
                    v3[:, :, 0:d],
                    ps[:, :].rearrange("p (h t) -> p h t", t=d),
                    bv_sb[:, :].rearrange("p (h t) -> p h t", t=d),
                )

        def proj_qk_unit(w_sb_t, xt, dst, bias, sb, c):
            """One [128,512] slab-column of a transposed projection."""
            ps = pp.tile([128, SQ], F32, tag="pp", name="ps_pj")
            wv_ = w3(w_sb_t)
            for ec in range(nec):
                nc.tensor.matmul(
                    ps[:, :],
                    wv_[:, ec, c * 128 : (c + 1) * 128],
                    x_t[xt, sb][:, ec * SQ : (ec + 1) * SQ],
                    start=(ec == 0),
                    stop=(ec == nec - 1),
                )
            nc.vector.tensor_scalar_add(
                dst[c][sb][:, :], ps[:, :], bias[:, c : c + 1]
            )

        def proj_v_unit(sb, ii):
            """One [128(s), dq] natural-layout V chunk (i = sb*4+ii)."""
            i = sb * 4 + ii
            ps = pp.tile([128, dq], F32, tag="pp", name="ps_v")
            wv_ = w3(wv_sb)
            for ec in range(nec):
                nc.tensor.matmul(
                    ps[:, :],
                    x_t["v", sb][:, ec * SQ + ii * 128 : ec * SQ + ii * 128 + 128],
                    wv_[:, ec, :],
                    start=(ec == 0),
                    stop=(ec == nec - 1),
                )
            v3 = v_sb[i].rearrange("p (h t) -> p h t", t=d + 1)
            nc.vector.tensor_add(
                v3[:, :, 0:d],
                ps[:, :].rearrange("p (h t) -> p h t", t=d),
                bv_sb[:, :].rearrange("p (h t) -> p h t", t=d),
            )

        # static PE/ACT occupancy estimate driving filler insertion
        eng_ns = {"pe": 0.0, "act": 0.0}

        def pe_rows(n):
            eng_ns["pe"] += n * 0.4167

        def act_cols(n):
            eng_ns["act"] += 1.3275 * (n * 0.8333 + 185.0)  # tuned filler bias

        def attention_head(qb, h, att_tiles, pre_last_cb=None,
                           act_norm=False):
            """scores+exp+PV+normalize for one (window, head).

            Generator: yields after each score-group / PV emission so the
            driver can interleave PE filler while ACT churns through exps.
            pre_last_cb: emitted right after the last score group (tail
            shortening for the final head). act_norm: do half the normalize
            multiplies on ACT (only sensible when ACT is idle afterwards).
            """
            c, hp = h // 2, (h % 2) * 64
            nkb = 4 * qb + 4 if causal else nsc
            # segments: (kb, qstart_global, width)
            segs = []
            for kb in range(nkb):
                if causal and kb >= 4 * qb:
                    qs = kb * 128
                else:
                    qs = qb * SQ
                segs.append((kb, qs, (qb + 1) * SQ - qs))
            # greedy-pack into exp groups of width <= GW
            groups, cur, curw = [], [], 0
            for seg in segs:
                if curw + seg[2] > GW:
                    groups.append(cur)
                    cur, curw = [], 0
                cur.append(seg)
                curw += seg[2]
            if cur:
                groups.append(cur)
            if len(groups) > 1:
                # smallest group first: its short exp lands while ACT still
                # drains the previous head, instead of bubbling at head end
                groups = groups[-2:] + groups[:-2]

            vpa = vp.tile([128, 4 * (d + 1)], F32, tag="vo", name="vpa")
            last_kb = nkb - 1
            npv = sum(
                1 for kb in range(nkb) for qcl in range(4)
                if not (causal and 4 * qb + qcl < kb))
            pv_n = [0]

            def emit_scores(grp):
                gw = sum(g[2] for g in grp)
                scp = sp.tile([128, GW], F32, tag="sc", name="scp")
                off = 0
                for kb, qs, w in grp:
                    ks = k_sb[c][kb // 4][hp : hp + d,
                                          (kb % 4) * 128 : (kb % 4) * 128 + 128]
                    qw_ = q_sb[c][qs // SQ]
                    if causal and kb >= 4 * qb:
                        # additive mask for the diagonal-crossing sub-block
                        nc.tensor.matmul(scp[:, off : off + 128], id_sb[:, :],
                                         mk_sb[:, :], start=True, stop=False)
                        nc.tensor.matmul(
                            scp[:, off : off + 128], ks,
                            qw_[hp : hp + d, qs % SQ : qs % SQ + 128],
                            start=False, stop=True,
                        )
                        pe_rows(256)
                        if w > 128:
                            nc.tensor.matmul(
                                scp[:, off + 128 : off + w], ks,
                                qw_[hp : hp + d, qs % SQ + 128 : qs % SQ + w],
                                start=True, stop=True,
                            )
                            pe_rows(w - 128)
                    else:
                        nc.tensor.matmul(
                            scp[:, off : off + w], ks,
                            qw_[hp : hp + d, qs % SQ : qs % SQ + w],
                            start=True, stop=True,
                        )
                        pe_rows(w)
                    off += w
                pt = ppool.tile([128, GW], BF16, tag="pt", name="pt")
                nc.scalar.activation(
                    pt[:, 0:gw], scp[:, 0:gw], AF.Exp,
                    scale=float(1.0 / np.sqrt(d)),
                )
                act_cols(gw)
                if debug and h == 0:
                    nc.sync.dma_start(
                        out=dbg_pt[:, dbg_off[0] : dbg_off[0] + gw],
                        in_=pt[:, 0:gw])
                    dbg_off[0] += gw
                return pt

            def emit_pv(grp, pt):
                # One psum accumulation group for the whole vpa bank: a
                # start marks the full 2KB zero-region pending-zero, so only
                # the first matmul may carry start and only the last stop;
                # each sub-region auto-initializes on its first write.
                off = 0
                for kb, qs, w in grp:
                    for qcl in range(4):
                        qg = 4 * qb + qcl           # global q chunk
                        if causal and qg < kb:
                            continue                 # fully masked block
                        boff = off + qcl * 128 + qb * SQ - qs
                        nc.tensor.matmul(
                            vpa[:, qcl * (d + 1) : (qcl + 1) * (d + 1)],
                            pt[:, boff : boff + 128],
                            v_sb[kb][:, h * (d + 1) : (h + 1) * (d + 1)],
                            start=(pv_n[0] == 0),
                            stop=(pv_n[0] == npv - 1),
                        )
                        pv_n[0] += 1
                        pe_rows(d + 1)
                    off += w

            # lag-1 software pipeline: scores g+1 overlaps exp g
            prev = None
            for gi, grp in enumerate(groups):
                pt = emit_scores(grp)
                if pre_last_cb is not None and gi == len(groups) - 1:
                    pre_last_cb()
                yield
                if prev is not None:
                    emit_pv(*prev)
                    yield
                prev = (grp, pt)
            emit_pv(*prev)

            v4 = vpa.rearrange("p (qc t) -> p qc t", t=d + 1)
            rcp = nrm.tile([128, 4], F32, tag="rcp", name="rcp")
            nc.vector.reciprocal(rcp[:, :], v4[:, :, d])
            if debug and h == 0:
                nc.sync.dma_start(out=dbg_rc[:, qb * 4 : qb * 4 + 4],
                                  in_=rcp[:, :])
            for qcl in range(4):
                if act_norm and qcl >= 2:
                    nc.scalar.activation(
                        att_tiles[qcl][:, h * d : (h + 1) * d],
                        v4[:, qcl, 0:d],
                        AF.Copy,
                        scale=rcp[:, qcl : qcl + 1],
                    )
                else:
                    nc.vector.tensor_scalar_mul(
                        att_tiles[qcl][:, h * d : (h + 1) * d],
                        v4[:, qcl, 0:d],
                        rcp[:, qcl : qcl + 1],
                    )

        def wo_transpose_unit(att_tiles, cc, at_store, copy_eng=None):
            """Transpose attn chunk cc (heads 2cc, 2cc+1) -> at_store[cc]."""
            tp = pp.tile([128, SQ], F16, tag="pp", name="tp")
            for qcl in range(4):
                nc.tensor.transpose(
                    tp[:, qcl * 128 : (qcl + 1) * 128],
                    att_tiles[qcl][:, cc * 128 : (cc + 1) * 128],
                    id_sb[:, :],
                )
                pe_rows(128)
            at_ = att.tile([128, SQ], F16, tag=f"at{cc}", name="at_")
            if copy_eng is None:
                nc.vector.tensor_copy(at_[:, :], tp[:, :])
            else:
                copy_eng.copy(at_[:, :], tp[:, :])
            at_store[cc] = at_

        def wo_matmul_unit(at_store, qb, i, copy_eng=None):
            """Output projection + store for s-chunk i of window qb."""
            wo3 = wo_sb.rearrange("p (n m) -> p n m", m=e)
            ot = opool.tile([128, e], F16, tag="ot", name="ot")
            si = qb * 4 + i
            for ob in range(2):
                ps = pp.tile([128, 512], F32, tag="pp", name="ps_o")
                for cc in range(ndq):
                    nc.tensor.matmul(
                        ps[:, :],
                        at_store[cc][:, i * 128 : (i + 1) * 128],
                        wo3[:, cc, ob * 512 : (ob + 1) * 512],
                        start=(cc == 0),
                        stop=(cc == ndq - 1),
                    )
                    pe_rows(512)
                if copy_eng is None:
                    nc.vector.tensor_copy(
                        ot[:, ob * 512 : (ob + 1) * 512], ps[:, :])
                else:
                    copy_eng.copy(ot[:, ob * 512 : (ob + 1) * 512], ps[:, :])
                nc.sync.dma_start(
                    out=out[si * 128 : (si + 1) * 128,
                            ob * 512 : (ob + 1) * 512],
                    in_=ot[:, ob * 512 : (ob + 1) * 512],
                )

        # ---- projection queue, deadline-ordered ----
        # Per window sb: q/k chunk c due just before head 2c; v slab due
        # during head 0's score groups (its diag PV needs it). Deadline key:
        # (sb, h_due) with v at h_due=1 (forced explicitly at h0's yields).
        proj_queue = []
        for sb in range(nwin):
            proj_queue.append((sb, 0, "q", sb, 0))
            proj_queue.append((sb, 0, "k", sb, 0))
            for ii in range(4):
                proj_queue.append((sb, 1, "v", sb, ii))
            for c in range(1, ndq):
                proj_queue.append((sb, 2 * c, "q", sb, c))
                proj_queue.append((sb, 2 * c, "k", sb, c))
        wo_queue = []

        def emit_proj_unit():
            _, _, kind, sb, j = proj_queue.pop(0)
            if kind == "q":
                proj_qk_unit(wq_sb, "q", q_sb, bq_sb, sb, j)
            elif kind == "k":
                proj_qk_unit(wk_sb, "k", k_sb, bk_sb, sb, j)
            else:
                proj_v_unit(sb, j)
            pe_rows(nec * SQ)

        def balance_filler(qb):
            # Keep PE fed while ACT is the pacing engine — but don't consume
            # units whose deadline lets them fill a FUTURE window's ACT-bound
            # stretch (they are the only legal filler there).
            if open_ps:
                return  # a phase-split unit owns a pp slot; don't rotate pp
            while eng_ns["pe"] < eng_ns["act"]:
                if proj_queue and (
                    (proj_queue[0][0], proj_queue[0][1]) < (qb + 1, 1)
                ):
                    emit_proj_unit()
                elif wo_queue:
                    wo_queue.pop(0)()
                else:
                    return

        def force_due(qb, h):
            while proj_queue and (proj_queue[0][0], proj_queue[0][1]) <= (qb, h):
                emit_proj_unit()

        def wo_full(qb, att_tiles, last=False):
            at_store = [None] * ndq
            for cc in range(ndq):
                wo_transpose_unit(att_tiles, cc, at_store)
            if debug:
                for qcl in range(4):
                    nc.sync.dma_start(
                        out=dbg_at[(qb * 4 + qcl) * 128 :
                                   (qb * 4 + qcl + 1) * 128, :],
                        in_=att_tiles[qcl][:, :],
                    )
            for i in range(4):
                # final window: ACT is idle by now, DVE is not
                wo_matmul_unit(at_store, qb, i,
                               copy_eng=nc.scalar if last else None)

        # ---- emission ----
        # bootstrap: the startup is DMA-bound; emit phase-split units in
        # A,A,B,B order so every unit's first contraction half runs while
        # the second DMA halves are still in flight
        boot = {("q", 0, 0), ("q", 0, 1), ("q", 0, 2), ("q", 0, 3),
                ("k", 0, 0), ("k", 0, 1), ("v", 0, 0), ("v", 0, 1),
                ("v", 0, 2), ("v", 0, 3)}
        for c0, c1 in ((0, 1), (2, 3)):
            proj_qk_phase(wq_sb, "q", q_sb, bq_sb, 0, c0, 0)
            proj_qk_phase(wq_sb, "q", q_sb, bq_sb, 0, c1, 0)
            proj_qk_phase(wq_sb, "q", q_sb, bq_sb, 0, c0, 1)
            proj_qk_phase(wq_sb, "q", q_sb, bq_sb, 0, c1, 1)
        proj_qk_phase(wk_sb, "k", k_sb, bk_sb, 0, 0, 0)
        proj_qk_phase(wk_sb, "k", k_sb, bk_sb, 0, 1, 0)
        proj_qk_phase(wk_sb, "k", k_sb, bk_sb, 0, 0, 1)
        proj_qk_phase(wk_sb, "k", k_sb, bk_sb, 0, 1, 1)
        proj_queue = [u for u in proj_queue if (u[2], u[3], u[4]) not in boot]

        prev = None  # deferred (qb, att_tiles, at_store) for wo
        last_store = [None] * ndq
        for qb in range(nwin):
            att_tiles = [
                atn.tile([128, dq], F16, tag=f"an{qcl}", name=f"an{qcl}_{qb}")
                for qcl in range(4)
            ]
            for h in range(hpc):
                force_due(qb, h)
                if h == 6 and qb + 1 < nwin:
                    # pre-force next window's first q/k chunks: the boundary
                    # head's scores start with zero projection latency
                    force_due(qb + 1, 0)
                yi = 0
                for _ in attention_head(qb, h, att_tiles):
                    yi += 1
                    if h == 0 and qb == 0:
                        # window 0's v slab is still streaming in: run the
                        # first contraction halves while the rest arrives
                        if yi == 1:
                            proj_v_phase(0, 0, 0)
                            proj_v_phase(0, 1, 0)
                        elif yi == 2:
                            proj_v_phase(0, 0, 1)
                            proj_v_phase(0, 1, 1)
                        elif yi == 3:
                            proj_v_phase(0, 2, 0)
                            proj_v_phase(0, 3, 0)
                            proj_v_phase(0, 2, 1)
                            proj_v_phase(0, 3, 1)
                    elif h == 0 and yi <= 2:
                        # v slab for this window's diagonal, 2 units per yield
                        for _ in range(2):
                            if proj_queue and proj_queue[0][2] == "v" \
                                    and proj_queue[0][3] == qb:
                                emit_proj_unit()
                    balance_filler(qb)
            # defer this window's Wo into the balance queue: it is the only
            # PE work with no deadline, so it belongs in the late ACT-bound
            # holes (atn/att bufs=4 make any emission order inversion-free)
            pqb, ptiles, pstore = qb, att_tiles, [None] * ndq

            def mk_tr(ptiles=ptiles, pstore=pstore, pqb=pqb):
                for cc in range(ndq):
                    wo_transpose_unit(ptiles, cc, pstore)
                if debug:
                    for qcl in range(4):
                        nc.sync.dma_start(
                            out=dbg_at[(pqb * 4 + qcl) * 128 :
                                       (pqb * 4 + qcl + 1) * 128, :],
                            in_=ptiles[qcl][:, :],
                        )

            if qb < nwin - 1:
                wo_queue.append(mk_tr)
                for i in range(4):
                    wo_queue.append(
                        lambda st=pstore, w=pqb, j=i: wo_matmul_unit(st, w, j))
            else:
                prev = (qb, att_tiles)
        while proj_queue:
            emit_proj_unit()
        while wo_queue:
            wo_queue.pop(0)()
        wo_full(*prev, last=True)

        if debug:
            for c in range(ndq):
                for w in range(nwin):
                    cs = slice(c * 128, (c + 1) * 128)
                    ws = slice(w * SQ, (w + 1) * SQ)
                    nc.sync.dma_start(out=dbg_q[cs, ws], in_=q_sb[c][w][:, :])
                    nc.sync.dma_start(out=dbg_k[cs, ws], in_=k_sb[c][w][:, :])
            for i in range(nsc):
                nc.sync.dma_start(
                    out=dbg_v[i * 128 : (i + 1) * 128, :], in_=v_sb[i][:, :]
                )

    if split_waits:
        split_excess_waits(nc)
    return nc


def make_crossmask():
    kk = np.arange(128)[:, None]
    qq = np.arange(128)[None, :]
    return np.where(kk <= qq, 0.0, NEG).astype(np.float16)


def classify_mask(mask):
    m = np.asarray(mask).reshape(S, S)
    if np.array_equal(m, np.tril(np.ones((S, S), bool))):
        return "causal"
    if m.all():
        return "dense"
    return "generic"


def prep_core_inputs(query, key, value, Wq, bq, Wk, bk, Wv, bv, Wo, bo, mask):
    """Shard + lay out host-side numpy inputs for the 8 cores."""
    kind = classify_mask(mask)
    maps = []
    for core in range(NCORES):
        b, gi = core // NGROUPS, core % NGROUPS
        gs = slice(gi * DQ, (gi + 1) * DQ)
        im = {
            "xq_t": np.ascontiguousarray(
                np.asarray(query[b]).T.astype(np.float16)),
            "xk_t": np.ascontiguousarray(
                np.asarray(key[b]).T.astype(np.float16)),
            "xv_t": np.ascontiguousarray(
                np.asarray(value[b]).T.astype(np.float16)),
            "wq_t": np.ascontiguousarray(
                np.asarray(Wq)[gs, :].T.astype(np.float16)),
            "wk_t": np.ascontiguousarray(
                np.asarray(Wk)[gs, :].T.astype(np.float16)),
            "wv_t": np.ascontiguousarray(
                np.asarray(Wv)[gs, :].T.astype(np.float16)),
            "wo_t": np.ascontiguousarray(
                np.asarray(Wo)[:, gs].T.astype(np.float16)),
            "consts_f32": np.ascontiguousarray(np.concatenate([
                np.asarray(bq)[gs].astype(np.float32).reshape(-1, 128).T,
                np.asarray(bk)[gs].astype(np.float32).reshape(-1, 128).T,
                np.broadcast_to(
                    np.asarray(bv)[gs].astype(np.float32), (128, DQ)),
            ], axis=1)),
            "consts_f16": np.ascontiguousarray(np.concatenate([
                np.eye(128, dtype=np.float16), make_crossmask()
            ], axis=1)),
        }
        maps.append(im)
    return maps, kind


def make_runner(nc, n_cores=NCORES):
    """Build a reusable jitted SPMD executor for `nc` on cores 0..n_cores-1."""
    import jax
    from jax.experimental.shard_map import shard_map
    from jax.sharding import Mesh, PartitionSpec

    from concourse import bass2jax, mybir as _mybir

    bass2jax.install_neuronx_cc_hook()

    partition_name = (
        nc.partition_id_tensor.name if nc.partition_id_tensor else None
    )
    in_names, out_names, out_avals, zero_shapes = [], [], [], []
    for alloc in nc.m.functions[0].allocations:
        if not isinstance(alloc, _mybir.MemoryLocationSet):
            continue
        name = alloc.memorylocations[0].name
        if alloc.kind == "ExternalInput":
            if name != partition_name:
                in_names.append(name)
        elif alloc.kind == "ExternalOutput":
            out_names.append(name)
            shape = tuple(alloc.tensor_shape)
            dtype = _mybir.dt.np(alloc.dtype)
            out_avals.append(jax.core.ShapedArray(shape, dtype))
            zero_shapes.append((shape, dtype))
    n_params = len(in_names)
    all_in = list(in_names) + list(out_names)
    if partition_name is not None:
        all_in.append(partition_name)

    def _body(*args):
        operands = list(args)
        if partition_name is not None:
            operands.append(bass2jax.partition_id_tensor())
        outs = bass2jax._bass_exec_p.bind(
            *operands,
            out_avals=tuple(out_avals),
            in_names=tuple(all_in),
            out_names=tuple(out_names),
            lowering_input_output_aliases=(),
            sim_require_finite=True,
            sim_require_nnan=True,
            nc=nc,
        )
        return tuple(outs)

    devices = jax.devices()[:n_cores]
    assert len(devices) == n_cores
    mesh = Mesh(np.asarray(devices), ("core",))
    in_specs = (PartitionSpec("core"),) * (n_params + len(out_names))
    out_specs = (PartitionSpec("core"),) * len(out_names)
    sharded = jax.jit(
        shard_map(
            _body,
            mesh=mesh,
            in_specs=in_specs,
            out_specs=out_specs,
            check_rep=False,
        ),
        keep_unused=True,
    )
    zeros = [
        np.zeros((n_cores * sh[0], *sh[1:]), dt) for sh, dt in zero_shapes
    ]

    def concat_inputs(in_maps):
        return [
            np.concatenate(
                [np.asarray(in_maps[c][n]) for c in range(n_cores)], axis=0
            )
            for n in in_names
        ]

    def run(in_maps):
        out_arrs = sharded(*concat_inputs(in_maps), *zeros)
        return [
            {
                name: np.asarray(out_arrs[i]).reshape(
                    n_cores, *out_avals[i].shape
                )[c]
                for i, name in enumerate(out_names)
            }
            for c in range(n_cores)
        ]

    run.sharded = sharded
    run.concat_inputs = concat_inputs
    run.zeros = zeros
    run.out_names = out_names
    run.out_avals = out_avals
    return run


_CACHE = {}


def get_runner(kind="causal"):
    if kind not in _CACHE:
        nc = build_kernel(causal=(kind == "causal"))
        _CACHE[kind] = make_runner(nc)
    return _CACHE[kind]


def _numpy_reference(query, key, value, Wq, bq, Wk, bk, Wv, bv, Wo, bo, mask):
    q = (query @ Wq.T + bq).reshape(B, S, H, D).transpose(0, 2, 1, 3)
    k = (key @ Wk.T + bk).reshape(B, S, H, D).transpose(0, 2, 1, 3)
    v = (value @ Wv.T + bv).reshape(B, S, H, D).transpose(0, 2, 1, 3)
    sc = np.einsum("bhqd,bhkd->bhqk", q, k) / np.sqrt(D)
    sc = np.where(np.asarray(mask).reshape(1, 1, S, S), sc, -np.inf)
    sc -= sc.max(axis=-1, keepdims=True)
    p = np.exp(sc)
    p /= p.sum(axis=-1, keepdims=True)
    o = np.einsum("bhqk,bhkd->bhqd", p, v)
    o = o.transpose(0, 2, 1, 3).reshape(B, S, E)
    return o @ Wo.T + bo


def kernel(**inputs) -> np.ndarray:
    kind = classify_mask(inputs["mask"])
    if kind == "generic":
        fp = {k: np.asarray(v, np.float32) for k, v in inputs.items()
              if k != "mask"}
        return _numpy_reference(mask=inputs["mask"], **fp).astype(np.float32)
    in_maps, kind = prep_core_inputs(**inputs)
    run = get_runner(kind)
    results = run(in_maps)
    bo = np.asarray(inputs["bo"], dtype=np.float32)
    out = np.empty((B, S, E), dtype=np.float32)
    for b in range(B):
        acc = results[b * NGROUPS]["out"].astype(np.float32)
        for gi in range(1, NGROUPS):
            acc = acc + results[b * NGROUPS + gi]["out"].astype(np.float32)
        out[b] = acc + bo[None, :]
    return out


# revision 13
# speedup vs baseline: 1.0759x; 1.0005x over previous
"""Trainium2 Bass kernel: 16-head causal attention (B=4, S=2048, E=1024).

Sharding: 8 cores = 4 batches x 2 head-groups (8 heads each); host sums the
two head-group partials (fp32) and adds bo.

Per-core pipeline (fp16/bf16 matmul operands; PSUM accumulates fp32):
  - q^T = Wq_g X^T, k^T = Wk_g X^T    (transposed projections, [dq, S] f16)
  - V   = X^T.T Wv_g^T                (natural [S, dv] bf16, +ones column per
                                       head so PV also yields denominators)
  - scores^T[k, q] at 128x128 causal granularity: fully-masked sub-blocks are
    skipped; each diagonal-crossing sub-block gets one [128,128] additive mask
    matmul (identity stationary, f16 mask moving, NEG=-60000).
  - P^T = exp(scores^T/8) on ACT -> bf16 (range-safe: exp can reach ~1.3e8,
    which overflows f16; masked lanes underflow to exactly 0)
  - PV: out[q, 65] += P^T_block^T V_aug: stationary = P^T [128,128], moving =
    V_aug [128,65] bf16 -> full 128 output partitions at 65 rows/block. One
    PSUM accumulation group per vpa bank (single start/stop; sub-regions
    auto-initialize via the pending-zero mechanism).
  - normalize: DVE reciprocal of the denominator column + tensor_scalar_mul
  - attn [q, dq] f16 -> PE-transpose [dq, q] -> Wo matmul -> f16 partials
Scheduling: the emitter interleaves projection/output-projection work into the
ACT-bound attention windows (deadline queue + PE-vs-ACT balance heuristic),
batches DMAs into ~45 large transfers, and software-pipelines scores/exp/PV
with a lag of one exp group.
"""

import contextlib

import numpy as np

import bass_rust
import concourse.bass as bass
import concourse.mybir as mybir
import concourse.tile as tile

F32 = mybir.dt.float32
F16 = mybir.dt.float16
BF16 = mybir.dt.bfloat16
AF = mybir.ActivationFunctionType

B, S, E = 4, 2048, 1024
H, D = 16, 64
NCORES = 8
NGROUPS = 2            # head groups (tensor parallel)
HPC = H // NGROUPS     # heads per core
DQ = HPC * D           # per-core projection width = 512
NEG = -60000.0         # f16-representable; exp(NEG/8) == 0.0 in fp32

SK = 128               # k sub-block (partition dim of scores^T)
SQ = 512               # q window
GW = 1024              # exp group width (psum [128, GW])


def split_excess_waits(nc, maxw=1):
    """This container's walrus supports one sem wait per instruction;
    hoist extras onto same-engine nops just before the instruction."""
    n_new = 0
    for bb in nc.main_func.blocks:
        new_list = []
        changed = False
        for inst in list(bb.instructions):
            si = inst.sync_info
            waits = list(si.on_wait) if si and si.on_wait else []
            if len(waits) > maxw:
                changed = True
                extra, keep = waits[:-maxw], waits[-maxw:]
                for ci in range(0, len(extra), maxw):
                    nop = bass_rust.InstNoOp(
                        name=f"I-waitsplit-{n_new}", ins=[], outs=[]
                    )
                    n_new += 1
                    nop.engine = inst.engine
                    nop.sync_info = mybir.SyncInfo(
                        on_wait=extra[ci : ci + maxw], on_update=[]
                    )
                    new_list.append(nop)
                inst.sync_info = mybir.SyncInfo(
                    on_wait=keep,
                    on_update=list(si.on_update) if si.on_update else [],
                )
            new_list.append(inst)
        if changed:
            bb.instructions = new_list
    return n_new


def build_kernel(causal=True, split_waits=True, debug=False):
    s, e, hpc, d = S, E, HPC, D
    dq = hpc * d              # 512
    nec = e // 128            # 8 input-feature chunks
    ndq = dq // 128           # 4 projection partition chunks
    nwin = s // SQ            # 4 q windows
    nsc = s // 128            # 16 s chunks

    nc = bass.Bass()

    xq = nc.declare_dram_parameter("xq_t", [e, s], F16, isOutput=False)
    xk = nc.declare_dram_parameter("xk_t", [e, s], F16, isOutput=False)
    xv = nc.declare_dram_parameter("xv_t", [e, s], F16, isOutput=False)
    wqd = nc.declare_dram_parameter("wq_t", [e, dq], F16, isOutput=False)
    wkd = nc.declare_dram_parameter("wk_t", [e, dq], F16, isOutput=False)
    wvd = nc.declare_dram_parameter("wv_t", [e, dq], F16, isOutput=False)
    wod = nc.declare_dram_parameter("wo_t", [dq, e], F16, isOutput=False)
    # packed constants: [bq(4) | bk(4) | bv_b(512)] f32, [ident | crossmask] f16
    cfd = nc.declare_dram_parameter("consts_f32", [128, 2 * ndq + dq], F32,
                                    isOutput=False)
    chd = nc.declare_dram_parameter("consts_f16", [128, 256], F16,
                                    isOutput=False)
    out = nc.declare_dram_parameter("out", [s, e], F16, isOutput=True)
    if debug:
        dbg_q = nc.declare_dram_parameter("dbg_q", [dq, s], F16, isOutput=True)
        dbg_k = nc.declare_dram_parameter("dbg_k", [dq, s], F16, isOutput=True)
        dbg_v = nc.declare_dram_parameter(
            "dbg_v", [s, hpc * (d + 1)], BF16, isOutput=True
        )
        dbg_at = nc.declare_dram_parameter("dbg_at", [s, dq], F16, isOutput=True)
        dbg_pt = nc.declare_dram_parameter("dbg_pt", [128, 17408], BF16,
                                           isOutput=True)
        dbg_rc = nc.declare_dram_parameter("dbg_rc", [128, 16], F32,
                                           isOutput=True)
        dbg_off = [0]

    with tile.TileContext(nc) as tc, contextlib.ExitStack() as ctx:
        pers = ctx.enter_context(tc.tile_pool(name="pers", bufs=1))
        xpool = ctx.enter_context(tc.tile_pool(name="xp", bufs=3))
        ppool = ctx.enter_context(tc.tile_pool(name="ppl", bufs=4))
        atn = ctx.enter_context(tc.tile_pool(name="atn", bufs=4))
        att = ctx.enter_context(tc.tile_pool(name="att", bufs=4))
        nrm = ctx.enter_context(tc.tile_pool(name="nrm", bufs=4))
        opool = ctx.enter_context(tc.tile_pool(name="opl", bufs=3))
        pp = ctx.enter_context(tc.tile_pool(name="pp", bufs=2, space="PSUM"))
        sp = ctx.enter_context(tc.tile_pool(name="sp", bufs=2, space="PSUM"))
        vp = ctx.enter_context(tc.tile_pool(name="vp", bufs=2, space="PSUM"))

        # ---- persistent tensors ----
        cf_sb = pers.tile([128, 2 * ndq + dq], F32, name="cf_sb")
        ch_sb = pers.tile([128, 256], F16, name="ch_sb")
        bq_sb = cf_sb[:, 0:ndq]
        bk_sb = cf_sb[:, ndq : 2 * ndq]
        bv_sb = cf_sb[:, 2 * ndq : 2 * ndq + dq]
        id_sb = ch_sb[:, 0:128]
        mk_sb = ch_sb[:, 128:256]
        q_sb = [
            [pers.tile([128, SQ], F16, name=f"q_sb{c}_{w}") for w in range(nwin)]
            for c in range(ndq)
        ]
        k_sb = [
            [pers.tile([128, SQ], F16, name=f"k_sb{c}_{w}") for w in range(nwin)]
            for c in range(ndq)
        ]
        v_sb = [
            pers.tile([128, hpc * (d + 1)], BF16, name=f"v_sb{i}")
            for i in range(nsc)
        ]
        wq_sb = pers.tile([128, nec * dq], F16, name="wq_sb")
        wk_sb = pers.tile([128, nec * dq], F16, name="wk_sb")
        wv_sb = pers.tile([128, nec * dq], F16, name="wv_sb")
        wo_sb = pers.tile([128, ndq * e], F16, name="wo_sb")

        # ---- DMA helpers (SP engine -> one HWDGE queue, program order) ----
        def load_w_part(wt, dst, part, nparts=2):
            # e-chunk group `part` of [e, dq] -> dst cols
            g = nec // nparts
            src = wt.rearrange("(n p) m -> p n m", p=128)
            nc.sync.dma_start(
                out=dst.rearrange("p (n m) -> p n m", m=dq)[
                    :, part * g : (part + 1) * g, :
                ],
                in_=src[:, part * g : (part + 1) * g, :],
            )

        def load_x_slab(xt, dst, sb, part=None, nparts=2):
            # dst: [128, nec*512] tile; cols [sb*512,(sb+1)*512) of [e, s]
            src = xt.rearrange("(n p) m -> p n m", p=128)
            d3 = dst.rearrange("p (n m) -> p n m", m=SQ)
            if part is None:
                nc.sync.dma_start(
                    out=d3[:, :, :],
                    in_=src[:, :, sb * SQ : (sb + 1) * SQ],
                )
            else:
                g = nec // nparts
                nc.sync.dma_start(
                    out=d3[:, part * g : (part + 1) * g, :],
                    in_=src[:, part * g : (part + 1) * g,
                            sb * SQ : (sb + 1) * SQ],
                )



        x_t = {}  # (tensor, sb) -> slab tile
        for t, xd in (("q", xq), ("k", xk), ("v", xv)):
            x_t[t, 0] = xpool.tile([128, nec * SQ], F16, tag=f"x{t}",
                                   name=f"x{t}0", bufs=3)
        # slab 0 interleaved with weight pieces for earliest unblock;
        # wq/xq0 in quarters so the first projection matmuls start ASAP
        for part in range(4):
            load_w_part(wqd, wq_sb, part, nparts=4)
            load_x_slab(xq, x_t["q", 0], 0, part=part, nparts=4)
        # packed constants (biases for the first bias-add, mask for h0 scores)
        nc.sync.dma_start(out=cf_sb[:, :], in_=cfd[:, :])
        nc.sync.dma_start(out=ch_sb[:, :], in_=chd[:, :])
        load_w_part(wkd, wk_sb, 0)
        load_x_slab(xk, x_t["k", 0], 0, part=0)
        load_w_part(wkd, wk_sb, 1)
        load_x_slab(xk, x_t["k", 0], 0, part=1)
        load_w_part(wvd, wv_sb, 0)
        load_x_slab(xv, x_t["v", 0], 0, part=0)
        load_w_part(wvd, wv_sb, 1)
        load_x_slab(xv, x_t["v", 0], 0, part=1)
        x_t["q", 1] = xpool.tile([128, nec * SQ], F16, tag="xq",
                                 name="xq1", bufs=3)
        load_x_slab(xq, x_t["q", 1], 1)
        for sb in range(1, nwin):
            for t, xd in (("q", xq), ("k", xk), ("v", xv)):
                if (t, sb) in x_t:
                    continue
                x_t[t, sb] = xpool.tile([128, nec * SQ], F16, tag=f"x{t}",
                                        name=f"x{t}{sb}", bufs=3)
                load_x_slab(xd, x_t[t, sb], sb)
            if sb == 1:
                nc.sync.dma_start(
                    out=wo_sb.rearrange("p (n m) -> p n m", m=e),
                    in_=wod.rearrange("(n p) m -> p n m", p=128),
                )

        # ones columns of v_sb, once, on the idle gpsimd engine
        for i in range(nsc):
            v3 = v_sb[i].rearrange("p (h t) -> p h t", t=d + 1)
            nc.gpsimd.memset(v3[:, :, d], 1.0)

        # ---- compute unit generators ----
        def w3(wt):
            return wt.rearrange("p (n m) -> p n m", m=dq)

        open_ps = {}

        def proj_qk_phase(w_sb_t, xt, dst, bias, sb, c, phase):
            """Half-contraction phase of a q/k projection unit. Phase 0
            allocates the psum tile and contracts ec 0..3; phase 1 finishes
            ec 4..7 and applies the bias. Between a unit's phases at most one
            other pp allocation may occur (pp bufs=2)."""
            key = ("qk", xt, sb, c)
            if phase == 0:
                ps = pp.tile([128, SQ], F32, tag="pp", name="ps_pj")
                open_ps[key] = ps
                ecs = range(0, nec // 2)
            else:
                ps = open_ps.pop(key)
                ecs = range(nec // 2, nec)
            for ec in ecs:
                nc.tensor.matmul(
                    ps[:, :],
                    w3(w_sb_t)[:, ec, c * 128 : (c + 1) * 128],
                    x_t[xt, sb][:, ec * SQ : (ec + 1) * SQ],
                    start=(ec == 0),
                    stop=(ec == nec - 1),
                )
            pe_rows(nec * SQ // 2)
            if phase == 1:
                nc.vector.tensor_scalar_add(
                    dst[c][sb][:, :], ps[:, :], bias[:, c : c + 1]
                )

        def proj_v_phase(sb, ii, phase):
            key = ("v", sb, ii)
            if phase == 0:
                ps = pp.tile([128, dq], F32, tag="pp", name="ps_v")
                open_ps[key] = ps
                ecs = range(0, nec // 2)
            else:
                ps = open_ps.pop(key)
                ecs = range(nec // 2, nec)
            wv_ = w3(wv_sb)
            for ec in ecs:
                nc.tensor.matmul(
                    ps[:, :],
                    x_t["v", sb][:, ec * SQ + ii * 128 : ec * SQ + ii * 128 + 128],
                    wv_[:, ec, :],
                    start=(ec == 0),
                    stop=(ec == nec - 1),
                )
            pe_rows(nec * SQ // 2)
            if phase == 1:
                i = sb * 4 + ii
                v3 = v_sb[i].rearrange("p (h t) -> p h t", t=d + 1)
                nc.vector.tensor_add(
                    v3[:, :, 0:d],
                    ps[:, :].rearrange("p (h t) -> p h t", t=d),
                    bv_sb[:, :].rearrange("p (h t) -> p h t", t=d),
                )

        def proj_qk_unit(w_sb_t, xt, dst, bias, sb, c):
            """One [128,512] slab-column of a transposed projection."""
            ps = pp.tile([128, SQ], F32, tag="pp", name="ps_pj")
            wv_ = w3(w_sb_t)
            for ec in range(nec):
                nc.tensor.matmul(
                    ps[:, :],
                    wv_[:, ec, c * 128 : (c + 1) * 128],
                    x_t[xt, sb][:, ec * SQ : (ec + 1) * SQ],
                    start=(ec == 0),
                    stop=(ec == nec - 1),
                )
            nc.vector.tensor_scalar_add(
                dst[c][sb][:, :], ps[:, :], bias[:, c : c + 1]
            )

        def proj_v_unit(sb, ii):
            """One [128(s), dq] natural-layout V chunk (i = sb*4+ii)."""
            i = sb * 4 + ii
            ps = pp.tile([128, dq], F32, tag="pp", name="ps_v")
            wv_ = w3(wv_sb)
            for ec in range(nec):
                nc.tensor.matmul(
                    ps[:, :],
                    x_t["v", sb][:, ec * SQ + ii * 128 : ec * SQ + ii * 128 + 128],
                    wv_[:, ec, :],
                    start=(ec == 0),
                    stop=(ec == nec - 1),
                )
            v3 = v_sb[i].rearrange("p (h t) -> p h t", t=d + 1)
            nc.vector.tensor_add(
                v3[:, :, 0:d],
                ps[:, :].rearrange("p (h t) -> p h t", t=d),
                bv_sb[:, :].rearrange("p (h t) -> p h t", t=d),
            )

        # static PE/ACT occupancy estimate driving filler insertion
        eng_ns = {"pe": 0.0, "act": 0.0}

        def pe_rows(n):
            eng_ns["pe"] += n * 0.4167

        def act_cols(n):
            eng_ns["act"] += 1.326 * (n * 0.8333 + 185.0)  # tuned filler bias

        def attention_head(qb, h, att_tiles, pre_last_cb=None,
                           act_norm=False):
            """scores+exp+PV+normalize for one (window, head).

            Generator: yields after each score-group / PV emission so the
            driver can interleave PE filler while ACT churns through exps.
            pre_last_cb: emitted right after the last score group (tail
            shortening for the final head). act_norm: do half the normalize
            multiplies on ACT (only sensible when ACT is idle afterwards).
            """
            c, hp = h // 2, (h % 2) * 64
            nkb = 4 * qb + 4 if causal else nsc
            # segments: (kb, qstart_global, width)
            segs = []
            for kb in range(nkb):
                if causal and kb >= 4 * qb:
                    qs = kb * 128
                else:
                    qs = qb * SQ
                segs.append((kb, qs, (qb + 1) * SQ - qs))
            # greedy-pack into exp groups of width <= GW
            groups, cur, curw = [], [], 0
            for seg in segs:
                if curw + seg[2] > GW:
                    groups.append(cur)
                    cur, curw = [], 0
                cur.append(seg)
                curw += seg[2]
            if cur:
                groups.append(cur)
            if len(groups) > 1:
                # smallest group first: its short exp lands while ACT still
                # drains the previous head, instead of bubbling at head end
                groups = groups[-2:] + groups[:-2]

            vpa = vp.tile([128, 4 * (d + 1)], F32, tag="vo", name="vpa")
            last_kb = nkb - 1
            npv = sum(
                1 for kb in range(nkb) for qcl in range(4)
                if not (causal and 4 * qb + qcl < kb))
            pv_n = [0]

            def emit_scores(grp):
                gw = sum(g[2] for g in grp)
                scp = sp.tile([128, GW], F32, tag="sc", name="scp")
                off = 0
                for kb, qs, w in grp:
                    ks = k_sb[c][kb // 4][hp : hp + d,
                                          (kb % 4) * 128 : (kb % 4) * 128 + 128]
                    qw_ = q_sb[c][qs // SQ]
                    if causal and kb >= 4 * qb:
                        # additive mask for the diagonal-crossing sub-block
                        nc.tensor.matmul(scp[:, off : off + 128], id_sb[:, :],
                                         mk_sb[:, :], start=True, stop=False)
                        nc.tensor.matmul(
                            scp[:, off : off + 128], ks,
                            qw_[hp : hp + d, qs % SQ : qs % SQ + 128],
                            start=False, stop=True,
                        )
                        pe_rows(256)
                        if w > 128:
                            nc.tensor.matmul(
                                scp[:, off + 128 : off + w], ks,
                                qw_[hp : hp + d, qs % SQ + 128 : qs % SQ + w],
                                start=True, stop=True,
                            )
                            pe_rows(w - 128)
                    else:
                        nc.tensor.matmul(
                            scp[:, off : off + w], ks,
                            qw_[hp : hp + d, qs % SQ : qs % SQ + w],
                            start=True, stop=True,
                        )
                        pe_rows(w)
                    off += w
                pt = ppool.tile([128, GW], BF16, tag="pt", name="pt")
                nc.scalar.activation(
                    pt[:, 0:gw], scp[:, 0:gw], AF.Exp,
                    scale=float(1.0 / np.sqrt(d)),
                )
                act_cols(gw)
                if debug and h == 0:
                    nc.sync.dma_start(
                        out=dbg_pt[:, dbg_off[0] : dbg_off[0] + gw],
                        in_=pt[:, 0:gw])
                    dbg_off[0] += gw
                return pt

            def emit_pv(grp, pt):
                # One psum accumulation group for the whole vpa bank: a
                # start marks the full 2KB zero-region pending-zero, so only
                # the first matmul may carry start and only the last stop;
                # each sub-region auto-initializes on its first write.
                off = 0
                for kb, qs, w in grp:
                    for qcl in range(4):
                        qg = 4 * qb + qcl           # global q chunk
                        if causal and qg < kb:
                            continue                 # fully masked block
                        boff = off + qcl * 128 + qb * SQ - qs
                        nc.tensor.matmul(
                            vpa[:, qcl * (d + 1) : (qcl + 1) * (d + 1)],
                            pt[:, boff : boff + 128],
                            v_sb[kb][:, h * (d + 1) : (h + 1) * (d + 1)],
                            start=(pv_n[0] == 0),
                            stop=(pv_n[0] == npv - 1),
                        )
                        pv_n[0] += 1
                        pe_rows(d + 1)
                    off += w

            # lag-1 software pipeline: scores g+1 overlaps exp g
            prev = None
            for gi, grp in enumerate(groups):
                pt = emit_scores(grp)
                if pre_last_cb is not None and gi == len(groups) - 1:
                    pre_last_cb()
                yield
                if prev is not None:
                    emit_pv(*prev)
                    yield
                prev = (grp, pt)
            emit_pv(*prev)

            v4 = vpa.rearrange("p (qc t) -> p qc t", t=d + 1)
            rcp = nrm.tile([128, 4], F32, tag="rcp", name="rcp")
            nc.vector.reciprocal(rcp[:, :], v4[:, :, d])
            if debug and h == 0:
                nc.sync.dma_start(out=dbg_rc[:, qb * 4 : qb * 4 + 4],
                                  in_=rcp[:, :])
            for qcl in range(4):
                if act_norm and qcl >= 2:
                    nc.scalar.activation(
                        att_tiles[qcl][:, h * d : (h + 1) * d],
                        v4[:, qcl, 0:d],
                        AF.Copy,
                        scale=rcp[:, qcl : qcl + 1],
                    )
                else:
                    nc.vector.tensor_scalar_mul(
                        att_tiles[qcl][:, h * d : (h + 1) * d],
                        v4[:, qcl, 0:d],
                        rcp[:, qcl : qcl + 1],
                    )

        def wo_transpose_unit(att_tiles, cc, at_store, copy_eng=None):
            """Transpose attn chunk cc (heads 2cc, 2cc+1) -> at_store[cc]."""
            tp = pp.tile([128, SQ], F16, tag="pp", name="tp")
            for qcl in range(4):
                nc.tensor.transpose(
                    tp[:, qcl * 128 : (qcl + 1) * 128],
                    att_tiles[qcl][:, cc * 128 : (cc + 1) * 128],
                    id_sb[:, :],
                )
                pe_rows(128)
            at_ = att.tile([128, SQ], F16, tag=f"at{cc}", name="at_")
            if copy_eng is None:
                nc.vector.tensor_copy(at_[:, :], tp[:, :])
            else:
                copy_eng.copy(at_[:, :], tp[:, :])
            at_store[cc] = at_

        def wo_matmul_unit(at_store, qb, i, copy_eng=None):
            """Output projection + store for s-chunk i of window qb."""
            wo3 = wo_sb.rearrange("p (n m) -> p n m", m=e)
            ot = opool.tile([128, e], F16, tag="ot", name="ot")
            si = qb * 4 + i
            for ob in range(2):
                ps = pp.tile([128, 512], F32, tag="pp", name="ps_o")
                for cc in range(ndq):
                    nc.tensor.matmul(
                        ps[:, :],
                        at_store[cc][:, i * 128 : (i + 1) * 128],
                        wo3[:, cc, ob * 512 : (ob + 1) * 512],
                        start=(cc == 0),
                        stop=(cc == ndq - 1),
                    )
                    pe_rows(512)
                if copy_eng is None:
                    nc.vector.tensor_copy(
                        ot[:, ob * 512 : (ob + 1) * 512], ps[:, :])
                else:
                    copy_eng.copy(ot[:, ob * 512 : (ob + 1) * 512], ps[:, :])
                nc.sync.dma_start(
                    out=out[si * 128 : (si + 1) * 128,
                            ob * 512 : (ob + 1) * 512],
                    in_=ot[:, ob * 512 : (ob + 1) * 512],
                )

        # ---- projection queue, deadline-ordered ----
        # Per window sb: q/k chunk c due just before head 2c; v slab due
        # during head 0's score groups (its diag PV needs it). Deadline key:
        # (sb, h_due) with v at h_due=1 (forced explicitly at h0's yields).
        proj_queue = []
        for sb in range(nwin):
            proj_queue.append((sb, 0, "q", sb, 0))
            proj_queue.append((sb, 0, "k", sb, 0))
            for ii in range(4):
                proj_queue.append((sb, 1, "v", sb, ii))
            for c in range(1, ndq):
                proj_queue.append((sb, 2 * c, "q", sb, c))
                proj_queue.append((sb, 2 * c, "k", sb, c))
        wo_queue = []

        def emit_proj_unit():
            _, _, kind, sb, j = proj_queue.pop(0)
            if kind == "q":
                proj_qk_unit(wq_sb, "q", q_sb, bq_sb, sb, j)
            elif kind == "k":
                proj_qk_unit(wk_sb, "k", k_sb, bk_sb, sb, j)
            else:
                proj_v_unit(sb, j)
            pe_rows(nec * SQ)

        def balance_filler(qb):
            # Keep PE fed while ACT is the pacing engine — but don't consume
            # units whose deadline lets them fill a FUTURE window's ACT-bound
            # stretch (they are the only legal filler there).
            if open_ps:
                return  # a phase-split unit owns a pp slot; don't rotate pp
            while eng_ns["pe"] < eng_ns["act"]:
                if proj_queue and (
                    (proj_queue[0][0], proj_queue[0][1]) < (qb + 1, 1)
                ):
                    emit_proj_unit()
                elif wo_queue:
                    wo_queue.pop(0)()
                else:
                    return

        def force_due(qb, h):
            while proj_queue and (proj_queue[0][0], proj_queue[0][1]) <= (qb, h):
                emit_proj_unit()

        def wo_full(qb, att_tiles, last=False):
            at_store = [None] * ndq
            for cc in range(ndq):
                wo_transpose_unit(att_tiles, cc, at_store)
            if debug:
                for qcl in range(4):
                    nc.sync.dma_start(
                        out=dbg_at[(qb * 4 + qcl) * 128 :
                                   (qb * 4 + qcl + 1) * 128, :],
                        in_=att_tiles[qcl][:, :],
                    )
            for i in range(4):
                # final window: ACT is idle by now, DVE is not
                wo_matmul_unit(at_store, qb, i,
                               copy_eng=nc.scalar if last else None)

        # ---- emission ----
        # bootstrap: the startup is DMA-bound; emit phase-split units in
        # A,A,B,B order so every unit's first contraction half runs while
        # the second DMA halves are still in flight
        boot = {("q", 0, 0), ("q", 0, 1), ("q", 0, 2), ("q", 0, 3),
                ("k", 0, 0), ("k", 0, 1), ("v", 0, 0), ("v", 0, 1),
                ("v", 0, 2), ("v", 0, 3)}
        for c0, c1 in ((0, 1), (2, 3)):
            proj_qk_phase(wq_sb, "q", q_sb, bq_sb, 0, c0, 0)
            proj_qk_phase(wq_sb, "q", q_sb, bq_sb, 0, c1, 0)
            proj_qk_phase(wq_sb, "q", q_sb, bq_sb, 0, c0, 1)
            proj_qk_phase(wq_sb, "q", q_sb, bq_sb, 0, c1, 1)
        proj_qk_phase(wk_sb, "k", k_sb, bk_sb, 0, 0, 0)
        proj_qk_phase(wk_sb, "k", k_sb, bk_sb, 0, 1, 0)
        proj_qk_phase(wk_sb, "k", k_sb, bk_sb, 0, 0, 1)
        proj_qk_phase(wk_sb, "k", k_sb, bk_sb, 0, 1, 1)
        proj_queue = [u for u in proj_queue if (u[2], u[3], u[4]) not in boot]

        prev = None  # deferred (qb, att_tiles, at_store) for wo
        last_store = [None] * ndq
        for qb in range(nwin):
            att_tiles = [
                atn.tile([128, dq], F16, tag=f"an{qcl}", name=f"an{qcl}_{qb}")
                for qcl in range(4)
            ]
            for h in range(hpc):
                force_due(qb, h)
                if h == 6 and qb + 1 < nwin:
                    # pre-force next window's first q/k chunks: the boundary
                    # head's scores start with zero projection latency
                    force_due(qb + 1, 0)
                yi = 0
                for _ in attention_head(qb, h, att_tiles):
                    yi += 1
                    if h == 0 and qb == 0:
                        # window 0's v slab is still streaming in: run the
                        # first contraction halves while the rest arrives
                        if yi == 1:
                            proj_v_phase(0, 0, 0)
                            proj_v_phase(0, 1, 0)
                        elif yi == 2:
                            proj_v_phase(0, 0, 1)
                            proj_v_phase(0, 1, 1)
                        elif yi == 3:
                            proj_v_phase(0, 2, 0)
                            proj_v_phase(0, 3, 0)
                            proj_v_phase(0, 2, 1)
                            proj_v_phase(0, 3, 1)
                    elif h == 0 and yi <= 2:
                        # v slab for this window's diagonal, 2 units per yield
                        for _ in range(2):
                            if proj_queue and proj_queue[0][2] == "v" \
                                    and proj_queue[0][3] == qb:
                                emit_proj_unit()
                    balance_filler(qb)
            # defer this window's Wo into the balance queue: it is the only
            # PE work with no deadline, so it belongs in the late ACT-bound
            # holes (atn/att bufs=4 make any emission order inversion-free)
            pqb, ptiles, pstore = qb, att_tiles, [None] * ndq

            def mk_tr(ptiles=ptiles, pstore=pstore, pqb=pqb):
                for cc in range(ndq):
                    wo_transpose_unit(ptiles, cc, pstore)
                if debug:
                    for qcl in range(4):
                        nc.sync.dma_start(
                            out=dbg_at[(pqb * 4 + qcl) * 128 :
                                       (pqb * 4 + qcl + 1) * 128, :],
                            in_=ptiles[qcl][:, :],
                        )

            if qb < nwin - 1:
                wo_queue.append(mk_tr)
                for i in range(4):
                    wo_queue.append(
                        lambda st=pstore, w=pqb, j=i: wo_matmul_unit(st, w, j))
            else:
                prev = (qb, att_tiles)
        while proj_queue:
            emit_proj_unit()
        while wo_queue:
            wo_queue.pop(0)()
        wo_full(*prev, last=True)

        if debug:
            for c in range(ndq):
                for w in range(nwin):
                    cs = slice(c * 128, (c + 1) * 128)
                    ws = slice(w * SQ, (w + 1) * SQ)
                    nc.sync.dma_start(out=dbg_q[cs, ws], in_=q_sb[c][w][:, :])
                    nc.sync.dma_start(out=dbg_k[cs, ws], in_=k_sb[c][w][:, :])
            for i in range(nsc):
                nc.sync.dma_start(
                    out=dbg_v[i * 128 : (i + 1) * 128, :], in_=v_sb[i][:, :]
                )

    if split_waits:
        split_excess_waits(nc)
    return nc


def make_crossmask():
    kk = np.arange(128)[:, None]
    qq = np.arange(128)[None, :]
    return np.where(kk <= qq, 0.0, NEG).astype(np.float16)


def classify_mask(mask):
    m = np.asarray(mask).reshape(S, S)
    if np.array_equal(m, np.tril(np.ones((S, S), bool))):
        return "causal"
    if m.all():
        return "dense"
    return "generic"


def prep_core_inputs(query, key, value, Wq, bq, Wk, bk, Wv, bv, Wo, bo, mask):
    """Shard + lay out host-side numpy inputs for the 8 cores."""
    kind = classify_mask(mask)
    maps = []
    for core in range(NCORES):
        b, gi = core // NGROUPS, core % NGROUPS
        gs = slice(gi * DQ, (gi + 1) * DQ)
        im = {
            "xq_t": np.ascontiguousarray(
                np.asarray(query[b]).T.astype(np.float16)),
            "xk_t": np.ascontiguousarray(
                np.asarray(key[b]).T.astype(np.float16)),
            "xv_t": np.ascontiguousarray(
                np.asarray(value[b]).T.astype(np.float16)),
            "wq_t": np.ascontiguousarray(
                np.asarray(Wq)[gs, :].T.astype(np.float16)),
            "wk_t": np.ascontiguousarray(
                np.asarray(Wk)[gs, :].T.astype(np.float16)),
            "wv_t": np.ascontiguousarray(
                np.asarray(Wv)[gs, :].T.astype(np.float16)),
            "wo_t": np.ascontiguousarray(
                np.asarray(Wo)[:, gs].T.astype(np.float16)),
            "consts_f32": np.ascontiguousarray(np.concatenate([
                np.asarray(bq)[gs].astype(np.float32).reshape(-1, 128).T,
                np.asarray(bk)[gs].astype(np.float32).reshape(-1, 128).T,
                np.broadcast_to(
                    np.asarray(bv)[gs].astype(np.float32), (128, DQ)),
            ], axis=1)),
            "consts_f16": np.ascontiguousarray(np.concatenate([
                np.eye(128, dtype=np.float16), make_crossmask()
            ], axis=1)),
        }
        maps.append(im)
    return maps, kind


def make_runner(nc, n_cores=NCORES):
    """Build a reusable jitted SPMD executor for `nc` on cores 0..n_cores-1."""
    import jax
    from jax.experimental.shard_map import shard_map
    from jax.sharding import Mesh, PartitionSpec

    from concourse import bass2jax, mybir as _mybir

    bass2jax.install_neuronx_cc_hook()

    partition_name = (
        nc.partition_id_tensor.name if nc.partition_id_tensor else None
    )
    in_names, out_names, out_avals, zero_shapes = [], [], [], []
    for alloc in nc.m.functions[0].allocations:
        if not isinstance(alloc, _mybir.MemoryLocationSet):
            continue
        name = alloc.memorylocations[0].name
        if alloc.kind == "ExternalInput":
            if name != partition_name:
                in_names.append(name)
        elif alloc.kind == "ExternalOutput":
            out_names.append(name)
            shape = tuple(alloc.tensor_shape)
            dtype = _mybir.dt.np(alloc.dtype)
            out_avals.append(jax.core.ShapedArray(shape, dtype))
            zero_shapes.append((shape, dtype))
    n_params = len(in_names)
    all_in = list(in_names) + list(out_names)
    if partition_name is not None:
        all_in.append(partition_name)

    def _body(*args):
        operands = list(args)
        if partition_name is not None:
            operands.append(bass2jax.partition_id_tensor())
        outs = bass2jax._bass_exec_p.bind(
            *operands,
            out_avals=tuple(out_avals),
            in_names=tuple(all_in),
            out_names=tuple(out_names),
            lowering_input_output_aliases=(),
            sim_require_finite=True,
            sim_require_nnan=True,
            nc=nc,
        )
        return tuple(outs)

    devices = jax.devices()[:n_cores]
    assert len(devices) == n_cores
    mesh = Mesh(np.asarray(devices), ("core",))
    in_specs = (PartitionSpec("core"),) * (n_params + len(out_names))
    out_specs = (PartitionSpec("core"),) * len(out_names)
    sharded = jax.jit(
        shard_map(
            _body,
            mesh=mesh,
            in_specs=in_specs,
            out_specs=out_specs,
            check_rep=False,
        ),
        keep_unused=True,
    )
    zeros = [
        np.zeros((n_cores * sh[0], *sh[1:]), dt) for sh, dt in zero_shapes
    ]

    def concat_inputs(in_maps):
        return [
            np.concatenate(
                [np.asarray(in_maps[c][n]) for c in range(n_cores)], axis=0
            )
            for n in in_names
        ]

    def run(in_maps):
        out_arrs = sharded(*concat_inputs(in_maps), *zeros)
        return [
            {
                name: np.asarray(out_arrs[i]).reshape(
                    n_cores, *out_avals[i].shape
                )[c]
                for i, name in enumerate(out_names)
            }
            for c in range(n_cores)
        ]

    run.sharded = sharded
    run.concat_inputs = concat_inputs
    run.zeros = zeros
    run.out_names = out_names
    run.out_avals = out_avals
    return run


_CACHE = {}


def get_runner(kind="causal"):
    if kind not in _CACHE:
        nc = build_kernel(causal=(kind == "causal"))
        _CACHE[kind] = make_runner(nc)
    return _CACHE[kind]


def _numpy_reference(query, key, value, Wq, bq, Wk, bk, Wv, bv, Wo, bo, mask):
    q = (query @ Wq.T + bq).reshape(B, S, H, D).transpose(0, 2, 1, 3)
    k = (key @ Wk.T + bk).reshape(B, S, H, D).transpose(0, 2, 1, 3)
    v = (value @ Wv.T + bv).reshape(B, S, H, D).transpose(0, 2, 1, 3)
    sc = np.einsum("bhqd,bhkd->bhqk", q, k) / np.sqrt(D)
    sc = np.where(np.asarray(mask).reshape(1, 1, S, S), sc, -np.inf)
    sc -= sc.max(axis=-1, keepdims=True)
    p = np.exp(sc)
    p /= p.sum(axis=-1, keepdims=True)
    o = np.einsum("bhqk,bhkd->bhqd", p, v)
    o = o.transpose(0, 2, 1, 3).reshape(B, S, E)
    return o @ Wo.T + bo


def kernel(**inputs) -> np.ndarray:
    kind = classify_mask(inputs["mask"])
    if kind == "generic":
        fp = {k: np.asarray(v, np.float32) for k, v in inputs.items()
              if k != "mask"}
        return _numpy_reference(mask=inputs["mask"], **fp).astype(np.float32)
    in_maps, kind = prep_core_inputs(**inputs)
    run = get_runner(kind)
    results = run(in_maps)
    bo = np.asarray(inputs["bo"], dtype=np.float32)
    out = np.empty((B, S, E), dtype=np.float32)
    for b in range(B):
        acc = results[b * NGROUPS]["out"].astype(np.float32)
        for gi in range(1, NGROUPS):
            acc = acc + results[b * NGROUPS + gi]["out"].astype(np.float32)
        out[b] = acc + bo[None, :]
    return out
